# revision 49
# baseline (speedup 1.0000x reference)
"""Trainium2 Bass kernel for nn_LinearTemporalDiffusionTransformerDecoderLayer.

v3 (on top of v2's pass-restructured phases / streamed K/V einsum1):
 - fp8 (e4m3, DoubleRow) FFN GEMMs with x64 weight pre-scale (halves the
   biggest PE load; rel_err 0.0137 vs the 2e-2 gate, deterministic inputs)
 - table-free DVE Newton-rsqrt for every LayerNorm (no natural_log ACT
   table ping-pong) + scheduler-only ordering edges that group exp/silu/gelu
   activations per phase: 69 -> 7 ACT table loads (~2.7us each on HW)
 - software-pipelined batch loop: stylize_pre(b-1)'s serial DVE stats chains
   are emitted after batch b's PE-heavy attention work, so PE stays fed
 - cross-phase LN stats hoisting (CA x-stats emitted right behind each SA
   residual update)
 - weight loads via gpsimd cast-on-DMA (f32->bf16) with parallel whole-row
   DMAs; scaled/fp8 weights convert from a bf16 stage (no f32 stage ping-pong)
 - bf16 residual stream (halves x DMA + SBUF; final residual evicts f32
   straight to the output DMA so the last rounding is not taken)

TimelineSim cost model: 602us/core vs 910us for v2 (HW slope-timing is
tunnel-noise-limited; same-window A/B showed 876us vs ~1420us).

Self-contained: builds and compiles an 8-core SPMD Bass program on first call,
shards the batch dimension (B=32 -> 8 cores x 4), executes via PJRT, and
reassembles the full output.  kernel(**inputs) -> np.ndarray [32, 1024, 512].
"""
import sys
for _p in ("/opt/trn_rl_repo",):
    if _p not in sys.path:
        sys.path.insert(0, _p)
import numpy as np
import jax
import concourse.bass as bass
import concourse.bacc as bacc
import concourse.tile as tile
from concourse import mybir
from concourse.tile_rust import add_dep_helper
from concourse.masks import make_identity
from contextlib import ExitStack

dt = mybir.dt
F32, BF16 = dt.float32, dt.bfloat16
FP8 = dt.float8e4
AF = mybir.ActivationFunctionType
OP = mybir.AluOpType
DR = mybir.MatmulPerfMode.DoubleRow
P = 128
T, D, H, DH = 1024, 512, 8, 64
NCTX, L, TE, FF = 256, 512, 512, 2048
NT, FT = T // P, D // P          # 8 token tiles, 4 feature tiles
NTC = NCTX // P                  # 2 ctx tiles
EPS = 1e-5
MASK_NEG = -80.0
WS = 64.0                        # fp8 weight pre-scale
RWS = 1.0 / WS

PARAM_SHAPES = {
    "sa_norm_g": (D,), "sa_norm_b": (D,),
    "sa_q_w": (D, D), "sa_q_b": (D,), "sa_k_w": (D, D), "sa_k_b": (D,),
    "sa_v_w": (D, D), "sa_v_b": (D,),
    "sa_emb_w": (TE, 2 * D), "sa_emb_b": (2 * D,),
    "sa_pnorm_g": (D,), "sa_pnorm_b": (D,),
    "sa_out_w": (D, D), "sa_out_b": (D,),
    "ca_norm_g": (D,), "ca_norm_b": (D,), "ca_tnorm_g": (L,), "ca_tnorm_b": (L,),
    "ca_q_w": (D, D), "ca_q_b": (D,), "ca_k_w": (L, D), "ca_k_b": (D,),
    "ca_v_w": (L, D), "ca_v_b": (D,),
    "ca_emb_w": (TE, 2 * D), "ca_emb_b": (2 * D,),
    "ca_pnorm_g": (D,), "ca_pnorm_b": (D,),
    "ca_out_w": (D, D), "ca_out_b": (D,),
    "ffn_w1": (D, FF), "ffn_b1": (FF,), "ffn_w2": (FF, D), "ffn_b2": (D,),
    "ffn_emb_w": (TE, 2 * D), "ffn_emb_b": (2 * D,),
    "ffn_pnorm_g": (D,), "ffn_pnorm_b": (D,),
    "ffn_out_w": (D, D), "ffn_out_b": (D,),
}


def build(n_batch=4, taps=(), fp8_ffn=False, fp8_qkv=False, fp8_out=False,
          act_group=True, pre_dve=True):
    BL = n_batch
    nc = bacc.Bacc(None, target_bir_lowering=False, debug=False)
    tap_tensors = {}

    x_d = nc.declare_dram_parameter("x", [BL, T, D], F32, isOutput=False)
    xf_d = nc.declare_dram_parameter("xf", [BL, NCTX, L], F32, isOutput=False)
    emb_d = nc.declare_dram_parameter("emb", [BL, TE], F32, isOutput=False)
    mask_d = nc.declare_dram_parameter("src_mask", [BL, T, 1], F32, isOutput=False)
    W = {}
    for name, shp in PARAM_SHAPES.items():
        W[name] = nc.declare_dram_parameter(name, list(shp), F32, isOutput=False)
    out_d = nc.declare_dram_parameter("out", [BL, T, D], F32, isOutput=True)

    with tile.TileContext(nc) as tc, ExitStack() as root:
        const = root.enter_context(tc.tile_pool(name="const", bufs=1))
        small = root.enter_context(tc.tile_pool(name="small", bufs=2))

        xpool = root.enter_context(tc.tile_pool(name="xpool", bufs=1))
        zpool = root.enter_context(tc.tile_pool(name="zpool", bufs=2))
        fmp = root.enter_context(tc.tile_pool(name="fmp", bufs=2))
        qpool = root.enter_context(tc.tile_pool(name="qpool", bufs=2))
        kvp = root.enter_context(tc.tile_pool(name="kvp", bufs=2))
        ypool = root.enter_context(tc.tile_pool(name="ypool", bufs=2))
        zfmp = root.enter_context(tc.tile_pool(name="zfmp", bufs=4))
        sfmp = root.enter_context(tc.tile_pool(name="sfmp", bufs=1))
        abp = root.enter_context(tc.tile_pool(name="abp", bufs=2))
        vecp = root.enter_context(tc.tile_pool(name="vecp", bufs=1))
        xop = root.enter_context(tc.tile_pool(name="xop", bufs=3))
        ps = {}

        # ---- ACT table grouping: scheduler-only ordering edges keep the
        # scalar engine's table-switching funcs (exp/silu/gelu) from
        # interleaving across phases (each switch = ~2.7us table reload) ----
        act_seq = {"prev": None, "cur": []}

        def act_reg(bi):
            if not act_group:
                return bi
            if act_seq["prev"] is not None:
                add_dep_helper(bi.ins, act_seq["prev"], sync=False,
                               reason="act-table-group")
            act_seq["cur"].append(bi)
            return bi

        def act_break():
            if not act_group or not act_seq["cur"]:
                return
            j = nc.scalar.nop()
            for bi in act_seq["cur"]:
                add_dep_helper(j.ins, bi.ins, sync=False,
                               reason="act-table-junction")
            act_seq["prev"] = j.ins
            act_seq["cur"] = []

        def tap(name, ap):
            if name in taps and name not in tap_tensors:
                tdn = "tap_" + name
                td = nc.declare_dram_parameter(tdn, list(ap.shape),
                                               ap.dtype, isOutput=True)
                nc.sync.dma_start(out=td[:], in_=ap)
                tap_tensors[name] = (tdn, tuple(ap.shape))

        # ---------------- constants ----------------
        ident_bf = const.tile([P, P], BF16)
        make_identity(nc, ident_bf)
        ident_f = const.tile([P, P], F32)
        make_identity(nc, ident_f)
        ones_row_bf = const.tile([1, P], BF16)       # K=1 lhsT for row broadcast
        nc.vector.memset(ones_row_bf, 1.0)
        sel_bf = const.tile([P, 2], BF16)            # head-pair selector
        nc.vector.memset(sel_bf, 0.0)
        nc.vector.memset(sel_bf[0:64, 0:1], 1.0)
        nc.vector.memset(sel_bf[64:128, 1:2], 1.0)
        eps_t = const.tile([P, 1], F32)
        nc.vector.memset(eps_t, EPS)

        # ---------------- load helpers ----------------
        def load_fm_vec(pool, ap1d, n=D, tag=None):
            nkt = n // P
            t = pool.tile([P, nkt], F32, tag=tag)
            nc.sync.dma_start(out=t, in_=ap1d.rearrange("(k p) -> p k", p=P))
            return t

        def load_row(pool, ap1d, n, tag=None, bufs=None):
            kw = {} if bufs is None else {"bufs": bufs}
            t = pool.tile([1, n], F32, tag=tag, **kw)
            nc.sync.dma_start(out=t, in_=ap1d.rearrange("(a n) -> a n", a=1))
            return t

        def load_w_fm(pool, wap, kdim, ndim, gfm=None, tag="w", dtype=BF16,
                      scale=None, paired=False, bufs=None):
            """Load weight [kdim, ndim] -> list of FM tiles.

            paired=False: kdim//P tiles of [P, ndim].
            paired=True: kdim//(2P) tiles of [P, 2, ndim] (DoubleRow k-pairs).
            gfm: per-row (input-feature) scale column tile [P, kdim//P].
            scale: extra scalar premultiplier (fp8 range scaling).
            """
            nkt = kdim // P
            tiles = []
            kw = {} if bufs is None else {"bufs": bufs}
            for kt in range(nkt):
                if paired:
                    if kt % 2 == 0:
                        wt = pool.tile([P, 2, ndim], dtype,
                                       tag=f"{tag}{kt // 2}", **kw)
                        tiles.append(wt)
                    dst = tiles[-1][:, kt % 2, :]
                else:
                    wt = pool.tile([P, ndim], dtype, tag=f"{tag}{kt}", **kw)
                    tiles.append(wt)
                    dst = wt
                if gfm is None and scale is None and dtype == BF16:
                    # cast-on-DMA (software DGE): skip the stage+convert hop
                    nc.gpsimd.dma_start(out=dst,
                                        in_=wap[kt * P:(kt + 1) * P, :])
                    continue
                # cast-on-DMA to a bf16 stage, then scale/convert in SBUF.
                # One whole-row DMA per k-tile: parallel DMA fan-out, no
                # per-chunk stage ping-pong.
                stg = pool.tile([P, ndim], BF16, tag=f"wstg{ndim}",
                                name="wstg", bufs=2)
                nc.gpsimd.dma_start(out=stg, in_=wap[kt * P:(kt + 1) * P, :])
                for c0 in range(0, ndim, 1024):
                    cw = min(1024, ndim - c0)
                    eng = nc.vector if (kt + c0 // 1024) % 2 == 0 else nc.scalar
                    if gfm is not None and scale is not None:
                        gs = small.tile([P, 1], F32, tag="gsc")
                        nc.vector.tensor_scalar(out=gs, in0=gfm[:, kt:kt + 1],
                                                scalar1=scale, scalar2=None,
                                                op0=OP.mult)
                        nc.vector.tensor_scalar(out=dst[:, c0:c0 + cw],
                                                in0=stg[:, c0:c0 + cw],
                                                scalar1=gs,
                                                scalar2=None, op0=OP.mult)
                    elif gfm is not None:
                        r = (kt + c0 // 1024) % 3
                        if r == 0:
                            nc.vector.tensor_scalar(out=dst[:, c0:c0 + cw],
                                                    in0=stg[:, c0:c0 + cw],
                                                    scalar1=gfm[:, kt:kt + 1],
                                                    scalar2=None, op0=OP.mult)
                        elif r == 1:
                            nc.scalar.activation(out=dst[:, c0:c0 + cw],
                                                 in_=stg[:, c0:c0 + cw],
                                                 func=AF.Copy,
                                                 scale=gfm[:, kt:kt + 1])
                        else:
                            nc.gpsimd.tensor_scalar(out=dst[:, c0:c0 + cw],
                                                    in0=stg[:, c0:c0 + cw],
                                                    scalar1=gfm[:, kt:kt + 1],
                                                    scalar2=None, op0=OP.mult)
                    elif scale is not None:
                        if eng is nc.vector:
                            nc.vector.tensor_scalar(out=dst[:, c0:c0 + cw],
                                                    in0=stg[:, c0:c0 + cw],
                                                    scalar1=scale, scalar2=None,
                                                    op0=OP.mult)
                        else:
                            nc.scalar.activation(out=dst[:, c0:c0 + cw],
                                                 in_=stg[:, c0:c0 + cw],
                                                 func=AF.Copy,
                                                 scale=scale)
                    else:
                        if eng is nc.vector:
                            nc.vector.tensor_copy(out=dst[:, c0:c0 + cw],
                                                  in_=stg[:, c0:c0 + cw])
                        else:
                            nc.scalar.copy(out=dst[:, c0:c0 + cw],
                                           in_=stg[:, c0:c0 + cw])
            return tiles

        def fold_bias(pool, w_tiles, blnbf, bproj_row, ndim, tag, wscale=1.0,
                      paired=False, want_bf=True):
            """bias' = b_ln @ W' + b_proj (undoing wscale) -> [1,ndim] f32/bf16."""
            pst = ps["S"].tile([1, ndim], F32, tag="sm", name="foldps")
            if paired:
                flat = []
                for wt in w_tiles:
                    flat.append(wt[:, 0, :])
                    flat.append(wt[:, 1, :])
            else:
                flat = w_tiles
            nk = len(flat)
            for kt, wt in enumerate(flat):
                nc.tensor.matmul(out=pst, lhsT=blnbf[:, kt:kt + 1], rhs=wt,
                                 start=(kt == 0), stop=(kt == nk - 1))
            row = pool.tile([1, ndim], F32, tag="foldtmp", bufs=1)
            if wscale != 1.0:
                nc.vector.scalar_tensor_tensor(out=row, in0=pst,
                                               scalar=1.0 / wscale,
                                               in1=bproj_row, op0=OP.mult,
                                               op1=OP.add)
            else:
                nc.vector.tensor_tensor(out=row, in0=pst, in1=bproj_row,
                                        op=OP.add)
            rowbf = None
            if want_bf:
                rowbf = pool.tile([1, ndim], BF16, tag=tag + "b")
                nc.vector.tensor_copy(out=rowbf, in_=row)
            return row, rowbf

        def row_to_fm(pool, row, n, tag):
            col = pool.tile([P, n // P], F32, tag=tag)
            for kt in range(n // P):
                pt = ps["S"].tile([P, 1], F32, tag="sm", name="r2fps")
                nc.tensor.transpose(out=pt, in_=row[:, kt * P:(kt + 1) * P],
                                    identity=ident_f[0:1, 0:1])
                nc.scalar.copy(out=col[:, kt:kt + 1], in_=pt)
            return col

        # ---------------- LN stats ----------------
        RSQRT_MAGIC = 0x5F3759DF
        magic_t = const.tile([P, 1], dt.uint32)
        nc.vector.memset(magic_t, RSQRT_MAGIC)

        def ln_stats(src_tile, nt, tag, engine="dve", bufs=None):
            if bufs is None:
                _sv = small.tile
            else:
                def _sv(shape, dtype, tag=None):
                    return small.tile(shape, dtype, tag=tag, bufs=bufs,
                                      name="ln" + (tag or "t"))
            """-> (stats, rstd [P,nt], neg_ms [P,nt] = -mu*rstd).

            engine="act": rstd = exp(-0.5*ln(var+eps)) on ACT (needs nle table).
            engine="dve": Newton rsqrt entirely on DVE (table-free)."""
            stats = _sv([P, nt, 2], F32, tag=tag + "st")
            for tt in range(nt):
                bstat = _sv([P, 6], F32, tag=tag + "bn")
                nc.vector.bn_stats(out=bstat, in_=src_tile[:, tt, :])
                nc.vector.bn_aggr(out=stats[:, tt, :], in_=bstat)
            rstd = _sv([P, nt], F32, tag=tag + "rs")
            if engine == "act":
                lnv = _sv([P, nt], F32, tag=tag + "lv")
                nc.scalar.activation(out=lnv, in_=stats[:, :, 1],
                                     func=AF.Ln, bias=eps_t)
                nc.scalar.activation(out=rstd, in_=lnv, func=AF.Exp, scale=-0.5)
            else:
                ve = _sv([P, nt], F32, tag=tag + "ve")
                nc.vector.tensor_scalar(out=ve, in0=stats[:, :, 1],
                                        scalar1=EPS, scalar2=None, op0=OP.add)
                sh = _sv([P, nt], dt.uint32, tag=tag + "sh")
                nc.vector.tensor_scalar(out=sh, in0=ve[:].bitcast(dt.uint32),
                                        scalar1=1, scalar2=None,
                                        op0=OP.logical_shift_right)
                mg = bass.AP(tensor=magic_t.tensor, offset=magic_t[:].offset,
                             ap=[magic_t[:].ap[0], [0, nt]])
                nc.vector.tensor_tensor(out=rstd[:].bitcast(dt.uint32),
                                        in0=mg, in1=sh, op=OP.subtract)
                for _ in range(2):
                    t1 = _sv([P, nt], F32, tag=tag + "t1")
                    nc.vector.tensor_tensor(out=t1, in0=rstd, in1=rstd,
                                            op=OP.mult)
                    nc.vector.tensor_tensor(out=t1, in0=t1, in1=ve, op=OP.mult)
                    nc.vector.tensor_scalar(out=t1, in0=t1, scalar1=-0.5,
                                            scalar2=1.5, op0=OP.mult, op1=OP.add)
                    nc.vector.tensor_tensor(out=rstd, in0=rstd, in1=t1,
                                            op=OP.mult)
            neg_ms = _sv([P, nt], F32, tag=tag + "nm")
            nc.vector.tensor_tensor(out=neg_ms, in0=stats[:, :, 0], in1=rstd,
                                    op=OP.mult)
            nc.vector.tensor_scalar(out=neg_ms, in0=neg_ms, scalar1=-1.0,
                                    scalar2=None, op0=OP.mult)
            return stats, rstd, neg_ms

        def make_z(src_tile, nt, st, dtype=BF16, tag="z", pool=None):
            """normalize (or plain convert) src [P,nt,512] -> z [P,nt,512] dtype.
            st = (stats, rstd, neg_ms) from ln_stats, or None for plain copy.
            Splits tt's between DVE and ACT."""
            pool = pool or zpool
            z = pool.tile([P, nt, D], dtype, tag=tag + ("8" if dtype == FP8 else ""))
            for tt in range(nt):
                if st is None:
                    if tt % 2 == 0:
                        nc.vector.tensor_copy(out=z[:, tt, :],
                                              in_=src_tile[:, tt, :])
                    else:
                        nc.scalar.copy(out=z[:, tt, :], in_=src_tile[:, tt, :])
                else:
                    stats, rstd, neg_ms = st
                    if tt % 2 == 0:
                        nc.vector.tensor_scalar(out=z[:, tt, :],
                                                in0=src_tile[:, tt, :],
                                                scalar1=stats[:, tt, 0:1],
                                                scalar2=rstd[:, tt:tt + 1],
                                                op0=OP.subtract, op1=OP.mult)
                    else:
                        nc.scalar.activation(out=z[:, tt, :],
                                             in_=src_tile[:, tt, :],
                                             func=AF.Identity,
                                             bias=neg_ms[:, tt:tt + 1],
                                             scale=rstd[:, tt:tt + 1])
            return z

        identities = {BF16: ident_bf, F32: ident_f}
        if fp8_ffn or fp8_qkv or fp8_out:
            ident_f8 = const.tile([P, P], FP8)
            make_identity(nc, ident_f8)
            identities[FP8] = ident_f8

        def transpose_fm(z, nt, pool, tag, dtype=BF16):
            """z [P,nt,D] -> fm [P, FT, nt*P] via PE transposes.

            fp8 out: the PE transpose itself runs in bf16 (hw fp8-transpose
            needs a packed output layout); the PSUM->SBUF copy converts."""
            tdt = BF16 if dtype == FP8 else dtype
            ident = identities[tdt]
            fm = pool.tile([P, FT, nt * P], dtype,
                           tag=tag + ("8" if dtype == FP8 else ""))
            ngrp = (nt + 3) // 4
            for ft in range(FT):
                for g in range(ngrp):
                    n_in = min(4, nt - g * 4)
                    pt = ps["B"].tile([P, 512], tdt, tag="tr", name="trps")
                    for i in range(n_in):
                        tt = g * 4 + i
                        nc.tensor.transpose(out=pt[:, i * P:(i + 1) * P],
                                            in_=z[:, tt, ft * P:(ft + 1) * P],
                                            identity=ident)
                    if ft % 2 == 0:
                        nc.vector.tensor_copy(
                            out=fm[:, ft, g * 512:g * 512 + n_in * P],
                            in_=pt[:, 0:n_in * P])
                    else:
                        nc.scalar.copy(
                            out=fm[:, ft, g * 512:g * 512 + n_in * P],
                            in_=pt[:, 0:n_in * P])
            return fm

        # ---------------- stylization vectors (silu table) ----------------
        AC = {}
        with tc.tile_pool(name="embp", bufs=1) as embp, \
             tc.tile_pool(name="psSe", bufs=2, space="PSUM") as _psSe:
            ps["S"] = _psSe
            e_sb = embp.tile([BL, TE], F32)
            nc.sync.dma_start(out=e_sb, in_=emb_d[:])
            semb = embp.tile([P, FT, BL], BF16)     # silu(emb)^T fm
            for kt in range(FT):
                pt = ps["S"].tile([P, BL], F32, tag="sm", name="embtr")
                nc.tensor.transpose(out=pt, in_=e_sb[:, kt * P:(kt + 1) * P],
                                    identity=ident_f[0:BL, 0:BL])
                act_reg(nc.scalar.activation(out=semb[:, kt, :], in_=pt,
                                             func=AF.Silu))
            for blk in ("sa", "ca", "ffn"):
                ew = load_w_fm(embp, W[blk + "_emb_w"][:], TE, 2 * D, tag="ew")
                eb_row = load_row(embp, W[blk + "_emb_b"][:], 2 * D, tag="ebr")
                eb_bf = embp.tile([1, 2 * D], BF16, tag="ebb")
                nc.vector.tensor_copy(out=eb_bf, in_=eb_row)
                pnb = load_fm_vec(embp, W[blk + "_pnorm_b"][:], D, tag="pnb")
                e_full = embp.tile([BL, 2 * D], F32, tag="ef")
                for half in range(2):
                    pse = ps["S"].tile([BL, 512], F32, tag="sm", name="embmm")
                    nc.tensor.matmul(out=pse, lhsT=ones_row_bf[:, 0:BL],
                                     rhs=eb_bf[:, half * 512:(half + 1) * 512],
                                     start=True, stop=False)
                    for kt in range(FT):
                        nc.tensor.matmul(out=pse, lhsT=semb[:, kt, :],
                                         rhs=ew[kt][:, half * 512:(half + 1) * 512],
                                         start=False, stop=(kt == FT - 1))
                    nc.scalar.copy(out=e_full[:, half * 512:(half + 1) * 512],
                                   in_=pse)
                A = vecp.tile([P, FT, BL], F32, tag=blk + "A")
                Cs = vecp.tile([P, FT, BL], F32, tag=blk + "C")
                sh = embp.tile([P, FT, BL], F32, tag="sh")
                for kt in range(FT):
                    pt = ps["S"].tile([P, BL], F32, tag="sm", name="embtr")
                    nc.tensor.transpose(out=pt, in_=e_full[:, kt * P:(kt + 1) * P],
                                        identity=ident_f[0:BL, 0:BL])
                    nc.vector.tensor_scalar(out=A[:, kt, :], in0=pt, scalar1=1.0,
                                            scalar2=None, op0=OP.add)
                    pt2 = ps["S"].tile([P, BL], F32, tag="sm", name="embtr2")
                    nc.tensor.transpose(out=pt2,
                                        in_=e_full[:, D + kt * P:D + (kt + 1) * P],
                                        identity=ident_f[0:BL, 0:BL])
                    nc.scalar.copy(out=sh[:, kt, :], in_=pt2)
                pnb_b = bass.AP(tensor=pnb.tensor, offset=pnb[:].offset,
                                ap=[pnb[:].ap[0], pnb[:].ap[1], [0, BL]])
                nc.vector.tensor_tensor(out=Cs, in0=A, in1=pnb_b, op=OP.mult)
                nc.vector.tensor_tensor(out=Cs, in0=Cs, in1=sh, op=OP.add)
                AC[blk] = (A, Cs)
        act_break()

        # ---------------- load x (TM), bf16 residual stream ----------------
        x_sb = []
        for b in range(BL):
            xt = xpool.tile([P, NT, D], BF16, tag=f"x{b}")
            for tt in range(NT):
                nc.gpsimd.dma_start(out=xt[:, tt, :],
                                    in_=x_d[b, tt * P:(tt + 1) * P, :])
            x_sb.append(xt)

        # ================= shared attention (SA/CA) =================
        def attention(b, xhat, wq, wk, wv, qb_fm, vb_bf, kv_fm, nkv,
                      m_sb, maskb, vscale, fp8, dbg=None):
            dr = DR if fp8 else None
            escale = RWS if fp8 else 1.0
            # ---- Q projection (FM out) + exp ----
            expq = qpool.tile([P, FT, T], BF16, tag="expq")
            for mt in range(FT):
                psq0 = ps["A"].tile([P, 512], F32, tag="mm", name="qmm0")
                psq1 = ps["A"].tile([P, 512], F32, tag="mm", name="qmm1")
                if fp8:
                    for p_ in range(2):
                        lhs = wq[p_][:, :, mt * P:(mt + 1) * P]
                        nc.tensor.matmul(out=psq0, lhsT=lhs,
                                         rhs=xhat[:, 2 * p_:2 * p_ + 2, 0:512],
                                         start=(p_ == 0), stop=(p_ == 1),
                                         perf_mode=dr)
                        nc.tensor.matmul(out=psq1, lhsT=lhs,
                                         rhs=xhat[:, 2 * p_:2 * p_ + 2, 512:1024],
                                         start=(p_ == 0), stop=(p_ == 1),
                                         perf_mode=dr)
                else:
                    for kt in range(FT):
                        lhs = wq[kt][:, mt * P:(mt + 1) * P]
                        nc.tensor.matmul(out=psq0, lhsT=lhs,
                                         rhs=xhat[:, kt, 0:512],
                                         start=(kt == 0), stop=(kt == FT - 1))
                        nc.tensor.matmul(out=psq1, lhsT=lhs,
                                         rhs=xhat[:, kt, 512:1024],
                                         start=(kt == 0), stop=(kt == FT - 1))
                for th, psq in ((0, psq0), (1, psq1)):
                    act_reg(nc.scalar.activation(
                        out=expq[:, mt, th * 512:(th + 1) * 512],
                        in_=psq, func=AF.Exp, scale=escale,
                        bias=qb_fm[:, mt:mt + 1]))
            # ---- K/V streamed into einsum1; s_k via mask column ----
            e1a = ps["B"].tile([P, 512], F32, tag="e1", name="e1a")
            e1b = ps["B"].tile([P, 512], F32, tag="e1", name="e1b")
            e1t = (e1a, e1a, e1b, e1b)
            for tt in range(nkv):
                kps = ps["A"].tile([P, 512], F32, tag="mm", name="kps")
                vps = ps["A"].tile([P, 512], F32, tag="mm", name="vps")
                nc.tensor.matmul(out=vps, lhsT=ones_row_bf, rhs=vb_bf,
                                 start=True, stop=False)
                if fp8:
                    for p_ in range(2):
                        lhs = kv_fm[:, 2 * p_:2 * p_ + 2, tt * P:(tt + 1) * P]
                        nc.tensor.matmul(out=kps, lhsT=lhs, rhs=wk[p_],
                                         start=(p_ == 0), stop=(p_ == 1),
                                         perf_mode=dr)
                        nc.tensor.matmul(out=vps, lhsT=lhs, rhs=wv[p_],
                                         start=False, stop=(p_ == 1),
                                         perf_mode=dr)
                else:
                    nkt = len(wk)
                    for kt in range(nkt):
                        lhs = kv_fm[:, kt, tt * P:(tt + 1) * P]
                        nc.tensor.matmul(out=kps, lhsT=lhs, rhs=wk[kt],
                                         start=(kt == 0), stop=(kt == nkt - 1))
                        nc.tensor.matmul(out=vps, lhsT=lhs, rhs=wv[kt],
                                         start=False, stop=(kt == nkt - 1))
                expk = kvp.tile([P, 512], BF16, tag="expk")
                v_ext = kvp.tile([P, FT, 132], BF16, tag="vext")
                if maskb is not None:
                    # mask folds entirely into expk's bias (exp(-80) == 0 to
                    # fp precision): v and the s_k ones-column stay unmasked
                    act_reg(nc.scalar.activation(out=expk, in_=kps, func=AF.Exp,
                                                 scale=escale,
                                                 bias=maskb[:, tt:tt + 1]))
                    if fp8:
                        nc.scalar.activation(out=v_ext[:, :, 0:128], in_=vps,
                                             func=AF.Copy, scale=escale)
                    else:
                        nc.scalar.copy(out=v_ext[:, :, 0:128], in_=vps)
                    nc.vector.memset(v_ext[:, :, 128:129], 1.0)
                else:
                    if fp8:
                        act_reg(nc.scalar.activation(out=expk, in_=kps,
                                                     func=AF.Exp, scale=escale))
                        nc.scalar.activation(out=v_ext[:, :, 0:128], in_=vps,
                                             func=AF.Copy, scale=escale)
                    else:
                        act_reg(nc.scalar.activation(out=expk, in_=kps,
                                                     func=AF.Exp))
                        nc.scalar.copy(out=v_ext[:, :, 0:128], in_=vps)
                    nc.vector.memset(v_ext[:, :, 128:129], 1.0)
                for ft in range(FT):
                    # one accumulation group per BANK: start=True clears the
                    # whole bank's has_written bits, so only the first matmul
                    # into each bank may set it (the ft-odd range then
                    # overwrites-where-clear at tt==0 and accumulates after).
                    off = (ft % 2) * 256
                    nc.tensor.matmul(out=e1t[ft][:, off:off + 129],
                                     lhsT=expk[:, ft * P:(ft + 1) * P],
                                     rhs=v_ext[:, ft, 0:129],
                                     start=(tt == 0 and ft % 2 == 0),
                                     stop=(tt == nkv - 1 and ft % 2 == 1))
            # ---- rsk + att_bd (block-diag per head pair) ----
            rsk = small.tile([P, FT], F32, tag="rsk")
            for ft in range(FT):
                off = (ft % 2) * 256
                nc.vector.reciprocal(out=rsk[:, ft:ft + 1],
                                     in_=e1t[ft][:, off + 128:off + 129])
            att_bd = abp.tile([P, FT, P], BF16, tag="attbd")
            nc.vector.memset(att_bd, 0.0)
            for ft in range(FT):
                off = (ft % 2) * 256
                for r in range(2):
                    s = slice(64 * r, 64 * r + 64)
                    nc.vector.tensor_scalar(
                        out=att_bd[s, ft, s],
                        in0=e1t[ft][s, off + 64 * r:off + 64 * r + 64],
                        scalar1=rsk[s, ft:ft + 1], scalar2=None, op0=OP.mult)
            if dbg is not None:
                tap(dbg + "_attbd", att_bd[:])
                tap(dbg + "_expq", expq[:])
            # ---- einsum2 + s_q + normalize -> y TM ----
            y = ypool.tile([P, NT, D], BF16, tag="y")
            sqps = ps["S"].tile([P, NT, H], F32, tag="sm", name="sqps")
            for tt in range(NT):
                yps = ps["A"].tile([P, 512], F32, tag="mm", name="ymm")
                for ft in range(FT):
                    lhs = expq[:, ft, tt * P:(tt + 1) * P]
                    nc.tensor.matmul(out=yps[:, ft * P:(ft + 1) * P], lhsT=lhs,
                                     rhs=att_bd[:, ft, :], start=True, stop=True)
                    nc.tensor.matmul(out=sqps[:, tt, 2 * ft:2 * ft + 2], lhsT=lhs,
                                     rhs=sel_bf, start=True, stop=True)
                rsq = small.tile([P, H], F32, tag="rsq")
                nc.vector.reciprocal(out=rsq, in_=sqps[:, tt, :])
                nc.vector.tensor_tensor(
                    out=y[:, tt, :].rearrange("p (g d) -> p g d", g=H),
                    in0=yps[:].rearrange("p (g d) -> p g d", g=H),
                    in1=bass.AP(tensor=rsq.tensor, offset=rsq[:].offset,
                                ap=[rsq[:].ap[0], rsq[:].ap[1], [0, DH]]),
                    op=OP.mult)
            return y

        # ---- stylize split: pre (nle table) / post (silu table) ----
        def stylize_pre(y, engine=None):
            if engine is None:
                engine = "dve" if pre_dve else "act"
            st = ln_stats(y, NT, tag="pn", engine=engine)
            zy = make_z(y, NT, st)
            return transpose_fm(zy, NT, zfmp, tag="zfm")

        def stylize_post(blk, b, zfm, wo, ob_bf, fp8, final=False):
            A, Cs = AC[blk]
            sdt = FP8 if fp8 else BF16
            for th in range(2):
                sfm = sfmp.tile([P, FT, 512], sdt, tag="sfm", bufs=2)
                for ft in range(FT):
                    act_reg(nc.scalar.activation(
                        out=sfm[:, ft, :],
                        in_=zfm[:, ft, th * 512:(th + 1) * 512],
                        func=AF.Silu, scale=A[:, ft, b:b + 1],
                        bias=Cs[:, ft, b:b + 1]))
                for ti in range(4):
                    tt = th * 4 + ti
                    ops = ps["A"].tile([P, 512], F32, tag="mm", name="omm")
                    nc.tensor.matmul(out=ops, lhsT=ones_row_bf, rhs=ob_bf,
                                     start=True, stop=False)
                    if final:
                        # last residual: evict in f32 straight to output DMA
                        dst = xop.tile([P, 512], F32, tag="xo", name="xo")
                    else:
                        dst = x_sb[b][:, tt, :]
                    if fp8:
                        for p_ in range(2):
                            nc.tensor.matmul(
                                out=ops,
                                lhsT=sfm[:, 2 * p_:2 * p_ + 2,
                                         ti * P:(ti + 1) * P],
                                rhs=wo[p_], start=False, stop=(p_ == 1),
                                perf_mode=DR)
                        nc.vector.scalar_tensor_tensor(
                            out=dst, in0=ops, scalar=RWS,
                            in1=x_sb[b][:, tt, :], op0=OP.mult, op1=OP.add)
                    else:
                        for ft in range(FT):
                            nc.tensor.matmul(out=ops,
                                             lhsT=sfm[:, ft, ti * P:(ti + 1) * P],
                                             rhs=wo[ft], start=False,
                                             stop=(ft == FT - 1))
                        nc.vector.tensor_tensor(out=dst, in0=ops,
                                                in1=x_sb[b][:, tt, :],
                                                op=OP.add)
                    if final:
                        nc.sync.dma_start(out=out_d[b, tt * P:(tt + 1) * P, :],
                                          in_=dst)

        # ================= SA phase =================
        wdt = FP8 if fp8_qkv else BF16
        wodt = FP8 if fp8_out else BF16
        wsc = WS if fp8_qkv else None
        wosc = WS if fp8_out else None
        with tc.tile_pool(name="psA_sa", bufs=3, space="PSUM") as _pa, \
             tc.tile_pool(name="psB_sa", bufs=2, space="PSUM") as _pb, \
             tc.tile_pool(name="psS_sa", bufs=1, space="PSUM") as _psx, \
             tc.tile_pool(name="wmain", bufs=1) as wmain, \
             tc.tile_pool(name="wrow", bufs=1) as wrow, \
             tc.tile_pool(name="cap", bufs=1) as cap:
            ps["A"], ps["B"], ps["S"] = _pa, _pb, _psx

            def prep_attn(pre):
                """Load+fold one attention block's weights. pre in {sa, ca}."""
                g_fm = load_fm_vec(wrow, W[pre + "_norm_g"][:], D, tag="g")
                bln = load_fm_vec(wrow, W[pre + "_norm_b"][:], D, tag="bln")
                if pre == "ca":
                    tg_fm = load_fm_vec(wrow, W["ca_tnorm_g"][:], L, tag="tg")
                    tbln = load_fm_vec(wrow, W["ca_tnorm_b"][:], L, tag="tbln")
                else:
                    tg_fm, tbln = g_fm, bln
                kvdim = L if pre == "ca" else D
                wq = load_w_fm(wmain, W[pre + "_q_w"][:], D, D, gfm=g_fm,
                               tag="wq", dtype=wdt, scale=wsc, paired=fp8_qkv)
                wk = load_w_fm(wmain, W[pre + "_k_w"][:], kvdim, D, gfm=tg_fm,
                               tag="wk", dtype=wdt, scale=wsc, paired=fp8_qkv)
                wv = load_w_fm(wmain, W[pre + "_v_w"][:], kvdim, D, gfm=tg_fm,
                               tag="wv", dtype=wdt, scale=wsc, paired=fp8_qkv)
                wo = load_w_fm(wmain, W[pre + "_out_w"][:], D, D, tag="wo",
                               dtype=wodt, scale=wosc, paired=fp8_out, bufs=1)
                bln_c = wrow.tile([P, FT], wdt, tag="blnb")
                tbln_c = wrow.tile([P, FT], wdt, tag="tblnb")
                bsc = WS if fp8_qkv else 1.0
                nc.vector.tensor_scalar(out=bln_c, in0=bln, scalar1=bsc,
                                        scalar2=None, op0=OP.mult)
                nc.vector.tensor_scalar(out=tbln_c, in0=tbln, scalar1=bsc,
                                        scalar2=None, op0=OP.mult)
                wsq = (WS * WS) if fp8_qkv else 1.0
                qb_row = load_row(wrow, W[pre + "_q_b"][:], D, tag="rowtmp",
                                  bufs=2)
                vb_row = load_row(wrow, W[pre + "_v_b"][:], D, tag="rowtmp",
                                  bufs=2)
                qb_row2, _ = fold_bias(wrow, wq, bln_c, qb_row, D, tag="qbf",
                                       wscale=wsq, paired=fp8_qkv,
                                       want_bf=False)
                qb_fm = row_to_fm(wrow, qb_row2, D, tag="qbfm")
                ob_row = load_row(wrow, W[pre + "_out_b"][:], D, tag="rowtmp",
                                  bufs=2)
                vb_row2, vb_bf = fold_bias(wrow, wv, tbln_c, vb_row, D,
                                           tag="vbf", wscale=wsq,
                                           paired=fp8_qkv,
                                           want_bf=not fp8_qkv)
                if fp8_qkv:
                    vb_sc = wrow.tile([1, D], BF16, tag="vbfb")
                    nc.vector.tensor_scalar(out=vb_sc, in0=vb_row2, scalar1=WS,
                                            scalar2=None, op0=OP.mult)
                    vb_bf = vb_sc
                ob_bf = wrow.tile([1, D], BF16, tag="obbf")
                if fp8_out:
                    nc.vector.tensor_scalar(out=ob_bf, in0=ob_row, scalar1=WS,
                                            scalar2=None, op0=OP.mult)
                else:
                    nc.vector.tensor_copy(out=ob_bf, in_=ob_row)
                return wq, wk, wv, wo, qb_fm, vb_bf, ob_bf

            # SA x stats: x_sb fresh from DMA, compute during weight prep
            sts = [ln_stats(x_sb[b], NT, tag=f"sxln{b}", bufs=1)
                   for b in range(BL)]
            for pre in ("sa", "ca"):
                wq, wk, wv, wo, qb_fm, vb_bf, ob_bf = prep_attn(pre)
                zfms = []
                ys = {}

                def stage_a(b):
                    if pre == "sa":
                        m_sb = small.tile([P, NT], F32, tag="msb",
                                          name="m_sb")
                        for tt in range(NT):
                            nc.sync.dma_start(out=m_sb[:, tt:tt + 1],
                                              in_=mask_d[b, tt * P:(tt + 1) * P, :])
                        maskb = small.tile([P, NT], F32, tag="mbias",
                                           name="maskb")
                        nc.vector.tensor_scalar(out=maskb, in0=m_sb,
                                                scalar1=-1.0, scalar2=-MASK_NEG,
                                                op0=OP.add, op1=OP.mult)
                        if fp8_qkv:
                            vscale = small.tile([P, NT], F32, tag="msc",
                                                name="vscale")
                            nc.vector.tensor_scalar(out=vscale, in0=m_sb,
                                                    scalar1=RWS, scalar2=None,
                                                    op0=OP.mult)
                        else:
                            vscale = m_sb
                        kv_src, nkv = None, NT
                    else:
                        m_sb = maskb = vscale = None
                        xf_sb = cap.tile([P, NTC, L], F32, tag="xfsb", bufs=1,
                                         name="xf_sb")
                        for tt in range(NTC):
                            nc.sync.dma_start(out=xf_sb[:, tt, :],
                                              in_=xf_d[b, tt * P:(tt + 1) * P, :])
                        tst = ln_stats(xf_sb, NTC, tag="tln")
                        ztn = make_z(xf_sb, NTC, tst, tag="zt", pool=cap)
                        kv_src = transpose_fm(ztn, NTC, cap, tag="tnfm",
                                              dtype=wdt)
                        nkv = NTC
                    zx = make_z(x_sb[b], NT, sts[b])
                    xhat = transpose_fm(zx, NT, fmp, tag="fm", dtype=wdt)
                    if kv_src is None:
                        kv_src = xhat
                    return attention(b, xhat, wq, wk, wv, qb_fm, vb_bf,
                                     kv_src, nkv, m_sb, maskb, vscale,
                                     fp8_qkv, dbg=(pre if b == 0 else None))

                # software-pipelined: stylize_pre(b-1)'s serial DVE stats are
                # emitted after batch b's PE-heavy attention work
                for b in range(BL):
                    ys[b] = stage_a(b)
                    if b == 0:
                        tap(pre + "_y", ys[b][:])
                    if b >= 1:
                        zfms.append(stylize_pre(ys.pop(b - 1)))
                zfms.append(stylize_pre(ys.pop(BL - 1)))
                act_break()   # exps before silus
                nxt = []
                for b in range(BL):
                    stylize_post(pre, b, zfms[b], wo, ob_bf, fp8_out)
                    if pre == "sa":
                        # CA x stats right behind each residual update
                        nxt.append(ln_stats(x_sb[b], NT, tag=f"cxln{b}",
                                            bufs=1))
                    if b == 0:
                        tap("x_after_" + pre, x_sb[b][:])
                act_break()   # silus before next phase's exps
                sts = nxt

        # ================= FFN phase =================
        fdt = FP8 if fp8_ffn else BF16
        fsc = WS if fp8_ffn else None
        with tc.tile_pool(name="ffn_w", bufs=1) as wp, \
             tc.tile_pool(name="gelu_p", bufs=2) as gp, \
             tc.tile_pool(name="psA_f", bufs=2, space="PSUM") as _pa, \
             tc.tile_pool(name="psB_f", bufs=2, space="PSUM") as _pb, \
             tc.tile_pool(name="psyf", bufs=1, space="PSUM") as psyf:
            ps["A"], ps["B"], ps["S"] = _pa, _pb, _pb
            w1 = load_w_fm(wp, W["ffn_w1"][:], D, FF, tag="w1", dtype=fdt,
                           scale=fsc, paired=fp8_ffn)
            w2 = load_w_fm(wp, W["ffn_w2"][:], FF, D, tag="w2", dtype=fdt,
                           scale=fsc, paired=fp8_ffn)
            b1_fm = load_fm_vec(wp, W["ffn_b1"][:], FF, tag="b1")
            b2_row = load_row(wp, W["ffn_b2"][:], D, tag="rowtmp", bufs=1)
            b2_bf = wp.tile([1, D], BF16, tag="b2b")
            if fp8_ffn:
                nc.vector.tensor_scalar(out=b2_bf, in0=b2_row, scalar1=WS,
                                        scalar2=None, op0=OP.mult)
            else:
                nc.vector.tensor_copy(out=b2_bf, in_=b2_row)
            wo = load_w_fm(wp, W["ffn_out_w"][:], D, D, tag="wo", dtype=wodt,
                           scale=wosc, paired=fp8_out)
            ob_row = load_row(wp, W["ffn_out_b"][:], D, tag="rowtmp", bufs=1)
            ob_bf = wp.tile([1, D], BF16, tag="obbf")
            if fp8_out:
                nc.vector.tensor_scalar(out=ob_bf, in0=ob_row, scalar1=WS,
                                        scalar2=None, op0=OP.mult)
            else:
                nc.vector.tensor_copy(out=ob_bf, in_=ob_row)

            zfms = []
            ys = {}
            for b in range(BL):
                zx = make_z(x_sb[b], NT, None, tag="z")
                x_fm = transpose_fm(zx, NT, fmp, tag="fm", dtype=fdt)
                y = ypool.tile([P, NT, D], BF16, tag="y")
                gsc = RWS if fp8_ffn else None
                for th in range(2):
                    yps = [psyf.tile([P, 512], F32, tag=f"yf{i}",
                                     name=f"yf{i}") for i in range(4)]
                    for i in range(4):
                        nc.tensor.matmul(out=yps[i], lhsT=ones_row_bf,
                                         rhs=b2_bf, start=True, stop=False)
                    nmt = FF // P
                    for mt in range(nmt):
                        gps = ps["A"].tile([P, 512], F32, tag="mm", name="gmm")
                        if fp8_ffn:
                            for p_ in range(2):
                                nc.tensor.matmul(
                                    out=gps,
                                    lhsT=w1[p_][:, :, mt * P:(mt + 1) * P],
                                    rhs=x_fm[:, 2 * p_:2 * p_ + 2,
                                             th * 512:(th + 1) * 512],
                                    start=(p_ == 0), stop=(p_ == 1),
                                    perf_mode=DR)
                        else:
                            for kt in range(FT):
                                nc.tensor.matmul(
                                    out=gps, lhsT=w1[kt][:, mt * P:(mt + 1) * P],
                                    rhs=x_fm[:, kt, th * 512:(th + 1) * 512],
                                    start=(kt == 0), stop=(kt == FT - 1))
                        if fp8_ffn:
                            if mt % 2 == 0:
                                gpair = gp.tile([P, 2, 512], FP8, tag="g")
                            act_reg(nc.scalar.activation(
                                out=gpair[:, mt % 2, :], in_=gps, func=AF.Gelu,
                                scale=RWS, bias=b1_fm[:, mt:mt + 1]))
                            if mt % 2 == 1:
                                for i in range(4):
                                    nc.tensor.matmul(
                                        out=yps[i],
                                        lhsT=gpair[:, :, i * P:(i + 1) * P],
                                        rhs=w2[mt // 2], start=False,
                                        stop=(mt == nmt - 1), perf_mode=DR)
                        else:
                            gsb = gp.tile([P, 512], BF16, tag="g")
                            act_reg(nc.scalar.activation(
                                out=gsb, in_=gps, func=AF.Gelu,
                                bias=b1_fm[:, mt:mt + 1]))
                            for i in range(4):
                                nc.tensor.matmul(
                                    out=yps[i], lhsT=gsb[:, i * P:(i + 1) * P],
                                    rhs=w2[mt], start=False,
                                    stop=(mt == nmt - 1))
                    for i in range(4):
                        tt = th * 4 + i
                        if fp8_ffn:
                            if i % 2 == 0:
                                nc.vector.tensor_scalar(out=y[:, tt, :],
                                                        in0=yps[i], scalar1=RWS,
                                                        scalar2=None,
                                                        op0=OP.mult)
                            else:
                                nc.scalar.activation(out=y[:, tt, :],
                                                     in_=yps[i], func=AF.Copy,
                                                     scale=RWS)
                        elif i % 2 == 0:
                            nc.vector.tensor_copy(out=y[:, tt, :], in_=yps[i])
                        else:
                            nc.scalar.copy(out=y[:, tt, :], in_=yps[i])
                if b == 0:
                    tap("ffn_y", y[:])
                ys[b] = y
                if b >= 1:
                    zfms.append(stylize_pre(ys.pop(b - 1), engine="dve"))
            zfms.append(stylize_pre(ys.pop(BL - 1), engine="dve"))
            act_break()   # gelus before final silus
            for b in range(BL):
                stylize_post("ffn", b, zfms[b], wo, ob_bf, fp8_out, final=True)

    nc.compile()
    return nc, tap_tensors
# ======================= runner =======================


def make_runner(nc, n_cores=8):
    from concourse.bass2jax import (_bass_exec_p, install_neuronx_cc_hook,
                                    partition_id_tensor)
    from jax.sharding import Mesh, PartitionSpec
    from jax.experimental.shard_map import shard_map
    install_neuronx_cc_hook()
    partition_name = nc.partition_id_tensor.name if nc.partition_id_tensor else None
    in_names, out_names, out_avals, zero_outs = [], [], [], []
    for alloc in nc.m.functions[0].allocations:
        if not isinstance(alloc, mybir.MemoryLocationSet):
            continue
        name = alloc.memorylocations[0].name
        if alloc.kind == "ExternalInput":
            if name != partition_name:
                in_names.append(name)
        elif alloc.kind == "ExternalOutput":
            out_names.append(name)
            shape = tuple(alloc.tensor_shape)
            dtype = mybir.dt.np(alloc.dtype)
            out_avals.append(jax.core.ShapedArray(shape, dtype))
            zero_outs.append(np.zeros(shape, dtype))
    n_params = len(in_names)
    in_names_full = list(in_names) + out_names + ([partition_name] if partition_name else [])

    def _body(*args):
        operands = list(args)
        if partition_name is not None:
            operands.append(partition_id_tensor())
        return tuple(_bass_exec_p.bind(
            *operands, out_avals=tuple(out_avals), in_names=tuple(in_names_full),
            out_names=tuple(out_names), lowering_input_output_aliases=(),
            sim_require_finite=False, sim_require_nnan=False, nc=nc))

    devices = jax.devices()[:n_cores]
    mesh = Mesh(np.asarray(devices), ("core",))
    in_specs = (PartitionSpec("core"),) * (n_params + len(out_names))
    out_specs = (PartitionSpec("core"),) * len(out_names)
    sharded = jax.jit(shard_map(_body, mesh=mesh, in_specs=in_specs,
                                out_specs=out_specs, check_rep=False),
                      keep_unused=True)

    class Runner:
        def __init__(self):
            self.sharded = sharded
            self.in_names = in_names
            self.out_names = out_names
            self.zero_outs = zero_outs
            self.n_cores = n_cores

        def upload(self, in_maps):
            '''Pre-place inputs on device; returns device arg list.'''
            from jax.sharding import NamedSharding, PartitionSpec
            concat_in = [np.concatenate([np.asarray(in_maps[c][n])
                                         for c in range(self.n_cores)], axis=0)
                         for n in self.in_names]
            concat_zeros = [np.zeros((self.n_cores * z.shape[0], *z.shape[1:]),
                                     z.dtype) for z in self.zero_outs]
            sh = NamedSharding(mesh, PartitionSpec("core"))
            args = [jax.device_put(a, sh) for a in concat_in + concat_zeros]
            jax.block_until_ready(args)
            return args

        def run_dev(self, args):
            outs = sharded(*args)
            jax.block_until_ready(outs)
            return outs

        def __call__(self, in_maps):
            args = self.upload(in_maps)
            outs = self.run_dev(args)
            return [{name: np.asarray(outs[i]).reshape(self.n_cores,
                                                       *self.zero_outs[i].shape)[c]
                     for i, name in enumerate(self.out_names)}
                    for c in range(self.n_cores)]
    return Runner()


# ======================= public entry point =======================
_CACHE = {}
N_CORES = 8
B_FULL = 32
NB = B_FULL // N_CORES


import os
_FP8_FFN = os.environ.get("KERNEL_FP8_FFN", "1") == "1"
_FP8_QKV = os.environ.get("KERNEL_FP8_QKV", "0") == "1"
_FP8_OUT = os.environ.get("KERNEL_FP8_OUT", "0") == "1"


def _get_runner():
    if "runner" not in _CACHE:
        nc, _ = build(n_batch=NB, taps=(), fp8_ffn=_FP8_FFN,
                      fp8_qkv=_FP8_QKV, fp8_out=_FP8_OUT)
        _CACHE["runner"] = make_runner(nc, n_cores=N_CORES)
    return _CACHE["runner"]


def kernel(**inputs) -> np.ndarray:
    runner = _get_runner()
    sharded_keys = ("x", "xf", "emb", "src_mask")
    inp = {k: np.ascontiguousarray(np.asarray(v, dtype=np.float32))
           for k, v in inputs.items()}
    in_maps = []
    for c in range(N_CORES):
        m = {}
        for k, v in inp.items():
            m[k] = v[c * NB:(c + 1) * NB] if k in sharded_keys else v
        in_maps.append(m)
    res = runner(in_maps)
    out = np.concatenate([res[c]["out"] for c in range(N_CORES)], axis=0)
    return out.astype(np.float32)



# revision 51
# speedup vs baseline: 2.7671x; 2.7671x over previous
"""Trainium2 Bass kernel for nn_LinearTemporalDiffusionTransformerDecoderLayer.

v3 (on top of v2's pass-restructured phases / streamed K/V einsum1):
 - fp8 (e4m3, DoubleRow) FFN GEMMs with x64 weight pre-scale (halves the
   biggest PE load; rel_err 0.0137 vs the 2e-2 gate, deterministic inputs)
 - table-free DVE Newton-rsqrt for every LayerNorm (no natural_log ACT
   table ping-pong) + scheduler-only ordering edges that group exp/silu/gelu
   activations per phase: 69 -> 7 ACT table loads (~2.7us each on HW)
 - software-pipelined batch loop: stylize_pre(b-1)'s serial DVE stats chains
   are emitted after batch b's PE-heavy attention work, so PE stays fed
 - cross-phase LN stats hoisting (CA x-stats emitted right behind each SA
   residual update)
 - weight loads via gpsimd cast-on-DMA (f32->bf16) with parallel whole-row
   DMAs; scaled/fp8 weights convert from a bf16 stage (no f32 stage ping-pong)
 - bf16 residual stream (halves x DMA + SBUF; final residual evicts f32
   straight to the output DMA so the last rounding is not taken)

TimelineSim cost model: 602us/core vs 910us for v2 (HW slope-timing is
tunnel-noise-limited; same-window A/B showed 876us vs ~1420us).

Self-contained: builds and compiles an 8-core SPMD Bass program on first call,
shards the batch dimension (B=32 -> 8 cores x 4), executes via PJRT, and
reassembles the full output.  kernel(**inputs) -> np.ndarray [32, 1024, 512].
"""
import sys
for _p in ("/opt/trn_rl_repo",):
    if _p not in sys.path:
        sys.path.insert(0, _p)
import numpy as np
import jax
import concourse.bass as bass
import concourse.bacc as bacc
import concourse.tile as tile
from concourse import mybir
from concourse.tile_rust import add_dep_helper
from concourse.masks import make_identity
from contextlib import ExitStack

dt = mybir.dt
F32, BF16 = dt.float32, dt.bfloat16
FP8 = dt.float8e4
AF = mybir.ActivationFunctionType
OP = mybir.AluOpType
DR = mybir.MatmulPerfMode.DoubleRow
P = 128
T, D, H, DH = 1024, 512, 8, 64
NCTX, L, TE, FF = 256, 512, 512, 2048
NT, FT = T // P, D // P          # 8 token tiles, 4 feature tiles
NTC = NCTX // P                  # 2 ctx tiles
EPS = 1e-5
MASK_NEG = -80.0
WS = 64.0                        # fp8 weight pre-scale
RWS = 1.0 / WS

PARAM_SHAPES = {
    "sa_norm_g": (D,), "sa_norm_b": (D,),
    "sa_q_w": (D, D), "sa_q_b": (D,), "sa_k_w": (D, D), "sa_k_b": (D,),
    "sa_v_w": (D, D), "sa_v_b": (D,),
    "sa_emb_w": (TE, 2 * D), "sa_emb_b": (2 * D,),
    "sa_pnorm_g": (D,), "sa_pnorm_b": (D,),
    "sa_out_w": (D, D), "sa_out_b": (D,),
    "ca_norm_g": (D,), "ca_norm_b": (D,), "ca_tnorm_g": (L,), "ca_tnorm_b": (L,),
    "ca_q_w": (D, D), "ca_q_b": (D,), "ca_k_w": (L, D), "ca_k_b": (D,),
    "ca_v_w": (L, D), "ca_v_b": (D,),
    "ca_emb_w": (TE, 2 * D), "ca_emb_b": (2 * D,),
    "ca_pnorm_g": (D,), "ca_pnorm_b": (D,),
    "ca_out_w": (D, D), "ca_out_b": (D,),
    "ffn_w1": (D, FF), "ffn_b1": (FF,), "ffn_w2": (FF, D), "ffn_b2": (D,),
    "ffn_emb_w": (TE, 2 * D), "ffn_emb_b": (2 * D,),
    "ffn_pnorm_g": (D,), "ffn_pnorm_b": (D,),
    "ffn_out_w": (D, D), "ffn_out_b": (D,),
}


def build(n_batch=4, taps=(), fp8_ffn=False, fp8_qkv=False, fp8_out=False,
          act_group=True, pre_dve=True):
    BL = n_batch
    nc = bacc.Bacc(None, target_bir_lowering=False, debug=False)
    tap_tensors = {}

    x_d = nc.declare_dram_parameter("x", [BL, T, D], F32, isOutput=False)
    xf_d = nc.declare_dram_parameter("xf", [BL, NCTX, L], F32, isOutput=False)
    emb_d = nc.declare_dram_parameter("emb", [BL, TE], F32, isOutput=False)
    mask_d = nc.declare_dram_parameter("src_mask", [BL, T, 1], F32, isOutput=False)
    # all 41 weight tensors packed into one dram array: per-call dispatch
    # marshaling scales with arg count, so 47 args -> 6
    total_w = sum(int(np.prod(s)) for s in PARAM_SHAPES.values())
    wflat_d = nc.declare_dram_parameter("wflat", [1, total_w], F32,
                                        isOutput=False)
    W = {}
    off = 0
    for name, shp in PARAM_SHAPES.items():
        sz = int(np.prod(shp))
        apw = wflat_d[0, off:off + sz]
        if len(shp) == 2:
            apw = apw.rearrange("(a b) -> a b", a=shp[0])
        W[name] = apw
        off += sz
    out_d = nc.declare_dram_parameter("out", [BL, T, D], F32, isOutput=True)

    with tile.TileContext(nc) as tc, ExitStack() as root:
        const = root.enter_context(tc.tile_pool(name="const", bufs=1))
        small = root.enter_context(tc.tile_pool(name="small", bufs=2))

        xpool = root.enter_context(tc.tile_pool(name="xpool", bufs=1))
        zpool = root.enter_context(tc.tile_pool(name="zpool", bufs=2))
        fmp = root.enter_context(tc.tile_pool(name="fmp", bufs=2))
        qpool = root.enter_context(tc.tile_pool(name="qpool", bufs=2))
        kvp = root.enter_context(tc.tile_pool(name="kvp", bufs=2))
        ypool = root.enter_context(tc.tile_pool(name="ypool", bufs=2))
        zfmp = root.enter_context(tc.tile_pool(name="zfmp", bufs=4))
        sfmp = root.enter_context(tc.tile_pool(name="sfmp", bufs=1))
        abp = root.enter_context(tc.tile_pool(name="abp", bufs=2))
        vecp = root.enter_context(tc.tile_pool(name="vecp", bufs=1))
        xop = root.enter_context(tc.tile_pool(name="xop", bufs=3))
        ps = {}

        # ---- ACT table grouping: scheduler-only ordering edges keep the
        # scalar engine's table-switching funcs (exp/silu/gelu) from
        # interleaving across phases (each switch = ~2.7us table reload) ----
        act_seq = {"prev": None, "cur": []}

        def act_reg(bi):
            if not act_group:
                return bi
            if act_seq["prev"] is not None:
                add_dep_helper(bi.ins, act_seq["prev"], sync=False,
                               reason="act-table-group")
            act_seq["cur"].append(bi)
            return bi

        def act_break():
            if not act_group or not act_seq["cur"]:
                return
            j = nc.scalar.nop()
            for bi in act_seq["cur"]:
                add_dep_helper(j.ins, bi.ins, sync=False,
                               reason="act-table-junction")
            act_seq["prev"] = j.ins
            act_seq["cur"] = []

        def tap(name, ap):
            if name in taps and name not in tap_tensors:
                tdn = "tap_" + name
                td = nc.declare_dram_parameter(tdn, list(ap.shape),
                                               ap.dtype, isOutput=True)
                nc.sync.dma_start(out=td[:], in_=ap)
                tap_tensors[name] = (tdn, tuple(ap.shape))

        # ---------------- constants ----------------
        ident_bf = const.tile([P, P], BF16)
        make_identity(nc, ident_bf)
        ident_f = const.tile([P, P], F32)
        make_identity(nc, ident_f)
        ones_row_bf = const.tile([1, P], BF16)       # K=1 lhsT for row broadcast
        nc.vector.memset(ones_row_bf, 1.0)
        sel_bf = const.tile([P, 2], BF16)            # head-pair selector
        nc.vector.memset(sel_bf, 0.0)
        nc.vector.memset(sel_bf[0:64, 0:1], 1.0)
        nc.vector.memset(sel_bf[64:128, 1:2], 1.0)
        eps_t = const.tile([P, 1], F32)
        nc.vector.memset(eps_t, EPS)

        # ---------------- load helpers ----------------
        def load_fm_vec(pool, ap1d, n=D, tag=None):
            nkt = n // P
            t = pool.tile([P, nkt], F32, tag=tag)
            nc.sync.dma_start(out=t, in_=ap1d.rearrange("(k p) -> p k", p=P))
            return t

        def load_row(pool, ap1d, n, tag=None, bufs=None):
            kw = {} if bufs is None else {"bufs": bufs}
            t = pool.tile([1, n], F32, tag=tag, **kw)
            nc.sync.dma_start(out=t, in_=ap1d.rearrange("(a n) -> a n", a=1))
            return t

        def load_w_fm(pool, wap, kdim, ndim, gfm=None, tag="w", dtype=BF16,
                      scale=None, paired=False, bufs=None):
            """Load weight [kdim, ndim] -> list of FM tiles.

            paired=False: kdim//P tiles of [P, ndim].
            paired=True: kdim//(2P) tiles of [P, 2, ndim] (DoubleRow k-pairs).
            gfm: per-row (input-feature) scale column tile [P, kdim//P].
            scale: extra scalar premultiplier (fp8 range scaling).
            """
            nkt = kdim // P
            tiles = []
            kw = {} if bufs is None else {"bufs": bufs}
            for kt in range(nkt):
                if paired:
                    if kt % 2 == 0:
                        wt = pool.tile([P, 2, ndim], dtype,
                                       tag=f"{tag}{kt // 2}", **kw)
                        tiles.append(wt)
                    dst = tiles[-1][:, kt % 2, :]
                else:
                    wt = pool.tile([P, ndim], dtype, tag=f"{tag}{kt}", **kw)
                    tiles.append(wt)
                    dst = wt
                if gfm is None and scale is None and dtype == BF16:
                    # cast-on-DMA (software DGE): skip the stage+convert hop
                    nc.gpsimd.dma_start(out=dst,
                                        in_=wap[kt * P:(kt + 1) * P, :])
                    continue
                # cast-on-DMA to a bf16 stage, then scale/convert in SBUF.
                # One whole-row DMA per k-tile: parallel DMA fan-out, no
                # per-chunk stage ping-pong.
                stg = pool.tile([P, ndim], BF16, tag=f"wstg{ndim}",
                                name="wstg", bufs=2)
                nc.gpsimd.dma_start(out=stg, in_=wap[kt * P:(kt + 1) * P, :])
                for c0 in range(0, ndim, 1024):
                    cw = min(1024, ndim - c0)
                    eng = nc.vector if (kt + c0 // 1024) % 2 == 0 else nc.scalar
                    if gfm is not None and scale is not None:
                        gs = small.tile([P, 1], F32, tag="gsc")
                        nc.vector.tensor_scalar(out=gs, in0=gfm[:, kt:kt + 1],
                                                scalar1=scale, scalar2=None,
                                                op0=OP.mult)
                        nc.vector.tensor_scalar(out=dst[:, c0:c0 + cw],
                                                in0=stg[:, c0:c0 + cw],
                                                scalar1=gs,
                                                scalar2=None, op0=OP.mult)
                    elif gfm is not None:
                        r = (kt + c0 // 1024) % 3
                        if r == 0:
                            nc.vector.tensor_scalar(out=dst[:, c0:c0 + cw],
                                                    in0=stg[:, c0:c0 + cw],
                                                    scalar1=gfm[:, kt:kt + 1],
                                                    scalar2=None, op0=OP.mult)
                        elif r == 1:
                            nc.scalar.activation(out=dst[:, c0:c0 + cw],
                                                 in_=stg[:, c0:c0 + cw],
                                                 func=AF.Copy,
                                                 scale=gfm[:, kt:kt + 1])
                        else:
                            nc.gpsimd.tensor_scalar(out=dst[:, c0:c0 + cw],
                                                    in0=stg[:, c0:c0 + cw],
                                                    scalar1=gfm[:, kt:kt + 1],
                                                    scalar2=None, op0=OP.mult)
                    elif scale is not None:
                        if eng is nc.vector:
                            nc.vector.tensor_scalar(out=dst[:, c0:c0 + cw],
                                                    in0=stg[:, c0:c0 + cw],
                                                    scalar1=scale, scalar2=None,
                                                    op0=OP.mult)
                        else:
                            nc.scalar.activation(out=dst[:, c0:c0 + cw],
                                                 in_=stg[:, c0:c0 + cw],
                                                 func=AF.Copy,
                                                 scale=scale)
                    else:
                        if eng is nc.vector:
                            nc.vector.tensor_copy(out=dst[:, c0:c0 + cw],
                                                  in_=stg[:, c0:c0 + cw])
                        else:
                            nc.scalar.copy(out=dst[:, c0:c0 + cw],
                                           in_=stg[:, c0:c0 + cw])
            return tiles

        def fold_bias(pool, w_tiles, blnbf, bproj_row, ndim, tag, wscale=1.0,
                      paired=False, want_bf=True):
            """bias' = b_ln @ W' + b_proj (undoing wscale) -> [1,ndim] f32/bf16."""
            pst = ps["S"].tile([1, ndim], F32, tag="sm", name="foldps")
            if paired:
                flat = []
                for wt in w_tiles:
                    flat.append(wt[:, 0, :])
                    flat.append(wt[:, 1, :])
            else:
                flat = w_tiles
            nk = len(flat)
            for kt, wt in enumerate(flat):
                nc.tensor.matmul(out=pst, lhsT=blnbf[:, kt:kt + 1], rhs=wt,
                                 start=(kt == 0), stop=(kt == nk - 1))
            row = pool.tile([1, ndim], F32, tag="foldtmp", bufs=1)
            if wscale != 1.0:
                nc.vector.scalar_tensor_tensor(out=row, in0=pst,
                                               scalar=1.0 / wscale,
                                               in1=bproj_row, op0=OP.mult,
                                               op1=OP.add)
            else:
                nc.vector.tensor_tensor(out=row, in0=pst, in1=bproj_row,
                                        op=OP.add)
            rowbf = None
            if want_bf:
                rowbf = pool.tile([1, ndim], BF16, tag=tag + "b")
                nc.vector.tensor_copy(out=rowbf, in_=row)
            return row, rowbf

        def row_to_fm(pool, row, n, tag):
            col = pool.tile([P, n // P], F32, tag=tag)
            for kt in range(n // P):
                pt = ps["S"].tile([P, 1], F32, tag="sm", name="r2fps")
                nc.tensor.transpose(out=pt, in_=row[:, kt * P:(kt + 1) * P],
                                    identity=ident_f[0:1, 0:1])
                nc.scalar.copy(out=col[:, kt:kt + 1], in_=pt)
            return col

        # ---------------- LN stats ----------------
        RSQRT_MAGIC = 0x5F3759DF
        magic_t = const.tile([P, 1], dt.uint32)
        nc.vector.memset(magic_t, RSQRT_MAGIC)

        def ln_stats(src_tile, nt, tag, engine="dve", bufs=None):
            if bufs is None:
                _sv = small.tile
            else:
                def _sv(shape, dtype, tag=None):
                    return small.tile(shape, dtype, tag=tag, bufs=bufs,
                                      name="ln" + (tag or "t"))
            """-> (stats, rstd [P,nt], neg_ms [P,nt] = -mu*rstd).

            engine="act": rstd = exp(-0.5*ln(var+eps)) on ACT (needs nle table).
            engine="dve": Newton rsqrt entirely on DVE (table-free)."""
            stats = _sv([P, nt, 2], F32, tag=tag + "st")
            for tt in range(nt):
                bstat = _sv([P, 6], F32, tag=tag + "bn")
                nc.vector.bn_stats(out=bstat, in_=src_tile[:, tt, :])
                nc.vector.bn_aggr(out=stats[:, tt, :], in_=bstat)
            rstd = _sv([P, nt], F32, tag=tag + "rs")
            if engine == "act":
                lnv = _sv([P, nt], F32, tag=tag + "lv")
                nc.scalar.activation(out=lnv, in_=stats[:, :, 1],
                                     func=AF.Ln, bias=eps_t)
                nc.scalar.activation(out=rstd, in_=lnv, func=AF.Exp, scale=-0.5)
            else:
                ve = _sv([P, nt], F32, tag=tag + "ve")
                nc.vector.tensor_scalar(out=ve, in0=stats[:, :, 1],
                                        scalar1=EPS, scalar2=None, op0=OP.add)
                sh = _sv([P, nt], dt.uint32, tag=tag + "sh")
                nc.vector.tensor_scalar(out=sh, in0=ve[:].bitcast(dt.uint32),
                                        scalar1=1, scalar2=None,
                                        op0=OP.logical_shift_right)
                mg = bass.AP(tensor=magic_t.tensor, offset=magic_t[:].offset,
                             ap=[magic_t[:].ap[0], [0, nt]])
                nc.vector.tensor_tensor(out=rstd[:].bitcast(dt.uint32),
                                        in0=mg, in1=sh, op=OP.subtract)
                for _ in range(2):
                    t1 = _sv([P, nt], F32, tag=tag + "t1")
                    nc.vector.tensor_tensor(out=t1, in0=rstd, in1=rstd,
                                            op=OP.mult)
                    nc.vector.tensor_tensor(out=t1, in0=t1, in1=ve, op=OP.mult)
                    nc.vector.tensor_scalar(out=t1, in0=t1, scalar1=-0.5,
                                            scalar2=1.5, op0=OP.mult, op1=OP.add)
                    nc.vector.tensor_tensor(out=rstd, in0=rstd, in1=t1,
                                            op=OP.mult)
            neg_ms = _sv([P, nt], F32, tag=tag + "nm")
            nc.vector.tensor_tensor(out=neg_ms, in0=stats[:, :, 0], in1=rstd,
                                    op=OP.mult)
            nc.vector.tensor_scalar(out=neg_ms, in0=neg_ms, scalar1=-1.0,
                                    scalar2=None, op0=OP.mult)
            return stats, rstd, neg_ms

        def make_z(src_tile, nt, st, dtype=BF16, tag="z", pool=None):
            """normalize (or plain convert) src [P,nt,512] -> z [P,nt,512] dtype.
            st = (stats, rstd, neg_ms) from ln_stats, or None for plain copy.
            Splits tt's between DVE and ACT."""
            pool = pool or zpool
            z = pool.tile([P, nt, D], dtype, tag=tag + ("8" if dtype == FP8 else ""))
            for tt in range(nt):
                if st is None:
                    if tt % 2 == 0:
                        nc.vector.tensor_copy(out=z[:, tt, :],
                                              in_=src_tile[:, tt, :])
                    else:
                        nc.scalar.copy(out=z[:, tt, :], in_=src_tile[:, tt, :])
                else:
                    stats, rstd, neg_ms = st
                    if tt % 2 == 0:
                        nc.vector.tensor_scalar(out=z[:, tt, :],
                                                in0=src_tile[:, tt, :],
                                                scalar1=stats[:, tt, 0:1],
                                                scalar2=rstd[:, tt:tt + 1],
                                                op0=OP.subtract, op1=OP.mult)
                    else:
                        nc.scalar.activation(out=z[:, tt, :],
                                             in_=src_tile[:, tt, :],
                                             func=AF.Identity,
                                             bias=neg_ms[:, tt:tt + 1],
                                             scale=rstd[:, tt:tt + 1])
            return z

        identities = {BF16: ident_bf, F32: ident_f}
        if fp8_ffn or fp8_qkv or fp8_out:
            ident_f8 = const.tile([P, P], FP8)
            make_identity(nc, ident_f8)
            identities[FP8] = ident_f8

        def transpose_fm(z, nt, pool, tag, dtype=BF16):
            """z [P,nt,D] -> fm [P, FT, nt*P] via PE transposes.

            fp8 out: the PE transpose itself runs in bf16 (hw fp8-transpose
            needs a packed output layout); the PSUM->SBUF copy converts."""
            tdt = BF16 if dtype == FP8 else dtype
            ident = identities[tdt]
            fm = pool.tile([P, FT, nt * P], dtype,
                           tag=tag + ("8" if dtype == FP8 else ""))
            ngrp = (nt + 3) // 4
            for ft in range(FT):
                for g in range(ngrp):
                    n_in = min(4, nt - g * 4)
                    pt = ps["B"].tile([P, 512], tdt, tag="tr", name="trps")
                    for i in range(n_in):
                        tt = g * 4 + i
                        nc.tensor.transpose(out=pt[:, i * P:(i + 1) * P],
                                            in_=z[:, tt, ft * P:(ft + 1) * P],
                                            identity=ident)
                    if ft % 2 == 0:
                        nc.vector.tensor_copy(
                            out=fm[:, ft, g * 512:g * 512 + n_in * P],
                            in_=pt[:, 0:n_in * P])
                    else:
                        nc.scalar.copy(
                            out=fm[:, ft, g * 512:g * 512 + n_in * P],
                            in_=pt[:, 0:n_in * P])
            return fm

        # ---------------- stylization vectors (silu table) ----------------
        AC = {}
        with tc.tile_pool(name="embp", bufs=1) as embp, \
             tc.tile_pool(name="psSe", bufs=2, space="PSUM") as _psSe:
            ps["S"] = _psSe
            e_sb = embp.tile([BL, TE], F32)
            nc.sync.dma_start(out=e_sb, in_=emb_d[:])
            semb = embp.tile([P, FT, BL], BF16)     # silu(emb)^T fm
            for kt in range(FT):
                pt = ps["S"].tile([P, BL], F32, tag="sm", name="embtr")
                nc.tensor.transpose(out=pt, in_=e_sb[:, kt * P:(kt + 1) * P],
                                    identity=ident_f[0:BL, 0:BL])
                act_reg(nc.scalar.activation(out=semb[:, kt, :], in_=pt,
                                             func=AF.Silu))
            for blk in ("sa", "ca", "ffn"):
                ew = load_w_fm(embp, W[blk + "_emb_w"][:], TE, 2 * D, tag="ew")
                eb_row = load_row(embp, W[blk + "_emb_b"][:], 2 * D, tag="ebr")
                eb_bf = embp.tile([1, 2 * D], BF16, tag="ebb")
                nc.vector.tensor_copy(out=eb_bf, in_=eb_row)
                pnb = load_fm_vec(embp, W[blk + "_pnorm_b"][:], D, tag="pnb")
                e_full = embp.tile([BL, 2 * D], F32, tag="ef")
                for half in range(2):
                    pse = ps["S"].tile([BL, 512], F32, tag="sm", name="embmm")
                    nc.tensor.matmul(out=pse, lhsT=ones_row_bf[:, 0:BL],
                                     rhs=eb_bf[:, half * 512:(half + 1) * 512],
                                     start=True, stop=False)
                    for kt in range(FT):
                        nc.tensor.matmul(out=pse, lhsT=semb[:, kt, :],
                                         rhs=ew[kt][:, half * 512:(half + 1) * 512],
                                         start=False, stop=(kt == FT - 1))
                    nc.scalar.copy(out=e_full[:, half * 512:(half + 1) * 512],
                                   in_=pse)
                A = vecp.tile([P, FT, BL], F32, tag=blk + "A")
                Cs = vecp.tile([P, FT, BL], F32, tag=blk + "C")
                sh = embp.tile([P, FT, BL], F32, tag="sh")
                for kt in range(FT):
                    pt = ps["S"].tile([P, BL], F32, tag="sm", name="embtr")
                    nc.tensor.transpose(out=pt, in_=e_full[:, kt * P:(kt + 1) * P],
                                        identity=ident_f[0:BL, 0:BL])
                    nc.vector.tensor_scalar(out=A[:, kt, :], in0=pt, scalar1=1.0,
                                            scalar2=None, op0=OP.add)
                    pt2 = ps["S"].tile([P, BL], F32, tag="sm", name="embtr2")
                    nc.tensor.transpose(out=pt2,
                                        in_=e_full[:, D + kt * P:D + (kt + 1) * P],
                                        identity=ident_f[0:BL, 0:BL])
                    nc.scalar.copy(out=sh[:, kt, :], in_=pt2)
                pnb_b = bass.AP(tensor=pnb.tensor, offset=pnb[:].offset,
                                ap=[pnb[:].ap[0], pnb[:].ap[1], [0, BL]])
                nc.vector.tensor_tensor(out=Cs, in0=A, in1=pnb_b, op=OP.mult)
                nc.vector.tensor_tensor(out=Cs, in0=Cs, in1=sh, op=OP.add)
                AC[blk] = (A, Cs)
        act_break()

        # ---------------- load x (TM), bf16 residual stream ----------------
        x_sb = []
        for b in range(BL):
            xt = xpool.tile([P, NT, D], BF16, tag=f"x{b}")
            for tt in range(NT):
                nc.gpsimd.dma_start(out=xt[:, tt, :],
                                    in_=x_d[b, tt * P:(tt + 1) * P, :])
            x_sb.append(xt)

        # ================= shared attention (SA/CA) =================
        def attention(b, xhat, wq, wk, wv, qb_fm, vb_bf, kv_fm, nkv,
                      m_sb, maskb, vscale, fp8, dbg=None):
            dr = DR if fp8 else None
            escale = RWS if fp8 else 1.0
            # ---- Q projection (FM out) + exp ----
            expq = qpool.tile([P, FT, T], BF16, tag="expq")
            for mt in range(FT):
                psq0 = ps["A"].tile([P, 512], F32, tag="mm", name="qmm0")
                psq1 = ps["A"].tile([P, 512], F32, tag="mm", name="qmm1")
                if fp8:
                    for p_ in range(2):
                        lhs = wq[p_][:, :, mt * P:(mt + 1) * P]
                        nc.tensor.matmul(out=psq0, lhsT=lhs,
                                         rhs=xhat[:, 2 * p_:2 * p_ + 2, 0:512],
                                         start=(p_ == 0), stop=(p_ == 1),
                                         perf_mode=dr)
                        nc.tensor.matmul(out=psq1, lhsT=lhs,
                                         rhs=xhat[:, 2 * p_:2 * p_ + 2, 512:1024],
                                         start=(p_ == 0), stop=(p_ == 1),
                                         perf_mode=dr)
                else:
                    for kt in range(FT):
                        lhs = wq[kt][:, mt * P:(mt + 1) * P]
                        nc.tensor.matmul(out=psq0, lhsT=lhs,
                                         rhs=xhat[:, kt, 0:512],
                                         start=(kt == 0), stop=(kt == FT - 1))
                        nc.tensor.matmul(out=psq1, lhsT=lhs,
                                         rhs=xhat[:, kt, 512:1024],
                                         start=(kt == 0), stop=(kt == FT - 1))
                for th, psq in ((0, psq0), (1, psq1)):
                    act_reg(nc.scalar.activation(
                        out=expq[:, mt, th * 512:(th + 1) * 512],
                        in_=psq, func=AF.Exp, scale=escale,
                        bias=qb_fm[:, mt:mt + 1]))
            # ---- K/V streamed into einsum1; s_k via mask column ----
            e1a = ps["B"].tile([P, 512], F32, tag="e1", name="e1a")
            e1b = ps["B"].tile([P, 512], F32, tag="e1", name="e1b")
            e1t = (e1a, e1a, e1b, e1b)
            for tt in range(nkv):
                kps = ps["A"].tile([P, 512], F32, tag="mm", name="kps")
                vps = ps["A"].tile([P, 512], F32, tag="mm", name="vps")
                nc.tensor.matmul(out=vps, lhsT=ones_row_bf, rhs=vb_bf,
                                 start=True, stop=False)
                if fp8:
                    for p_ in range(2):
                        lhs = kv_fm[:, 2 * p_:2 * p_ + 2, tt * P:(tt + 1) * P]
                        nc.tensor.matmul(out=kps, lhsT=lhs, rhs=wk[p_],
                                         start=(p_ == 0), stop=(p_ == 1),
                                         perf_mode=dr)
                        nc.tensor.matmul(out=vps, lhsT=lhs, rhs=wv[p_],
                                         start=False, stop=(p_ == 1),
                                         perf_mode=dr)
                else:
                    nkt = len(wk)
                    for kt in range(nkt):
                        lhs = kv_fm[:, kt, tt * P:(tt + 1) * P]
                        nc.tensor.matmul(out=kps, lhsT=lhs, rhs=wk[kt],
                                         start=(kt == 0), stop=(kt == nkt - 1))
                        nc.tensor.matmul(out=vps, lhsT=lhs, rhs=wv[kt],
                                         start=False, stop=(kt == nkt - 1))
                expk = kvp.tile([P, 512], BF16, tag="expk")
                v_ext = kvp.tile([P, FT, 132], BF16, tag="vext")
                if maskb is not None:
                    # mask folds entirely into expk's bias (exp(-80) == 0 to
                    # fp precision): v and the s_k ones-column stay unmasked
                    act_reg(nc.scalar.activation(out=expk, in_=kps, func=AF.Exp,
                                                 scale=escale,
                                                 bias=maskb[:, tt:tt + 1]))
                    if fp8:
                        nc.scalar.activation(out=v_ext[:, :, 0:128], in_=vps,
                                             func=AF.Copy, scale=escale)
                    else:
                        nc.scalar.copy(out=v_ext[:, :, 0:128], in_=vps)
                    nc.vector.memset(v_ext[:, :, 128:129], 1.0)
                else:
                    if fp8:
                        act_reg(nc.scalar.activation(out=expk, in_=kps,
                                                     func=AF.Exp, scale=escale))
                        nc.scalar.activation(out=v_ext[:, :, 0:128], in_=vps,
                                             func=AF.Copy, scale=escale)
                    else:
                        act_reg(nc.scalar.activation(out=expk, in_=kps,
                                                     func=AF.Exp))
                        nc.scalar.copy(out=v_ext[:, :, 0:128], in_=vps)
                    nc.vector.memset(v_ext[:, :, 128:129], 1.0)
                for ft in range(FT):
                    # one accumulation group per BANK: start=True clears the
                    # whole bank's has_written bits, so only the first matmul
                    # into each bank may set it (the ft-odd range then
                    # overwrites-where-clear at tt==0 and accumulates after).
                    off = (ft % 2) * 256
                    nc.tensor.matmul(out=e1t[ft][:, off:off + 129],
                                     lhsT=expk[:, ft * P:(ft + 1) * P],
                                     rhs=v_ext[:, ft, 0:129],
                                     start=(tt == 0 and ft % 2 == 0),
                                     stop=(tt == nkv - 1 and ft % 2 == 1))
            # ---- rsk + att_bd (block-diag per head pair) ----
            rsk = small.tile([P, FT], F32, tag="rsk")
            for ft in range(FT):
                off = (ft % 2) * 256
                nc.vector.reciprocal(out=rsk[:, ft:ft + 1],
                                     in_=e1t[ft][:, off + 128:off + 129])
            att_bd = abp.tile([P, FT, P], BF16, tag="attbd")
            nc.vector.memset(att_bd, 0.0)
            for ft in range(FT):
                off = (ft % 2) * 256
                for r in range(2):
                    s = slice(64 * r, 64 * r + 64)
                    nc.vector.tensor_scalar(
                        out=att_bd[s, ft, s],
                        in0=e1t[ft][s, off + 64 * r:off + 64 * r + 64],
                        scalar1=rsk[s, ft:ft + 1], scalar2=None, op0=OP.mult)
            if dbg is not None:
                tap(dbg + "_attbd", att_bd[:])
                tap(dbg + "_expq", expq[:])
            # ---- einsum2 + s_q + normalize -> y TM ----
            y = ypool.tile([P, NT, D], BF16, tag="y")
            sqps = ps["S"].tile([P, NT, H], F32, tag="sm", name="sqps")
            for tt in range(NT):
                yps = ps["A"].tile([P, 512], F32, tag="mm", name="ymm")
                for ft in range(FT):
                    lhs = expq[:, ft, tt * P:(tt + 1) * P]
                    nc.tensor.matmul(out=yps[:, ft * P:(ft + 1) * P], lhsT=lhs,
                                     rhs=att_bd[:, ft, :], start=True, stop=True)
                    nc.tensor.matmul(out=sqps[:, tt, 2 * ft:2 * ft + 2], lhsT=lhs,
                                     rhs=sel_bf, start=True, stop=True)
                rsq = small.tile([P, H], F32, tag="rsq")
                nc.vector.reciprocal(out=rsq, in_=sqps[:, tt, :])
                nc.vector.tensor_tensor(
                    out=y[:, tt, :].rearrange("p (g d) -> p g d", g=H),
                    in0=yps[:].rearrange("p (g d) -> p g d", g=H),
                    in1=bass.AP(tensor=rsq.tensor, offset=rsq[:].offset,
                                ap=[rsq[:].ap[0], rsq[:].ap[1], [0, DH]]),
                    op=OP.mult)
            return y

        # ---- stylize split: pre (nle table) / post (silu table) ----
        def stylize_pre(y, engine=None):
            if engine is None:
                engine = "dve" if pre_dve else "act"
            st = ln_stats(y, NT, tag="pn", engine=engine)
            zy = make_z(y, NT, st)
            return transpose_fm(zy, NT, zfmp, tag="zfm")

        def stylize_post(blk, b, zfm, wo, ob_bf, fp8, final=False):
            A, Cs = AC[blk]
            sdt = FP8 if fp8 else BF16
            for th in range(2):
                sfm = sfmp.tile([P, FT, 512], sdt, tag="sfm", bufs=2)
                for ft in range(FT):
                    act_reg(nc.scalar.activation(
                        out=sfm[:, ft, :],
                        in_=zfm[:, ft, th * 512:(th + 1) * 512],
                        func=AF.Silu, scale=A[:, ft, b:b + 1],
                        bias=Cs[:, ft, b:b + 1]))
                for ti in range(4):
                    tt = th * 4 + ti
                    ops = ps["A"].tile([P, 512], F32, tag="mm", name="omm")
                    nc.tensor.matmul(out=ops, lhsT=ones_row_bf, rhs=ob_bf,
                                     start=True, stop=False)
                    if final:
                        # last residual: evict in f32 straight to output DMA
                        dst = xop.tile([P, 512], F32, tag="xo", name="xo")
                    else:
                        dst = x_sb[b][:, tt, :]
                    if fp8:
                        for p_ in range(2):
                            nc.tensor.matmul(
                                out=ops,
                                lhsT=sfm[:, 2 * p_:2 * p_ + 2,
                                         ti * P:(ti + 1) * P],
                                rhs=wo[p_], start=False, stop=(p_ == 1),
                                perf_mode=DR)
                        nc.vector.scalar_tensor_tensor(
                            out=dst, in0=ops, scalar=RWS,
                            in1=x_sb[b][:, tt, :], op0=OP.mult, op1=OP.add)
                    else:
                        for ft in range(FT):
                            nc.tensor.matmul(out=ops,
                                             lhsT=sfm[:, ft, ti * P:(ti + 1) * P],
                                             rhs=wo[ft], start=False,
                                             stop=(ft == FT - 1))
                        nc.vector.tensor_tensor(out=dst, in0=ops,
                                                in1=x_sb[b][:, tt, :],
                                                op=OP.add)
                    if final:
                        nc.sync.dma_start(out=out_d[b, tt * P:(tt + 1) * P, :],
                                          in_=dst)

        # ================= SA phase =================
        wdt = FP8 if fp8_qkv else BF16
        wodt = FP8 if fp8_out else BF16
        wsc = WS if fp8_qkv else None
        wosc = WS if fp8_out else None
        with tc.tile_pool(name="psA_sa", bufs=3, space="PSUM") as _pa, \
             tc.tile_pool(name="psB_sa", bufs=2, space="PSUM") as _pb, \
             tc.tile_pool(name="psS_sa", bufs=1, space="PSUM") as _psx, \
             tc.tile_pool(name="wmain", bufs=1) as wmain, \
             tc.tile_pool(name="wrow", bufs=1) as wrow, \
             tc.tile_pool(name="cap", bufs=1) as cap:
            ps["A"], ps["B"], ps["S"] = _pa, _pb, _psx

            def prep_attn(pre):
                """Load+fold one attention block's weights. pre in {sa, ca}."""
                g_fm = load_fm_vec(wrow, W[pre + "_norm_g"][:], D, tag="g")
                bln = load_fm_vec(wrow, W[pre + "_norm_b"][:], D, tag="bln")
                if pre == "ca":
                    tg_fm = load_fm_vec(wrow, W["ca_tnorm_g"][:], L, tag="tg")
                    tbln = load_fm_vec(wrow, W["ca_tnorm_b"][:], L, tag="tbln")
                else:
                    tg_fm, tbln = g_fm, bln
                kvdim = L if pre == "ca" else D
                wq = load_w_fm(wmain, W[pre + "_q_w"][:], D, D, gfm=g_fm,
                               tag="wq", dtype=wdt, scale=wsc, paired=fp8_qkv)
                wk = load_w_fm(wmain, W[pre + "_k_w"][:], kvdim, D, gfm=tg_fm,
                               tag="wk", dtype=wdt, scale=wsc, paired=fp8_qkv)
                wv = load_w_fm(wmain, W[pre + "_v_w"][:], kvdim, D, gfm=tg_fm,
                               tag="wv", dtype=wdt, scale=wsc, paired=fp8_qkv)
                wo = load_w_fm(wmain, W[pre + "_out_w"][:], D, D, tag="wo",
                               dtype=wodt, scale=wosc, paired=fp8_out, bufs=1)
                bln_c = wrow.tile([P, FT], wdt, tag="blnb")
                tbln_c = wrow.tile([P, FT], wdt, tag="tblnb")
                bsc = WS if fp8_qkv else 1.0
                nc.vector.tensor_scalar(out=bln_c, in0=bln, scalar1=bsc,
                                        scalar2=None, op0=OP.mult)
                nc.vector.tensor_scalar(out=tbln_c, in0=tbln, scalar1=bsc,
                                        scalar2=None, op0=OP.mult)
                wsq = (WS * WS) if fp8_qkv else 1.0
                qb_row = load_row(wrow, W[pre + "_q_b"][:], D, tag="rowtmp",
                                  bufs=2)
                vb_row = load_row(wrow, W[pre + "_v_b"][:], D, tag="rowtmp",
                                  bufs=2)
                qb_row2, _ = fold_bias(wrow, wq, bln_c, qb_row, D, tag="qbf",
                                       wscale=wsq, paired=fp8_qkv,
                                       want_bf=False)
                qb_fm = row_to_fm(wrow, qb_row2, D, tag="qbfm")
                ob_row = load_row(wrow, W[pre + "_out_b"][:], D, tag="rowtmp",
                                  bufs=2)
                vb_row2, vb_bf = fold_bias(wrow, wv, tbln_c, vb_row, D,
                                           tag="vbf", wscale=wsq,
                                           paired=fp8_qkv,
                                           want_bf=not fp8_qkv)
                if fp8_qkv:
                    vb_sc = wrow.tile([1, D], BF16, tag="vbfb")
                    nc.vector.tensor_scalar(out=vb_sc, in0=vb_row2, scalar1=WS,
                                            scalar2=None, op0=OP.mult)
                    vb_bf = vb_sc
                ob_bf = wrow.tile([1, D], BF16, tag="obbf")
                if fp8_out:
                    nc.vector.tensor_scalar(out=ob_bf, in0=ob_row, scalar1=WS,
                                            scalar2=None, op0=OP.mult)
                else:
                    nc.vector.tensor_copy(out=ob_bf, in_=ob_row)
                return wq, wk, wv, wo, qb_fm, vb_bf, ob_bf

            # SA x stats: x_sb fresh from DMA, compute during weight prep
            sts = [ln_stats(x_sb[b], NT, tag=f"sxln{b}", bufs=1)
                   for b in range(BL)]
            for pre in ("sa", "ca"):
                wq, wk, wv, wo, qb_fm, vb_bf, ob_bf = prep_attn(pre)
                zfms = []
                ys = {}

                def stage_a(b):
                    if pre == "sa":
                        m_sb = small.tile([P, NT], F32, tag="msb",
                                          name="m_sb")
                        for tt in range(NT):
                            nc.sync.dma_start(out=m_sb[:, tt:tt + 1],
                                              in_=mask_d[b, tt * P:(tt + 1) * P, :])
                        maskb = small.tile([P, NT], F32, tag="mbias",
                                           name="maskb")
                        nc.vector.tensor_scalar(out=maskb, in0=m_sb,
                                                scalar1=-1.0, scalar2=-MASK_NEG,
                                                op0=OP.add, op1=OP.mult)
                        if fp8_qkv:
                            vscale = small.tile([P, NT], F32, tag="msc",
                                                name="vscale")
                            nc.vector.tensor_scalar(out=vscale, in0=m_sb,
                                                    scalar1=RWS, scalar2=None,
                                                    op0=OP.mult)
                        else:
                            vscale = m_sb
                        kv_src, nkv = None, NT
                    else:
                        m_sb = maskb = vscale = None
                        xf_sb = cap.tile([P, NTC, L], F32, tag="xfsb", bufs=1,
                                         name="xf_sb")
                        for tt in range(NTC):
                            nc.sync.dma_start(out=xf_sb[:, tt, :],
                                              in_=xf_d[b, tt * P:(tt + 1) * P, :])
                        tst = ln_stats(xf_sb, NTC, tag="tln")
                        ztn = make_z(xf_sb, NTC, tst, tag="zt", pool=cap)
                        kv_src = transpose_fm(ztn, NTC, cap, tag="tnfm",
                                              dtype=wdt)
                        nkv = NTC
                    zx = make_z(x_sb[b], NT, sts[b])
                    xhat = transpose_fm(zx, NT, fmp, tag="fm", dtype=wdt)
                    if kv_src is None:
                        kv_src = xhat
                    return attention(b, xhat, wq, wk, wv, qb_fm, vb_bf,
                                     kv_src, nkv, m_sb, maskb, vscale,
                                     fp8_qkv, dbg=(pre if b == 0 else None))

                # software-pipelined: stylize_pre(b-1)'s serial DVE stats are
                # emitted after batch b's PE-heavy attention work
                for b in range(BL):
                    ys[b] = stage_a(b)
                    if b == 0:
                        tap(pre + "_y", ys[b][:])
                    if b >= 1:
                        zfms.append(stylize_pre(ys.pop(b - 1)))
                zfms.append(stylize_pre(ys.pop(BL - 1)))
                act_break()   # exps before silus
                nxt = []
                for b in range(BL):
                    stylize_post(pre, b, zfms[b], wo, ob_bf, fp8_out)
                    if pre == "sa":
                        # CA x stats right behind each residual update
                        nxt.append(ln_stats(x_sb[b], NT, tag=f"cxln{b}",
                                            bufs=1))
                    if b == 0:
                        tap("x_after_" + pre, x_sb[b][:])
                act_break()   # silus before next phase's exps
                sts = nxt

        # ================= FFN phase =================
        fdt = FP8 if fp8_ffn else BF16
        fsc = WS if fp8_ffn else None
        with tc.tile_pool(name="ffn_w", bufs=1) as wp, \
             tc.tile_pool(name="gelu_p", bufs=2) as gp, \
             tc.tile_pool(name="psA_f", bufs=2, space="PSUM") as _pa, \
             tc.tile_pool(name="psB_f", bufs=2, space="PSUM") as _pb, \
             tc.tile_pool(name="psyf", bufs=1, space="PSUM") as psyf:
            ps["A"], ps["B"], ps["S"] = _pa, _pb, _pb
            w1 = load_w_fm(wp, W["ffn_w1"][:], D, FF, tag="w1", dtype=fdt,
                           scale=fsc, paired=fp8_ffn)
            w2 = load_w_fm(wp, W["ffn_w2"][:], FF, D, tag="w2", dtype=fdt,
                           scale=fsc, paired=fp8_ffn)
            b1_fm = load_fm_vec(wp, W["ffn_b1"][:], FF, tag="b1")
            b2_row = load_row(wp, W["ffn_b2"][:], D, tag="rowtmp", bufs=1)
            b2_bf = wp.tile([1, D], BF16, tag="b2b")
            if fp8_ffn:
                nc.vector.tensor_scalar(out=b2_bf, in0=b2_row, scalar1=WS,
                                        scalar2=None, op0=OP.mult)
            else:
                nc.vector.tensor_copy(out=b2_bf, in_=b2_row)
            wo = load_w_fm(wp, W["ffn_out_w"][:], D, D, tag="wo", dtype=wodt,
                           scale=wosc, paired=fp8_out)
            ob_row = load_row(wp, W["ffn_out_b"][:], D, tag="rowtmp", bufs=1)
            ob_bf = wp.tile([1, D], BF16, tag="obbf")
            if fp8_out:
                nc.vector.tensor_scalar(out=ob_bf, in0=ob_row, scalar1=WS,
                                        scalar2=None, op0=OP.mult)
            else:
                nc.vector.tensor_copy(out=ob_bf, in_=ob_row)

            zfms = []
            ys = {}
            for b in range(BL):
                zx = make_z(x_sb[b], NT, None, tag="z")
                x_fm = transpose_fm(zx, NT, fmp, tag="fm", dtype=fdt)
                y = ypool.tile([P, NT, D], BF16, tag="y")
                gsc = RWS if fp8_ffn else None
                for th in range(2):
                    yps = [psyf.tile([P, 512], F32, tag=f"yf{i}",
                                     name=f"yf{i}") for i in range(4)]
                    for i in range(4):
                        nc.tensor.matmul(out=yps[i], lhsT=ones_row_bf,
                                         rhs=b2_bf, start=True, stop=False)
                    nmt = FF // P
                    for mt in range(nmt):
                        gps = ps["A"].tile([P, 512], F32, tag="mm", name="gmm")
                        if fp8_ffn:
                            for p_ in range(2):
                                nc.tensor.matmul(
                                    out=gps,
                                    lhsT=w1[p_][:, :, mt * P:(mt + 1) * P],
                                    rhs=x_fm[:, 2 * p_:2 * p_ + 2,
                                             th * 512:(th + 1) * 512],
                                    start=(p_ == 0), stop=(p_ == 1),
                                    perf_mode=DR)
                        else:
                            for kt in range(FT):
                                nc.tensor.matmul(
                                    out=gps, lhsT=w1[kt][:, mt * P:(mt + 1) * P],
                                    rhs=x_fm[:, kt, th * 512:(th + 1) * 512],
                                    start=(kt == 0), stop=(kt == FT - 1))
                        if fp8_ffn:
                            if mt % 2 == 0:
                                gpair = gp.tile([P, 2, 512], FP8, tag="g")
                            act_reg(nc.scalar.activation(
                                out=gpair[:, mt % 2, :], in_=gps, func=AF.Gelu,
                                scale=RWS, bias=b1_fm[:, mt:mt + 1]))
                            if mt % 2 == 1:
                                for i in range(4):
                                    nc.tensor.matmul(
                                        out=yps[i],
                                        lhsT=gpair[:, :, i * P:(i + 1) * P],
                                        rhs=w2[mt // 2], start=False,
                                        stop=(mt == nmt - 1), perf_mode=DR)
                        else:
                            gsb = gp.tile([P, 512], BF16, tag="g")
                            act_reg(nc.scalar.activation(
                                out=gsb, in_=gps, func=AF.Gelu,
                                bias=b1_fm[:, mt:mt + 1]))
                            for i in range(4):
                                nc.tensor.matmul(
                                    out=yps[i], lhsT=gsb[:, i * P:(i + 1) * P],
                                    rhs=w2[mt], start=False,
                                    stop=(mt == nmt - 1))
                    for i in range(4):
                        tt = th * 4 + i
                        if fp8_ffn:
                            if i % 2 == 0:
                                nc.vector.tensor_scalar(out=y[:, tt, :],
                                                        in0=yps[i], scalar1=RWS,
                                                        scalar2=None,
                                                        op0=OP.mult)
                            else:
                                nc.scalar.activation(out=y[:, tt, :],
                                                     in_=yps[i], func=AF.Copy,
                                                     scale=RWS)
                        elif i % 2 == 0:
                            nc.vector.tensor_copy(out=y[:, tt, :], in_=yps[i])
                        else:
                            nc.scalar.copy(out=y[:, tt, :], in_=yps[i])
                if b == 0:
                    tap("ffn_y", y[:])
                ys[b] = y
                if b >= 1:
                    zfms.append(stylize_pre(ys.pop(b - 1), engine="dve"))
            zfms.append(stylize_pre(ys.pop(BL - 1), engine="dve"))
            act_break()   # gelus before final silus
            for b in range(BL):
                stylize_post("ffn", b, zfms[b], wo, ob_bf, fp8_out, final=True)

    nc.compile()
    return nc, tap_tensors
# ======================= runner =======================


def make_runner(nc, n_cores=8):
    from concourse.bass2jax import (_bass_exec_p, install_neuronx_cc_hook,
                                    partition_id_tensor)
    from jax.sharding import Mesh, PartitionSpec
    from jax.experimental.shard_map import shard_map
    install_neuronx_cc_hook()
    partition_name = nc.partition_id_tensor.name if nc.partition_id_tensor else None
    in_names, out_names, out_avals, zero_outs = [], [], [], []
    for alloc in nc.m.functions[0].allocations:
        if not isinstance(alloc, mybir.MemoryLocationSet):
            continue
        name = alloc.memorylocations[0].name
        if alloc.kind == "ExternalInput":
            if name != partition_name:
                in_names.append(name)
        elif alloc.kind == "ExternalOutput":
            out_names.append(name)
            shape = tuple(alloc.tensor_shape)
            dtype = mybir.dt.np(alloc.dtype)
            out_avals.append(jax.core.ShapedArray(shape, dtype))
            zero_outs.append(np.zeros(shape, dtype))
    n_params = len(in_names)
    in_names_full = list(in_names) + out_names + ([partition_name] if partition_name else [])

    def _body(*args):
        operands = list(args)
        if partition_name is not None:
            operands.append(partition_id_tensor())
        return tuple(_bass_exec_p.bind(
            *operands, out_avals=tuple(out_avals), in_names=tuple(in_names_full),
            out_names=tuple(out_names), lowering_input_output_aliases=(),
            sim_require_finite=False, sim_require_nnan=False, nc=nc))

    devices = jax.devices()[:n_cores]
    mesh = Mesh(np.asarray(devices), ("core",))
    in_specs = (PartitionSpec("core"),) * (n_params + len(out_names))
    out_specs = (PartitionSpec("core"),) * len(out_names)
    sharded = jax.jit(shard_map(_body, mesh=mesh, in_specs=in_specs,
                                out_specs=out_specs, check_rep=False),
                      keep_unused=True)

    class Runner:
        def __init__(self):
            self.sharded = sharded
            self.in_names = in_names
            self.out_names = out_names
            self.zero_outs = zero_outs
            self.n_cores = n_cores

        def upload(self, in_maps):
            '''Pre-place inputs on device; returns device arg list.'''
            from jax.sharding import NamedSharding, PartitionSpec
            concat_in = [np.concatenate([np.asarray(in_maps[c][n])
                                         for c in range(self.n_cores)], axis=0)
                         for n in self.in_names]
            concat_zeros = [np.zeros((self.n_cores * z.shape[0], *z.shape[1:]),
                                     z.dtype) for z in self.zero_outs]
            sh = NamedSharding(mesh, PartitionSpec("core"))
            args = [jax.device_put(a, sh) for a in concat_in + concat_zeros]
            jax.block_until_ready(args)
            return args

        def run_dev(self, args):
            outs = sharded(*args)
            jax.block_until_ready(outs)
            return outs

        def __call__(self, in_maps):
            args = self.upload(in_maps)
            outs = self.run_dev(args)
            return [{name: np.asarray(outs[i]).reshape(self.n_cores,
                                                       *self.zero_outs[i].shape)[c]
                     for i, name in enumerate(self.out_names)}
                    for c in range(self.n_cores)]
    return Runner()


# ======================= public entry point =======================
_CACHE = {}
N_CORES = 8
B_FULL = 32
NB = B_FULL // N_CORES


import os
_FP8_FFN = os.environ.get("KERNEL_FP8_FFN", "1") == "1"
_FP8_QKV = os.environ.get("KERNEL_FP8_QKV", "0") == "1"
_FP8_OUT = os.environ.get("KERNEL_FP8_OUT", "0") == "1"


def _get_runner():
    if "runner" not in _CACHE:
        nc, _ = build(n_batch=NB, taps=(), fp8_ffn=_FP8_FFN,
                      fp8_qkv=_FP8_QKV, fp8_out=_FP8_OUT)
        _CACHE["runner"] = make_runner(nc, n_cores=N_CORES)
    return _CACHE["runner"]


def make_in_maps(inputs):
    """Pack the 41 weight tensors into one flat array + shard batch inputs."""
    inp = {k: np.ascontiguousarray(np.asarray(v, dtype=np.float32))
           for k, v in inputs.items()}
    wflat = np.ascontiguousarray(
        np.concatenate([inp[n].ravel() for n in PARAM_SHAPES])[None, :],
        dtype=np.float32)
    in_maps = []
    for c in range(N_CORES):
        m = {k: inp[k][c * NB:(c + 1) * NB]
             for k in ("x", "xf", "emb", "src_mask")}
        m["wflat"] = wflat
        in_maps.append(m)
    return in_maps


def kernel(**inputs) -> np.ndarray:
    runner = _get_runner()
    res = runner(make_in_maps(inputs))
    out = np.concatenate([res[c]["out"] for c in range(N_CORES)], axis=0)
    return out.astype(np.float32)



# revision 53
# speedup vs baseline: 3.4145x; 1.2340x over previous
"""Trainium2 Bass kernel for nn_LinearTemporalDiffusionTransformerDecoderLayer.

v3 (on top of v2's pass-restructured phases / streamed K/V einsum1):
 - fp8 (e4m3, DoubleRow) FFN GEMMs with x64 weight pre-scale (halves the
   biggest PE load; rel_err 0.0137 vs the 2e-2 gate, deterministic inputs)
 - table-free DVE Newton-rsqrt for every LayerNorm (no natural_log ACT
   table ping-pong) + scheduler-only ordering edges that group exp/silu/gelu
   activations per phase: 69 -> 7 ACT table loads (~2.7us each on HW)
 - software-pipelined batch loop: stylize_pre(b-1)'s serial DVE stats chains
   are emitted after batch b's PE-heavy attention work, so PE stays fed
 - cross-phase LN stats hoisting (CA x-stats emitted right behind each SA
   residual update)
 - weight loads via gpsimd cast-on-DMA (f32->bf16) with parallel whole-row
   DMAs; scaled/fp8 weights convert from a bf16 stage (no f32 stage ping-pong)
 - bf16 residual stream (halves x DMA + SBUF; final residual evicts f32
   straight to the output DMA so the last rounding is not taken)

TimelineSim cost model: 602us/core vs 910us for v2 (HW slope-timing is
tunnel-noise-limited; same-window A/B showed 876us vs ~1420us).

Self-contained: builds and compiles an 8-core SPMD Bass program on first call,
shards the batch dimension (B=32 -> 8 cores x 4), executes via PJRT, and
reassembles the full output.  kernel(**inputs) -> np.ndarray [32, 1024, 512].
"""
import sys
for _p in ("/opt/trn_rl_repo",):
    if _p not in sys.path:
        sys.path.insert(0, _p)
import numpy as np
import jax
import concourse.bass as bass
import concourse.bacc as bacc
import concourse.tile as tile
from concourse import mybir
from concourse.tile_rust import add_dep_helper
from concourse.masks import make_identity
from contextlib import ExitStack

dt = mybir.dt
F32, BF16 = dt.float32, dt.bfloat16
FP8 = dt.float8e4
AF = mybir.ActivationFunctionType
OP = mybir.AluOpType
DR = mybir.MatmulPerfMode.DoubleRow
P = 128
T, D, H, DH = 1024, 512, 8, 64
NCTX, L, TE, FF = 256, 512, 512, 2048
NT, FT = T // P, D // P          # 8 token tiles, 4 feature tiles
NTC = NCTX // P                  # 2 ctx tiles
EPS = 1e-5
MASK_NEG = -80.0
WS = 64.0                        # fp8 weight pre-scale
RWS = 1.0 / WS

PARAM_SHAPES = {
    "sa_norm_g": (D,), "sa_norm_b": (D,),
    "sa_q_w": (D, D), "sa_q_b": (D,), "sa_k_w": (D, D), "sa_k_b": (D,),
    "sa_v_w": (D, D), "sa_v_b": (D,),
    "sa_emb_w": (TE, 2 * D), "sa_emb_b": (2 * D,),
    "sa_pnorm_g": (D,), "sa_pnorm_b": (D,),
    "sa_out_w": (D, D), "sa_out_b": (D,),
    "ca_norm_g": (D,), "ca_norm_b": (D,), "ca_tnorm_g": (L,), "ca_tnorm_b": (L,),
    "ca_q_w": (D, D), "ca_q_b": (D,), "ca_k_w": (L, D), "ca_k_b": (D,),
    "ca_v_w": (L, D), "ca_v_b": (D,),
    "ca_emb_w": (TE, 2 * D), "ca_emb_b": (2 * D,),
    "ca_pnorm_g": (D,), "ca_pnorm_b": (D,),
    "ca_out_w": (D, D), "ca_out_b": (D,),
    "ffn_w1": (D, FF), "ffn_b1": (FF,), "ffn_w2": (FF, D), "ffn_b2": (D,),
    "ffn_emb_w": (TE, 2 * D), "ffn_emb_b": (2 * D,),
    "ffn_pnorm_g": (D,), "ffn_pnorm_b": (D,),
    "ffn_out_w": (D, D), "ffn_out_b": (D,),
}


def build(n_batch=4, taps=(), fp8_ffn=False, fp8_qkv=False, fp8_out=False,
          act_group=True, pre_dve=True):
    BL = n_batch
    nc = bacc.Bacc(None, target_bir_lowering=False, debug=False)
    tap_tensors = {}

    # the 4 sharded activations packed into one dram array (fewer dispatch args)
    n_x, n_xf, n_e, n_m = BL * T * D, BL * NCTX * L, BL * TE, BL * T
    inflat_d = nc.declare_dram_parameter("inflat", [1, n_x + n_xf + n_e + n_m],
                                         F32, isOutput=False)
    o1, o2, o3 = n_x, n_x + n_xf, n_x + n_xf + n_e
    x_d = inflat_d[0, 0:n_x].rearrange("(b t d) -> b t d", b=BL, t=T)
    xf_d = inflat_d[0, o1:o1 + n_xf].rearrange("(b t d) -> b t d", b=BL, t=NCTX)
    emb_d = inflat_d[0, o2:o2 + n_e].rearrange("(b e) -> b e", b=BL)
    mask_d = inflat_d[0, o3:o3 + n_m].rearrange("(b t d) -> b t d", b=BL, t=T)
    # all 41 weight tensors packed into one dram array: per-call dispatch
    # marshaling scales with arg count, so 47 args -> 6
    total_w = sum(int(np.prod(s)) for s in PARAM_SHAPES.values())
    wflat_d = nc.declare_dram_parameter("wflat", [1, total_w], F32,
                                        isOutput=False)
    W = {}
    off = 0
    for name, shp in PARAM_SHAPES.items():
        sz = int(np.prod(shp))
        apw = wflat_d[0, off:off + sz]
        if len(shp) == 2:
            apw = apw.rearrange("(a b) -> a b", a=shp[0])
        W[name] = apw
        off += sz
    out_d = nc.declare_dram_parameter("out", [BL, T, D], F32, isOutput=True)

    with tile.TileContext(nc) as tc, ExitStack() as root:
        const = root.enter_context(tc.tile_pool(name="const", bufs=1))
        small = root.enter_context(tc.tile_pool(name="small", bufs=2))

        xpool = root.enter_context(tc.tile_pool(name="xpool", bufs=1))
        zpool = root.enter_context(tc.tile_pool(name="zpool", bufs=2))
        fmp = root.enter_context(tc.tile_pool(name="fmp", bufs=2))
        qpool = root.enter_context(tc.tile_pool(name="qpool", bufs=2))
        kvp = root.enter_context(tc.tile_pool(name="kvp", bufs=2))
        ypool = root.enter_context(tc.tile_pool(name="ypool", bufs=2))
        zfmp = root.enter_context(tc.tile_pool(name="zfmp", bufs=4))
        sfmp = root.enter_context(tc.tile_pool(name="sfmp", bufs=1))
        abp = root.enter_context(tc.tile_pool(name="abp", bufs=2))
        vecp = root.enter_context(tc.tile_pool(name="vecp", bufs=1))
        xop = root.enter_context(tc.tile_pool(name="xop", bufs=3))
        ps = {}

        # ---- ACT table grouping: scheduler-only ordering edges keep the
        # scalar engine's table-switching funcs (exp/silu/gelu) from
        # interleaving across phases (each switch = ~2.7us table reload) ----
        act_seq = {"prev": None, "cur": []}

        def act_reg(bi):
            if not act_group:
                return bi
            if act_seq["prev"] is not None:
                add_dep_helper(bi.ins, act_seq["prev"], sync=False,
                               reason="act-table-group")
            act_seq["cur"].append(bi)
            return bi

        def act_break():
            if not act_group or not act_seq["cur"]:
                return
            j = nc.scalar.nop()
            for bi in act_seq["cur"]:
                add_dep_helper(j.ins, bi.ins, sync=False,
                               reason="act-table-junction")
            act_seq["prev"] = j.ins
            act_seq["cur"] = []

        def tap(name, ap):
            if name in taps and name not in tap_tensors:
                tdn = "tap_" + name
                td = nc.declare_dram_parameter(tdn, list(ap.shape),
                                               ap.dtype, isOutput=True)
                nc.sync.dma_start(out=td[:], in_=ap)
                tap_tensors[name] = (tdn, tuple(ap.shape))

        # ---------------- constants ----------------
        ident_bf = const.tile([P, P], BF16)
        make_identity(nc, ident_bf)
        ident_f = const.tile([P, P], F32)
        make_identity(nc, ident_f)
        ones_row_bf = const.tile([1, P], BF16)       # K=1 lhsT for row broadcast
        nc.vector.memset(ones_row_bf, 1.0)
        sel_bf = const.tile([P, 2], BF16)            # head-pair selector
        nc.vector.memset(sel_bf, 0.0)
        nc.vector.memset(sel_bf[0:64, 0:1], 1.0)
        nc.vector.memset(sel_bf[64:128, 1:2], 1.0)
        eps_t = const.tile([P, 1], F32)
        nc.vector.memset(eps_t, EPS)

        # ---------------- load helpers ----------------
        def load_fm_vec(pool, ap1d, n=D, tag=None):
            nkt = n // P
            t = pool.tile([P, nkt], F32, tag=tag)
            nc.sync.dma_start(out=t, in_=ap1d.rearrange("(k p) -> p k", p=P))
            return t

        def load_row(pool, ap1d, n, tag=None, bufs=None):
            kw = {} if bufs is None else {"bufs": bufs}
            t = pool.tile([1, n], F32, tag=tag, **kw)
            nc.sync.dma_start(out=t, in_=ap1d.rearrange("(a n) -> a n", a=1))
            return t

        def load_w_fm(pool, wap, kdim, ndim, gfm=None, tag="w", dtype=BF16,
                      scale=None, paired=False, bufs=None):
            """Load weight [kdim, ndim] -> list of FM tiles.

            paired=False: kdim//P tiles of [P, ndim].
            paired=True: kdim//(2P) tiles of [P, 2, ndim] (DoubleRow k-pairs).
            gfm: per-row (input-feature) scale column tile [P, kdim//P].
            scale: extra scalar premultiplier (fp8 range scaling).
            """
            nkt = kdim // P
            tiles = []
            kw = {} if bufs is None else {"bufs": bufs}
            for kt in range(nkt):
                if paired:
                    if kt % 2 == 0:
                        wt = pool.tile([P, 2, ndim], dtype,
                                       tag=f"{tag}{kt // 2}", **kw)
                        tiles.append(wt)
                    dst = tiles[-1][:, kt % 2, :]
                else:
                    wt = pool.tile([P, ndim], dtype, tag=f"{tag}{kt}", **kw)
                    tiles.append(wt)
                    dst = wt
                if gfm is None and scale is None and dtype == BF16:
                    # cast-on-DMA (software DGE): skip the stage+convert hop
                    nc.gpsimd.dma_start(out=dst,
                                        in_=wap[kt * P:(kt + 1) * P, :])
                    continue
                # cast-on-DMA to a bf16 stage, then scale/convert in SBUF.
                # One whole-row DMA per k-tile: parallel DMA fan-out, no
                # per-chunk stage ping-pong.
                stg = pool.tile([P, ndim], BF16, tag=f"wstg{ndim}",
                                name="wstg", bufs=2)
                nc.gpsimd.dma_start(out=stg, in_=wap[kt * P:(kt + 1) * P, :])
                for c0 in range(0, ndim, 1024):
                    cw = min(1024, ndim - c0)
                    eng = nc.vector if (kt + c0 // 1024) % 2 == 0 else nc.scalar
                    if gfm is not None and scale is not None:
                        gs = small.tile([P, 1], F32, tag="gsc")
                        nc.vector.tensor_scalar(out=gs, in0=gfm[:, kt:kt + 1],
                                                scalar1=scale, scalar2=None,
                                                op0=OP.mult)
                        nc.vector.tensor_scalar(out=dst[:, c0:c0 + cw],
                                                in0=stg[:, c0:c0 + cw],
                                                scalar1=gs,
                                                scalar2=None, op0=OP.mult)
                    elif gfm is not None:
                        r = (kt + c0 // 1024) % 3
                        if r == 0:
                            nc.vector.tensor_scalar(out=dst[:, c0:c0 + cw],
                                                    in0=stg[:, c0:c0 + cw],
                                                    scalar1=gfm[:, kt:kt + 1],
                                                    scalar2=None, op0=OP.mult)
                        elif r == 1:
                            nc.scalar.activation(out=dst[:, c0:c0 + cw],
                                                 in_=stg[:, c0:c0 + cw],
                                                 func=AF.Copy,
                                                 scale=gfm[:, kt:kt + 1])
                        else:
                            nc.gpsimd.tensor_scalar(out=dst[:, c0:c0 + cw],
                                                    in0=stg[:, c0:c0 + cw],
                                                    scalar1=gfm[:, kt:kt + 1],
                                                    scalar2=None, op0=OP.mult)
                    elif scale is not None:
                        if eng is nc.vector:
                            nc.vector.tensor_scalar(out=dst[:, c0:c0 + cw],
                                                    in0=stg[:, c0:c0 + cw],
                                                    scalar1=scale, scalar2=None,
                                                    op0=OP.mult)
                        else:
                            nc.scalar.activation(out=dst[:, c0:c0 + cw],
                                                 in_=stg[:, c0:c0 + cw],
                                                 func=AF.Copy,
                                                 scale=scale)
                    else:
                        if eng is nc.vector:
                            nc.vector.tensor_copy(out=dst[:, c0:c0 + cw],
                                                  in_=stg[:, c0:c0 + cw])
                        else:
                            nc.scalar.copy(out=dst[:, c0:c0 + cw],
                                           in_=stg[:, c0:c0 + cw])
            return tiles

        def fold_bias(pool, w_tiles, blnbf, bproj_row, ndim, tag, wscale=1.0,
                      paired=False, want_bf=True):
            """bias' = b_ln @ W' + b_proj (undoing wscale) -> [1,ndim] f32/bf16."""
            pst = ps["S"].tile([1, ndim], F32, tag="sm", name="foldps")
            if paired:
                flat = []
                for wt in w_tiles:
                    flat.append(wt[:, 0, :])
                    flat.append(wt[:, 1, :])
            else:
                flat = w_tiles
            nk = len(flat)
            for kt, wt in enumerate(flat):
                nc.tensor.matmul(out=pst, lhsT=blnbf[:, kt:kt + 1], rhs=wt,
                                 start=(kt == 0), stop=(kt == nk - 1))
            row = pool.tile([1, ndim], F32, tag="foldtmp", bufs=1)
            if wscale != 1.0:
                nc.vector.scalar_tensor_tensor(out=row, in0=pst,
                                               scalar=1.0 / wscale,
                                               in1=bproj_row, op0=OP.mult,
                                               op1=OP.add)
            else:
                nc.vector.tensor_tensor(out=row, in0=pst, in1=bproj_row,
                                        op=OP.add)
            rowbf = None
            if want_bf:
                rowbf = pool.tile([1, ndim], BF16, tag=tag + "b")
                nc.vector.tensor_copy(out=rowbf, in_=row)
            return row, rowbf

        def row_to_fm(pool, row, n, tag):
            col = pool.tile([P, n // P], F32, tag=tag)
            for kt in range(n // P):
                pt = ps["S"].tile([P, 1], F32, tag="sm", name="r2fps")
                nc.tensor.transpose(out=pt, in_=row[:, kt * P:(kt + 1) * P],
                                    identity=ident_f[0:1, 0:1])
                nc.scalar.copy(out=col[:, kt:kt + 1], in_=pt)
            return col

        # ---------------- LN stats ----------------
        RSQRT_MAGIC = 0x5F3759DF
        magic_t = const.tile([P, 1], dt.uint32)
        nc.vector.memset(magic_t, RSQRT_MAGIC)

        def ln_stats(src_tile, nt, tag, engine="dve", bufs=None):
            if bufs is None:
                _sv = small.tile
            else:
                def _sv(shape, dtype, tag=None):
                    return small.tile(shape, dtype, tag=tag, bufs=bufs,
                                      name="ln" + (tag or "t"))
            """-> (stats, rstd [P,nt], neg_ms [P,nt] = -mu*rstd).

            engine="act": rstd = exp(-0.5*ln(var+eps)) on ACT (needs nle table).
            engine="dve": Newton rsqrt entirely on DVE (table-free)."""
            stats = _sv([P, nt, 2], F32, tag=tag + "st")
            for tt in range(nt):
                bstat = _sv([P, 6], F32, tag=tag + "bn")
                nc.vector.bn_stats(out=bstat, in_=src_tile[:, tt, :])
                nc.vector.bn_aggr(out=stats[:, tt, :], in_=bstat)
            rstd = _sv([P, nt], F32, tag=tag + "rs")
            if engine == "act":
                lnv = _sv([P, nt], F32, tag=tag + "lv")
                nc.scalar.activation(out=lnv, in_=stats[:, :, 1],
                                     func=AF.Ln, bias=eps_t)
                nc.scalar.activation(out=rstd, in_=lnv, func=AF.Exp, scale=-0.5)
            else:
                ve = _sv([P, nt], F32, tag=tag + "ve")
                nc.vector.tensor_scalar(out=ve, in0=stats[:, :, 1],
                                        scalar1=EPS, scalar2=None, op0=OP.add)
                sh = _sv([P, nt], dt.uint32, tag=tag + "sh")
                nc.vector.tensor_scalar(out=sh, in0=ve[:].bitcast(dt.uint32),
                                        scalar1=1, scalar2=None,
                                        op0=OP.logical_shift_right)
                mg = bass.AP(tensor=magic_t.tensor, offset=magic_t[:].offset,
                             ap=[magic_t[:].ap[0], [0, nt]])
                nc.vector.tensor_tensor(out=rstd[:].bitcast(dt.uint32),
                                        in0=mg, in1=sh, op=OP.subtract)
                for _ in range(2):
                    t1 = _sv([P, nt], F32, tag=tag + "t1")
                    nc.vector.tensor_tensor(out=t1, in0=rstd, in1=rstd,
                                            op=OP.mult)
                    nc.vector.tensor_tensor(out=t1, in0=t1, in1=ve, op=OP.mult)
                    nc.vector.tensor_scalar(out=t1, in0=t1, scalar1=-0.5,
                                            scalar2=1.5, op0=OP.mult, op1=OP.add)
                    nc.vector.tensor_tensor(out=rstd, in0=rstd, in1=t1,
                                            op=OP.mult)
            neg_ms = _sv([P, nt], F32, tag=tag + "nm")
            nc.vector.tensor_tensor(out=neg_ms, in0=stats[:, :, 0], in1=rstd,
                                    op=OP.mult)
            nc.vector.tensor_scalar(out=neg_ms, in0=neg_ms, scalar1=-1.0,
                                    scalar2=None, op0=OP.mult)
            return stats, rstd, neg_ms

        def make_z(src_tile, nt, st, dtype=BF16, tag="z", pool=None):
            """normalize (or plain convert) src [P,nt,512] -> z [P,nt,512] dtype.
            st = (stats, rstd, neg_ms) from ln_stats, or None for plain copy.
            Splits tt's between DVE and ACT."""
            pool = pool or zpool
            z = pool.tile([P, nt, D], dtype, tag=tag + ("8" if dtype == FP8 else ""))
            for tt in range(nt):
                if st is None:
                    if tt % 2 == 0:
                        nc.vector.tensor_copy(out=z[:, tt, :],
                                              in_=src_tile[:, tt, :])
                    else:
                        nc.scalar.copy(out=z[:, tt, :], in_=src_tile[:, tt, :])
                else:
                    stats, rstd, neg_ms = st
                    if tt % 2 == 0:
                        nc.vector.tensor_scalar(out=z[:, tt, :],
                                                in0=src_tile[:, tt, :],
                                                scalar1=stats[:, tt, 0:1],
                                                scalar2=rstd[:, tt:tt + 1],
                                                op0=OP.subtract, op1=OP.mult)
                    else:
                        nc.scalar.activation(out=z[:, tt, :],
                                             in_=src_tile[:, tt, :],
                                             func=AF.Identity,
                                             bias=neg_ms[:, tt:tt + 1],
                                             scale=rstd[:, tt:tt + 1])
            return z

        identities = {BF16: ident_bf, F32: ident_f}
        if fp8_ffn or fp8_qkv or fp8_out:
            ident_f8 = const.tile([P, P], FP8)
            make_identity(nc, ident_f8)
            identities[FP8] = ident_f8

        def transpose_fm(z, nt, pool, tag, dtype=BF16):
            """z [P,nt,D] -> fm [P, FT, nt*P] via PE transposes.

            fp8 out: the PE transpose itself runs in bf16 (hw fp8-transpose
            needs a packed output layout); the PSUM->SBUF copy converts."""
            tdt = BF16 if dtype == FP8 else dtype
            ident = identities[tdt]
            fm = pool.tile([P, FT, nt * P], dtype,
                           tag=tag + ("8" if dtype == FP8 else ""))
            ngrp = (nt + 3) // 4
            for ft in range(FT):
                for g in range(ngrp):
                    n_in = min(4, nt - g * 4)
                    pt = ps["B"].tile([P, 512], tdt, tag="tr", name="trps")
                    for i in range(n_in):
                        tt = g * 4 + i
                        nc.tensor.transpose(out=pt[:, i * P:(i + 1) * P],
                                            in_=z[:, tt, ft * P:(ft + 1) * P],
                                            identity=ident)
                    if ft % 2 == 0:
                        nc.vector.tensor_copy(
                            out=fm[:, ft, g * 512:g * 512 + n_in * P],
                            in_=pt[:, 0:n_in * P])
                    else:
                        nc.scalar.copy(
                            out=fm[:, ft, g * 512:g * 512 + n_in * P],
                            in_=pt[:, 0:n_in * P])
            return fm

        # ---------------- stylization vectors (silu table) ----------------
        AC = {}
        with tc.tile_pool(name="embp", bufs=1) as embp, \
             tc.tile_pool(name="psSe", bufs=2, space="PSUM") as _psSe:
            ps["S"] = _psSe
            e_sb = embp.tile([BL, TE], F32)
            nc.sync.dma_start(out=e_sb, in_=emb_d[:])
            semb = embp.tile([P, FT, BL], BF16)     # silu(emb)^T fm
            for kt in range(FT):
                pt = ps["S"].tile([P, BL], F32, tag="sm", name="embtr")
                nc.tensor.transpose(out=pt, in_=e_sb[:, kt * P:(kt + 1) * P],
                                    identity=ident_f[0:BL, 0:BL])
                act_reg(nc.scalar.activation(out=semb[:, kt, :], in_=pt,
                                             func=AF.Silu))
            for blk in ("sa", "ca", "ffn"):
                ew = load_w_fm(embp, W[blk + "_emb_w"][:], TE, 2 * D, tag="ew")
                eb_row = load_row(embp, W[blk + "_emb_b"][:], 2 * D, tag="ebr")
                eb_bf = embp.tile([1, 2 * D], BF16, tag="ebb")
                nc.vector.tensor_copy(out=eb_bf, in_=eb_row)
                pnb = load_fm_vec(embp, W[blk + "_pnorm_b"][:], D, tag="pnb")
                e_full = embp.tile([BL, 2 * D], F32, tag="ef")
                for half in range(2):
                    pse = ps["S"].tile([BL, 512], F32, tag="sm", name="embmm")
                    nc.tensor.matmul(out=pse, lhsT=ones_row_bf[:, 0:BL],
                                     rhs=eb_bf[:, half * 512:(half + 1) * 512],
                                     start=True, stop=False)
                    for kt in range(FT):
                        nc.tensor.matmul(out=pse, lhsT=semb[:, kt, :],
                                         rhs=ew[kt][:, half * 512:(half + 1) * 512],
                                         start=False, stop=(kt == FT - 1))
                    nc.scalar.copy(out=e_full[:, half * 512:(half + 1) * 512],
                                   in_=pse)
                A = vecp.tile([P, FT, BL], F32, tag=blk + "A")
                Cs = vecp.tile([P, FT, BL], F32, tag=blk + "C")
                sh = embp.tile([P, FT, BL], F32, tag="sh")
                for kt in range(FT):
                    pt = ps["S"].tile([P, BL], F32, tag="sm", name="embtr")
                    nc.tensor.transpose(out=pt, in_=e_full[:, kt * P:(kt + 1) * P],
                                        identity=ident_f[0:BL, 0:BL])
                    nc.vector.tensor_scalar(out=A[:, kt, :], in0=pt, scalar1=1.0,
                                            scalar2=None, op0=OP.add)
                    pt2 = ps["S"].tile([P, BL], F32, tag="sm", name="embtr2")
                    nc.tensor.transpose(out=pt2,
                                        in_=e_full[:, D + kt * P:D + (kt + 1) * P],
                                        identity=ident_f[0:BL, 0:BL])
                    nc.scalar.copy(out=sh[:, kt, :], in_=pt2)
                pnb_b = bass.AP(tensor=pnb.tensor, offset=pnb[:].offset,
                                ap=[pnb[:].ap[0], pnb[:].ap[1], [0, BL]])
                nc.vector.tensor_tensor(out=Cs, in0=A, in1=pnb_b, op=OP.mult)
                nc.vector.tensor_tensor(out=Cs, in0=Cs, in1=sh, op=OP.add)
                AC[blk] = (A, Cs)
        act_break()

        # ---------------- load x (TM), bf16 residual stream ----------------
        x_sb = []
        for b in range(BL):
            xt = xpool.tile([P, NT, D], BF16, tag=f"x{b}")
            for tt in range(NT):
                nc.gpsimd.dma_start(out=xt[:, tt, :],
                                    in_=x_d[b, tt * P:(tt + 1) * P, :])
            x_sb.append(xt)

        # ================= shared attention (SA/CA) =================
        def attention(b, xhat, wq, wk, wv, qb_fm, vb_bf, kv_fm, nkv,
                      m_sb, maskb, vscale, fp8, dbg=None):
            dr = DR if fp8 else None
            escale = RWS if fp8 else 1.0
            # ---- Q projection (FM out) + exp ----
            expq = qpool.tile([P, FT, T], BF16, tag="expq")
            for mt in range(FT):
                psq0 = ps["A"].tile([P, 512], F32, tag="mm", name="qmm0")
                psq1 = ps["A"].tile([P, 512], F32, tag="mm", name="qmm1")
                if fp8:
                    for p_ in range(2):
                        lhs = wq[p_][:, :, mt * P:(mt + 1) * P]
                        nc.tensor.matmul(out=psq0, lhsT=lhs,
                                         rhs=xhat[:, 2 * p_:2 * p_ + 2, 0:512],
                                         start=(p_ == 0), stop=(p_ == 1),
                                         perf_mode=dr)
                        nc.tensor.matmul(out=psq1, lhsT=lhs,
                                         rhs=xhat[:, 2 * p_:2 * p_ + 2, 512:1024],
                                         start=(p_ == 0), stop=(p_ == 1),
                                         perf_mode=dr)
                else:
                    for kt in range(FT):
                        lhs = wq[kt][:, mt * P:(mt + 1) * P]
                        nc.tensor.matmul(out=psq0, lhsT=lhs,
                                         rhs=xhat[:, kt, 0:512],
                                         start=(kt == 0), stop=(kt == FT - 1))
                        nc.tensor.matmul(out=psq1, lhsT=lhs,
                                         rhs=xhat[:, kt, 512:1024],
                                         start=(kt == 0), stop=(kt == FT - 1))
                for th, psq in ((0, psq0), (1, psq1)):
                    act_reg(nc.scalar.activation(
                        out=expq[:, mt, th * 512:(th + 1) * 512],
                        in_=psq, func=AF.Exp, scale=escale,
                        bias=qb_fm[:, mt:mt + 1]))
            # ---- K/V streamed into einsum1; s_k via mask column ----
            e1a = ps["B"].tile([P, 512], F32, tag="e1", name="e1a")
            e1b = ps["B"].tile([P, 512], F32, tag="e1", name="e1b")
            e1t = (e1a, e1a, e1b, e1b)
            for tt in range(nkv):
                kps = ps["A"].tile([P, 512], F32, tag="mm", name="kps")
                vps = ps["A"].tile([P, 512], F32, tag="mm", name="vps")
                nc.tensor.matmul(out=vps, lhsT=ones_row_bf, rhs=vb_bf,
                                 start=True, stop=False)
                if fp8:
                    for p_ in range(2):
                        lhs = kv_fm[:, 2 * p_:2 * p_ + 2, tt * P:(tt + 1) * P]
                        nc.tensor.matmul(out=kps, lhsT=lhs, rhs=wk[p_],
                                         start=(p_ == 0), stop=(p_ == 1),
                                         perf_mode=dr)
                        nc.tensor.matmul(out=vps, lhsT=lhs, rhs=wv[p_],
                                         start=False, stop=(p_ == 1),
                                         perf_mode=dr)
                else:
                    nkt = len(wk)
                    for kt in range(nkt):
                        lhs = kv_fm[:, kt, tt * P:(tt + 1) * P]
                        nc.tensor.matmul(out=kps, lhsT=lhs, rhs=wk[kt],
                                         start=(kt == 0), stop=(kt == nkt - 1))
                        nc.tensor.matmul(out=vps, lhsT=lhs, rhs=wv[kt],
                                         start=False, stop=(kt == nkt - 1))
                expk = kvp.tile([P, 512], BF16, tag="expk")
                v_ext = kvp.tile([P, FT, 132], BF16, tag="vext")
                if maskb is not None:
                    # mask folds entirely into expk's bias (exp(-80) == 0 to
                    # fp precision): v and the s_k ones-column stay unmasked
                    act_reg(nc.scalar.activation(out=expk, in_=kps, func=AF.Exp,
                                                 scale=escale,
                                                 bias=maskb[:, tt:tt + 1]))
                    if fp8:
                        nc.scalar.activation(out=v_ext[:, :, 0:128], in_=vps,
                                             func=AF.Copy, scale=escale)
                    else:
                        nc.scalar.copy(out=v_ext[:, :, 0:128], in_=vps)
                    nc.vector.memset(v_ext[:, :, 128:129], 1.0)
                else:
                    if fp8:
                        act_reg(nc.scalar.activation(out=expk, in_=kps,
                                                     func=AF.Exp, scale=escale))
                        nc.scalar.activation(out=v_ext[:, :, 0:128], in_=vps,
                                             func=AF.Copy, scale=escale)
                    else:
                        act_reg(nc.scalar.activation(out=expk, in_=kps,
                                                     func=AF.Exp))
                        nc.scalar.copy(out=v_ext[:, :, 0:128], in_=vps)
                    nc.vector.memset(v_ext[:, :, 128:129], 1.0)
                for ft in range(FT):
                    # one accumulation group per BANK: start=True clears the
                    # whole bank's has_written bits, so only the first matmul
                    # into each bank may set it (the ft-odd range then
                    # overwrites-where-clear at tt==0 and accumulates after).
                    off = (ft % 2) * 256
                    nc.tensor.matmul(out=e1t[ft][:, off:off + 129],
                                     lhsT=expk[:, ft * P:(ft + 1) * P],
                                     rhs=v_ext[:, ft, 0:129],
                                     start=(tt == 0 and ft % 2 == 0),
                                     stop=(tt == nkv - 1 and ft % 2 == 1))
            # ---- rsk + att_bd (block-diag per head pair) ----
            rsk = small.tile([P, FT], F32, tag="rsk")
            for ft in range(FT):
                off = (ft % 2) * 256
                nc.vector.reciprocal(out=rsk[:, ft:ft + 1],
                                     in_=e1t[ft][:, off + 128:off + 129])
            att_bd = abp.tile([P, FT, P], BF16, tag="attbd")
            nc.vector.memset(att_bd, 0.0)
            for ft in range(FT):
                off = (ft % 2) * 256
                for r in range(2):
                    s = slice(64 * r, 64 * r + 64)
                    nc.vector.tensor_scalar(
                        out=att_bd[s, ft, s],
                        in0=e1t[ft][s, off + 64 * r:off + 64 * r + 64],
                        scalar1=rsk[s, ft:ft + 1], scalar2=None, op0=OP.mult)
            if dbg is not None:
                tap(dbg + "_attbd", att_bd[:])
                tap(dbg + "_expq", expq[:])
            # ---- einsum2 + s_q + normalize -> y TM ----
            y = ypool.tile([P, NT, D], BF16, tag="y")
            sqps = ps["S"].tile([P, NT, H], F32, tag="sm", name="sqps")
            for tt in range(NT):
                yps = ps["A"].tile([P, 512], F32, tag="mm", name="ymm")
                for ft in range(FT):
                    lhs = expq[:, ft, tt * P:(tt + 1) * P]
                    nc.tensor.matmul(out=yps[:, ft * P:(ft + 1) * P], lhsT=lhs,
                                     rhs=att_bd[:, ft, :], start=True, stop=True)
                    nc.tensor.matmul(out=sqps[:, tt, 2 * ft:2 * ft + 2], lhsT=lhs,
                                     rhs=sel_bf, start=True, stop=True)
                rsq = small.tile([P, H], F32, tag="rsq")
                nc.vector.reciprocal(out=rsq, in_=sqps[:, tt, :])
                nc.vector.tensor_tensor(
                    out=y[:, tt, :].rearrange("p (g d) -> p g d", g=H),
                    in0=yps[:].rearrange("p (g d) -> p g d", g=H),
                    in1=bass.AP(tensor=rsq.tensor, offset=rsq[:].offset,
                                ap=[rsq[:].ap[0], rsq[:].ap[1], [0, DH]]),
                    op=OP.mult)
            return y

        # ---- stylize split: pre (nle table) / post (silu table) ----
        def stylize_pre(y, engine=None):
            if engine is None:
                engine = "dve" if pre_dve else "act"
            st = ln_stats(y, NT, tag="pn", engine=engine)
            zy = make_z(y, NT, st)
            return transpose_fm(zy, NT, zfmp, tag="zfm")

        def stylize_post(blk, b, zfm, wo, ob_bf, fp8, final=False):
            A, Cs = AC[blk]
            sdt = FP8 if fp8 else BF16
            for th in range(2):
                sfm = sfmp.tile([P, FT, 512], sdt, tag="sfm", bufs=2)
                for ft in range(FT):
                    act_reg(nc.scalar.activation(
                        out=sfm[:, ft, :],
                        in_=zfm[:, ft, th * 512:(th + 1) * 512],
                        func=AF.Silu, scale=A[:, ft, b:b + 1],
                        bias=Cs[:, ft, b:b + 1]))
                for ti in range(4):
                    tt = th * 4 + ti
                    ops = ps["A"].tile([P, 512], F32, tag="mm", name="omm")
                    nc.tensor.matmul(out=ops, lhsT=ones_row_bf, rhs=ob_bf,
                                     start=True, stop=False)
                    if final:
                        # last residual: evict in f32 straight to output DMA
                        dst = xop.tile([P, 512], F32, tag="xo", name="xo")
                    else:
                        dst = x_sb[b][:, tt, :]
                    if fp8:
                        for p_ in range(2):
                            nc.tensor.matmul(
                                out=ops,
                                lhsT=sfm[:, 2 * p_:2 * p_ + 2,
                                         ti * P:(ti + 1) * P],
                                rhs=wo[p_], start=False, stop=(p_ == 1),
                                perf_mode=DR)
                        nc.vector.scalar_tensor_tensor(
                            out=dst, in0=ops, scalar=RWS,
                            in1=x_sb[b][:, tt, :], op0=OP.mult, op1=OP.add)
                    else:
                        for ft in range(FT):
                            nc.tensor.matmul(out=ops,
                                             lhsT=sfm[:, ft, ti * P:(ti + 1) * P],
                                             rhs=wo[ft], start=False,
                                             stop=(ft == FT - 1))
                        nc.vector.tensor_tensor(out=dst, in0=ops,
                                                in1=x_sb[b][:, tt, :],
                                                op=OP.add)
                    if final:
                        nc.sync.dma_start(out=out_d[b, tt * P:(tt + 1) * P, :],
                                          in_=dst)

        # ================= SA phase =================
        wdt = FP8 if fp8_qkv else BF16
        wodt = FP8 if fp8_out else BF16
        wsc = WS if fp8_qkv else None
        wosc = WS if fp8_out else None
        with tc.tile_pool(name="psA_sa", bufs=3, space="PSUM") as _pa, \
             tc.tile_pool(name="psB_sa", bufs=2, space="PSUM") as _pb, \
             tc.tile_pool(name="psS_sa", bufs=1, space="PSUM") as _psx, \
             tc.tile_pool(name="wmain", bufs=1) as wmain, \
             tc.tile_pool(name="wrow", bufs=1) as wrow, \
             tc.tile_pool(name="cap", bufs=1) as cap:
            ps["A"], ps["B"], ps["S"] = _pa, _pb, _psx

            def prep_attn(pre):
                """Load+fold one attention block's weights. pre in {sa, ca}."""
                g_fm = load_fm_vec(wrow, W[pre + "_norm_g"][:], D, tag="g")
                bln = load_fm_vec(wrow, W[pre + "_norm_b"][:], D, tag="bln")
                if pre == "ca":
                    tg_fm = load_fm_vec(wrow, W["ca_tnorm_g"][:], L, tag="tg")
                    tbln = load_fm_vec(wrow, W["ca_tnorm_b"][:], L, tag="tbln")
                else:
                    tg_fm, tbln = g_fm, bln
                kvdim = L if pre == "ca" else D
                wq = load_w_fm(wmain, W[pre + "_q_w"][:], D, D, gfm=g_fm,
                               tag="wq", dtype=wdt, scale=wsc, paired=fp8_qkv)
                wk = load_w_fm(wmain, W[pre + "_k_w"][:], kvdim, D, gfm=tg_fm,
                               tag="wk", dtype=wdt, scale=wsc, paired=fp8_qkv)
                wv = load_w_fm(wmain, W[pre + "_v_w"][:], kvdim, D, gfm=tg_fm,
                               tag="wv", dtype=wdt, scale=wsc, paired=fp8_qkv)
                wo = load_w_fm(wmain, W[pre + "_out_w"][:], D, D, tag="wo",
                               dtype=wodt, scale=wosc, paired=fp8_out, bufs=1)
                bln_c = wrow.tile([P, FT], wdt, tag="blnb")
                tbln_c = wrow.tile([P, FT], wdt, tag="tblnb")
                bsc = WS if fp8_qkv else 1.0
                nc.vector.tensor_scalar(out=bln_c, in0=bln, scalar1=bsc,
                                        scalar2=None, op0=OP.mult)
                nc.vector.tensor_scalar(out=tbln_c, in0=tbln, scalar1=bsc,
                                        scalar2=None, op0=OP.mult)
                wsq = (WS * WS) if fp8_qkv else 1.0
                qb_row = load_row(wrow, W[pre + "_q_b"][:], D, tag="rowtmp",
                                  bufs=2)
                vb_row = load_row(wrow, W[pre + "_v_b"][:], D, tag="rowtmp",
                                  bufs=2)
                qb_row2, _ = fold_bias(wrow, wq, bln_c, qb_row, D, tag="qbf",
                                       wscale=wsq, paired=fp8_qkv,
                                       want_bf=False)
                qb_fm = row_to_fm(wrow, qb_row2, D, tag="qbfm")
                ob_row = load_row(wrow, W[pre + "_out_b"][:], D, tag="rowtmp",
                                  bufs=2)
                vb_row2, vb_bf = fold_bias(wrow, wv, tbln_c, vb_row, D,
                                           tag="vbf", wscale=wsq,
                                           paired=fp8_qkv,
                                           want_bf=not fp8_qkv)
                if fp8_qkv:
                    vb_sc = wrow.tile([1, D], BF16, tag="vbfb")
                    nc.vector.tensor_scalar(out=vb_sc, in0=vb_row2, scalar1=WS,
                                            scalar2=None, op0=OP.mult)
                    vb_bf = vb_sc
                ob_bf = wrow.tile([1, D], BF16, tag="obbf")
                if fp8_out:
                    nc.vector.tensor_scalar(out=ob_bf, in0=ob_row, scalar1=WS,
                                            scalar2=None, op0=OP.mult)
                else:
                    nc.vector.tensor_copy(out=ob_bf, in_=ob_row)
                return wq, wk, wv, wo, qb_fm, vb_bf, ob_bf

            # SA x stats: x_sb fresh from DMA, compute during weight prep
            sts = [ln_stats(x_sb[b], NT, tag=f"sxln{b}", bufs=1)
                   for b in range(BL)]
            for pre in ("sa", "ca"):
                wq, wk, wv, wo, qb_fm, vb_bf, ob_bf = prep_attn(pre)
                zfms = []
                ys = {}

                def stage_a(b):
                    if pre == "sa":
                        m_sb = small.tile([P, NT], F32, tag="msb",
                                          name="m_sb")
                        for tt in range(NT):
                            nc.sync.dma_start(out=m_sb[:, tt:tt + 1],
                                              in_=mask_d[b, tt * P:(tt + 1) * P, :])
                        maskb = small.tile([P, NT], F32, tag="mbias",
                                           name="maskb")
                        nc.vector.tensor_scalar(out=maskb, in0=m_sb,
                                                scalar1=-1.0, scalar2=-MASK_NEG,
                                                op0=OP.add, op1=OP.mult)
                        if fp8_qkv:
                            vscale = small.tile([P, NT], F32, tag="msc",
                                                name="vscale")
                            nc.vector.tensor_scalar(out=vscale, in0=m_sb,
                                                    scalar1=RWS, scalar2=None,
                                                    op0=OP.mult)
                        else:
                            vscale = m_sb
                        kv_src, nkv = None, NT
                    else:
                        m_sb = maskb = vscale = None
                        xf_sb = cap.tile([P, NTC, L], F32, tag="xfsb", bufs=1,
                                         name="xf_sb")
                        for tt in range(NTC):
                            nc.sync.dma_start(out=xf_sb[:, tt, :],
                                              in_=xf_d[b, tt * P:(tt + 1) * P, :])
                        tst = ln_stats(xf_sb, NTC, tag="tln")
                        ztn = make_z(xf_sb, NTC, tst, tag="zt", pool=cap)
                        kv_src = transpose_fm(ztn, NTC, cap, tag="tnfm",
                                              dtype=wdt)
                        nkv = NTC
                    zx = make_z(x_sb[b], NT, sts[b])
                    xhat = transpose_fm(zx, NT, fmp, tag="fm", dtype=wdt)
                    if kv_src is None:
                        kv_src = xhat
                    return attention(b, xhat, wq, wk, wv, qb_fm, vb_bf,
                                     kv_src, nkv, m_sb, maskb, vscale,
                                     fp8_qkv, dbg=(pre if b == 0 else None))

                # software-pipelined: stylize_pre(b-1)'s serial DVE stats are
                # emitted after batch b's PE-heavy attention work
                for b in range(BL):
                    ys[b] = stage_a(b)
                    if b == 0:
                        tap(pre + "_y", ys[b][:])
                    if b >= 1:
                        zfms.append(stylize_pre(ys.pop(b - 1)))
                zfms.append(stylize_pre(ys.pop(BL - 1)))
                act_break()   # exps before silus
                nxt = []
                for b in range(BL):
                    stylize_post(pre, b, zfms[b], wo, ob_bf, fp8_out)
                    if pre == "sa":
                        # CA x stats right behind each residual update
                        nxt.append(ln_stats(x_sb[b], NT, tag=f"cxln{b}",
                                            bufs=1))
                    if b == 0:
                        tap("x_after_" + pre, x_sb[b][:])
                act_break()   # silus before next phase's exps
                sts = nxt

        # ================= FFN phase =================
        fdt = FP8 if fp8_ffn else BF16
        fsc = WS if fp8_ffn else None
        with tc.tile_pool(name="ffn_w", bufs=1) as wp, \
             tc.tile_pool(name="gelu_p", bufs=2) as gp, \
             tc.tile_pool(name="psA_f", bufs=2, space="PSUM") as _pa, \
             tc.tile_pool(name="psB_f", bufs=2, space="PSUM") as _pb, \
             tc.tile_pool(name="psyf", bufs=1, space="PSUM") as psyf:
            ps["A"], ps["B"], ps["S"] = _pa, _pb, _pb
            w1 = load_w_fm(wp, W["ffn_w1"][:], D, FF, tag="w1", dtype=fdt,
                           scale=fsc, paired=fp8_ffn)
            w2 = load_w_fm(wp, W["ffn_w2"][:], FF, D, tag="w2", dtype=fdt,
                           scale=fsc, paired=fp8_ffn)
            b1_fm = load_fm_vec(wp, W["ffn_b1"][:], FF, tag="b1")
            b2_row = load_row(wp, W["ffn_b2"][:], D, tag="rowtmp", bufs=1)
            b2_bf = wp.tile([1, D], BF16, tag="b2b")
            if fp8_ffn:
                nc.vector.tensor_scalar(out=b2_bf, in0=b2_row, scalar1=WS,
                                        scalar2=None, op0=OP.mult)
            else:
                nc.vector.tensor_copy(out=b2_bf, in_=b2_row)
            wo = load_w_fm(wp, W["ffn_out_w"][:], D, D, tag="wo", dtype=wodt,
                           scale=wosc, paired=fp8_out)
            ob_row = load_row(wp, W["ffn_out_b"][:], D, tag="rowtmp", bufs=1)
            ob_bf = wp.tile([1, D], BF16, tag="obbf")
            if fp8_out:
                nc.vector.tensor_scalar(out=ob_bf, in0=ob_row, scalar1=WS,
                                        scalar2=None, op0=OP.mult)
            else:
                nc.vector.tensor_copy(out=ob_bf, in_=ob_row)

            zfms = []
            ys = {}
            for b in range(BL):
                zx = make_z(x_sb[b], NT, None, tag="z")
                x_fm = transpose_fm(zx, NT, fmp, tag="fm", dtype=fdt)
                y = ypool.tile([P, NT, D], BF16, tag="y")
                gsc = RWS if fp8_ffn else None
                for th in range(2):
                    yps = [psyf.tile([P, 512], F32, tag=f"yf{i}",
                                     name=f"yf{i}") for i in range(4)]
                    for i in range(4):
                        nc.tensor.matmul(out=yps[i], lhsT=ones_row_bf,
                                         rhs=b2_bf, start=True, stop=False)
                    nmt = FF // P
                    for mt in range(nmt):
                        gps = ps["A"].tile([P, 512], F32, tag="mm", name="gmm")
                        if fp8_ffn:
                            for p_ in range(2):
                                nc.tensor.matmul(
                                    out=gps,
                                    lhsT=w1[p_][:, :, mt * P:(mt + 1) * P],
                                    rhs=x_fm[:, 2 * p_:2 * p_ + 2,
                                             th * 512:(th + 1) * 512],
                                    start=(p_ == 0), stop=(p_ == 1),
                                    perf_mode=DR)
                        else:
                            for kt in range(FT):
                                nc.tensor.matmul(
                                    out=gps, lhsT=w1[kt][:, mt * P:(mt + 1) * P],
                                    rhs=x_fm[:, kt, th * 512:(th + 1) * 512],
                                    start=(kt == 0), stop=(kt == FT - 1))
                        if fp8_ffn:
                            if mt % 2 == 0:
                                gpair = gp.tile([P, 2, 512], FP8, tag="g")
                            act_reg(nc.scalar.activation(
                                out=gpair[:, mt % 2, :], in_=gps, func=AF.Gelu,
                                scale=RWS, bias=b1_fm[:, mt:mt + 1]))
                            if mt % 2 == 1:
                                for i in range(4):
                                    nc.tensor.matmul(
                                        out=yps[i],
                                        lhsT=gpair[:, :, i * P:(i + 1) * P],
                                        rhs=w2[mt // 2], start=False,
                                        stop=(mt == nmt - 1), perf_mode=DR)
                        else:
                            gsb = gp.tile([P, 512], BF16, tag="g")
                            act_reg(nc.scalar.activation(
                                out=gsb, in_=gps, func=AF.Gelu,
                                bias=b1_fm[:, mt:mt + 1]))
                            for i in range(4):
                                nc.tensor.matmul(
                                    out=yps[i], lhsT=gsb[:, i * P:(i + 1) * P],
                                    rhs=w2[mt], start=False,
                                    stop=(mt == nmt - 1))
                    for i in range(4):
                        tt = th * 4 + i
                        if fp8_ffn:
                            if i % 2 == 0:
                                nc.vector.tensor_scalar(out=y[:, tt, :],
                                                        in0=yps[i], scalar1=RWS,
                                                        scalar2=None,
                                                        op0=OP.mult)
                            else:
                                nc.scalar.activation(out=y[:, tt, :],
                                                     in_=yps[i], func=AF.Copy,
                                                     scale=RWS)
                        elif i % 2 == 0:
                            nc.vector.tensor_copy(out=y[:, tt, :], in_=yps[i])
                        else:
                            nc.scalar.copy(out=y[:, tt, :], in_=yps[i])
                if b == 0:
                    tap("ffn_y", y[:])
                ys[b] = y
                if b >= 1:
                    zfms.append(stylize_pre(ys.pop(b - 1), engine="dve"))
            zfms.append(stylize_pre(ys.pop(BL - 1), engine="dve"))
            act_break()   # gelus before final silus
            for b in range(BL):
                stylize_post("ffn", b, zfms[b], wo, ob_bf, fp8_out, final=True)

    nc.compile()
    return nc, tap_tensors
# ======================= runner =======================


def make_runner(nc, n_cores=8):
    from concourse.bass2jax import (_bass_exec_p, install_neuronx_cc_hook,
                                    partition_id_tensor)
    from jax.sharding import Mesh, PartitionSpec
    from jax.experimental.shard_map import shard_map
    install_neuronx_cc_hook()
    partition_name = nc.partition_id_tensor.name if nc.partition_id_tensor else None
    in_names, out_names, out_avals, zero_outs = [], [], [], []
    for alloc in nc.m.functions[0].allocations:
        if not isinstance(alloc, mybir.MemoryLocationSet):
            continue
        name = alloc.memorylocations[0].name
        if alloc.kind == "ExternalInput":
            if name != partition_name:
                in_names.append(name)
        elif alloc.kind == "ExternalOutput":
            out_names.append(name)
            shape = tuple(alloc.tensor_shape)
            dtype = mybir.dt.np(alloc.dtype)
            out_avals.append(jax.core.ShapedArray(shape, dtype))
            zero_outs.append(np.zeros(shape, dtype))
    n_params = len(in_names)
    in_names_full = list(in_names) + out_names + ([partition_name] if partition_name else [])

    def _body(*args):
        operands = list(args)
        if partition_name is not None:
            operands.append(partition_id_tensor())
        return tuple(_bass_exec_p.bind(
            *operands, out_avals=tuple(out_avals), in_names=tuple(in_names_full),
            out_names=tuple(out_names), lowering_input_output_aliases=(),
            sim_require_finite=False, sim_require_nnan=False, nc=nc))

    devices = jax.devices()[:n_cores]
    mesh = Mesh(np.asarray(devices), ("core",))
    in_specs = (PartitionSpec("core"),) * (n_params + len(out_names))
    out_specs = (PartitionSpec("core"),) * len(out_names)
    sharded = jax.jit(shard_map(_body, mesh=mesh, in_specs=in_specs,
                                out_specs=out_specs, check_rep=False),
                      keep_unused=True)

    class Runner:
        def __init__(self):
            self.sharded = sharded
            self.in_names = in_names
            self.out_names = out_names
            self.zero_outs = zero_outs
            self.n_cores = n_cores

        def upload(self, in_maps):
            '''Pre-place inputs on device; returns device arg list.'''
            from jax.sharding import NamedSharding, PartitionSpec
            concat_in = [np.concatenate([np.asarray(in_maps[c][n])
                                         for c in range(self.n_cores)], axis=0)
                         for n in self.in_names]
            concat_zeros = [np.zeros((self.n_cores * z.shape[0], *z.shape[1:]),
                                     z.dtype) for z in self.zero_outs]
            sh = NamedSharding(mesh, PartitionSpec("core"))
            args = [jax.device_put(a, sh) for a in concat_in + concat_zeros]
            jax.block_until_ready(args)
            return args

        def run_dev(self, args):
            outs = sharded(*args)
            jax.block_until_ready(outs)
            return outs

        def __call__(self, in_maps):
            args = self.upload(in_maps)
            outs = self.run_dev(args)
            return [{name: np.asarray(outs[i]).reshape(self.n_cores,
                                                       *self.zero_outs[i].shape)[c]
                     for i, name in enumerate(self.out_names)}
                    for c in range(self.n_cores)]
    return Runner()


# ======================= public entry point =======================
_CACHE = {}
N_CORES = 8
B_FULL = 32
NB = B_FULL // N_CORES


import os
_FP8_FFN = os.environ.get("KERNEL_FP8_FFN", "1") == "1"
_FP8_QKV = os.environ.get("KERNEL_FP8_QKV", "0") == "1"
_FP8_OUT = os.environ.get("KERNEL_FP8_OUT", "0") == "1"


def _get_runner():
    if "runner" not in _CACHE:
        nc, _ = build(n_batch=NB, taps=(), fp8_ffn=_FP8_FFN,
                      fp8_qkv=_FP8_QKV, fp8_out=_FP8_OUT)
        _CACHE["runner"] = make_runner(nc, n_cores=N_CORES)
    return _CACHE["runner"]


def make_in_maps(inputs):
    """Pack the 41 weight tensors into one flat array + shard batch inputs."""
    inp = {k: np.ascontiguousarray(np.asarray(v, dtype=np.float32))
           for k, v in inputs.items()}
    wflat = np.ascontiguousarray(
        np.concatenate([inp[n].ravel() for n in PARAM_SHAPES])[None, :],
        dtype=np.float32)
    in_maps = []
    for c in range(N_CORES):
        sl = slice(c * NB, (c + 1) * NB)
        m = {"inflat": np.concatenate(
            [inp["x"][sl].ravel(), inp["xf"][sl].ravel(),
             inp["emb"][sl].ravel(), inp["src_mask"][sl].ravel()])[None, :],
             "wflat": wflat}
        in_maps.append(m)
    return in_maps


def kernel(**inputs) -> np.ndarray:
    runner = _get_runner()
    res = runner(make_in_maps(inputs))
    out = np.concatenate([res[c]["out"] for c in range(N_CORES)], axis=0)
    return out.astype(np.float32)



# revision 60
# speedup vs baseline: 6.1192x; 1.7921x over previous
"""Trainium2 Bass kernel for nn_LinearTemporalDiffusionTransformerDecoderLayer.

v3 (on top of v2's pass-restructured phases / streamed K/V einsum1):
 - fp8 (e4m3, DoubleRow) FFN GEMMs with x64 weight pre-scale (halves the
   biggest PE load; rel_err 0.0137 vs the 2e-2 gate, deterministic inputs)
 - table-free DVE Newton-rsqrt for every LayerNorm (no natural_log ACT
   table ping-pong) + scheduler-only ordering edges that group exp/silu/gelu
   activations per phase: 69 -> 7 ACT table loads (~2.7us each on HW)
 - software-pipelined batch loop: stylize_pre(b-1)'s serial DVE stats chains
   are emitted after batch b's PE-heavy attention work, so PE stays fed
 - cross-phase LN stats hoisting (CA x-stats emitted right behind each SA
   residual update)
 - weight loads via gpsimd cast-on-DMA (f32->bf16) with parallel whole-row
   DMAs; scaled/fp8 weights convert from a bf16 stage (no f32 stage ping-pong)
 - bf16 residual stream (halves x DMA + SBUF; final residual evicts f32
   straight to the output DMA so the last rounding is not taken)

TimelineSim cost model: 602us/core vs 910us for v2 (HW slope-timing is
tunnel-noise-limited; same-window A/B showed 876us vs ~1420us).

Self-contained: builds and compiles an 8-core SPMD Bass program on first call,
shards the batch dimension (B=32 -> 8 cores x 4), executes via PJRT, and
reassembles the full output.  kernel(**inputs) -> np.ndarray [32, 1024, 512].
"""
import sys
for _p in ("/opt/trn_rl_repo",):
    if _p not in sys.path:
        sys.path.insert(0, _p)
import numpy as np
import jax
import concourse.bass as bass
import concourse.bacc as bacc
import concourse.tile as tile
from concourse import mybir
from concourse.tile_rust import add_dep_helper
from concourse.masks import make_identity
from contextlib import ExitStack

dt = mybir.dt
F32, BF16 = dt.float32, dt.bfloat16
FP8 = dt.float8e4
AF = mybir.ActivationFunctionType
OP = mybir.AluOpType
DR = mybir.MatmulPerfMode.DoubleRow
P = 128
T, D, H, DH = 1024, 512, 8, 64
NCTX, L, TE, FF = 256, 512, 512, 2048
NT, FT = T // P, D // P          # 8 token tiles, 4 feature tiles
NTC = NCTX // P                  # 2 ctx tiles
EPS = 1e-5
MASK_NEG = -80.0
WS = 64.0                        # fp8 weight pre-scale
RWS = 1.0 / WS

PARAM_SHAPES = {
    "sa_norm_g": (D,), "sa_norm_b": (D,),
    "sa_q_w": (D, D), "sa_q_b": (D,), "sa_k_w": (D, D), "sa_k_b": (D,),
    "sa_v_w": (D, D), "sa_v_b": (D,),
    "sa_emb_w": (TE, 2 * D), "sa_emb_b": (2 * D,),
    "sa_pnorm_g": (D,), "sa_pnorm_b": (D,),
    "sa_out_w": (D, D), "sa_out_b": (D,),
    "ca_norm_g": (D,), "ca_norm_b": (D,), "ca_tnorm_g": (L,), "ca_tnorm_b": (L,),
    "ca_q_w": (D, D), "ca_q_b": (D,), "ca_k_w": (L, D), "ca_k_b": (D,),
    "ca_v_w": (L, D), "ca_v_b": (D,),
    "ca_emb_w": (TE, 2 * D), "ca_emb_b": (2 * D,),
    "ca_pnorm_g": (D,), "ca_pnorm_b": (D,),
    "ca_out_w": (D, D), "ca_out_b": (D,),
    "ffn_w1": (D, FF), "ffn_b1": (FF,), "ffn_w2": (FF, D), "ffn_b2": (D,),
    "ffn_emb_w": (TE, 2 * D), "ffn_emb_b": (2 * D,),
    "ffn_pnorm_g": (D,), "ffn_pnorm_b": (D,),
    "ffn_out_w": (D, D), "ffn_out_b": (D,),
}


def build(n_batch=4, taps=(), fp8_ffn=False, fp8_qkv=False, fp8_out=False,
          act_group=True, pre_dve=True):
    BL = n_batch
    nc = bacc.Bacc(None, target_bir_lowering=False, debug=False)
    tap_tensors = {}

    # the 4 sharded activations packed into one dram array (fewer dispatch args)
    n_x, n_xf, n_e, n_m = BL * T * D, BL * NCTX * L, BL * TE, BL * T
    inflat_d = nc.declare_dram_parameter("inflat", [1, n_x + n_xf + n_e + n_m],
                                         F32, isOutput=False)
    o1, o2, o3 = n_x, n_x + n_xf, n_x + n_xf + n_e
    x_d = inflat_d[0, 0:n_x].rearrange("(b t d) -> b t d", b=BL, t=T)
    xf_d = inflat_d[0, o1:o1 + n_xf].rearrange("(b t d) -> b t d", b=BL, t=NCTX)
    emb_d = inflat_d[0, o2:o2 + n_e].rearrange("(b e) -> b e", b=BL)
    mask_d = inflat_d[0, o3:o3 + n_m].rearrange("(b t d) -> b t d", b=BL, t=T)
    # all 41 weight tensors packed into one dram array: per-call dispatch
    # marshaling scales with arg count, so 47 args -> 6
    total_w = sum(int(np.prod(s)) for s in PARAM_SHAPES.values())
    wflat_d = nc.declare_dram_parameter("wflat", [1, total_w], F32,
                                        isOutput=False)
    W = {}
    off = 0
    for name, shp in PARAM_SHAPES.items():
        sz = int(np.prod(shp))
        apw = wflat_d[0, off:off + sz]
        if len(shp) == 2:
            apw = apw.rearrange("(a b) -> a b", a=shp[0])
        W[name] = apw
        off += sz
    out_d = nc.declare_dram_parameter("out", [BL, T, D], F32, isOutput=True)

    with tile.TileContext(nc) as tc, ExitStack() as root:
        const = root.enter_context(tc.tile_pool(name="const", bufs=1))
        small = root.enter_context(tc.tile_pool(name="small", bufs=2))

        xpool = root.enter_context(tc.tile_pool(name="xpool", bufs=1))
        zpool = root.enter_context(tc.tile_pool(name="zpool", bufs=2))
        fmp = root.enter_context(tc.tile_pool(name="fmp", bufs=2))
        qpool = root.enter_context(tc.tile_pool(name="qpool", bufs=2))
        kvp = root.enter_context(tc.tile_pool(name="kvp", bufs=3))
        ypool = root.enter_context(tc.tile_pool(name="ypool", bufs=2))
        zfmp = root.enter_context(tc.tile_pool(name="zfmp", bufs=4))
        sfmp = root.enter_context(tc.tile_pool(name="sfmp", bufs=2))
        abp = root.enter_context(tc.tile_pool(name="abp", bufs=2))
        vecp = root.enter_context(tc.tile_pool(name="vecp", bufs=1))
        xop = root.enter_context(tc.tile_pool(name="xop", bufs=3))
        ps = {}

        # ---- ACT table grouping: scheduler-only ordering edges keep the
        # scalar engine's table-switching funcs (exp/silu/gelu) from
        # interleaving across phases (each switch = ~2.7us table reload) ----
        act_seq = {"prev": None, "cur": []}

        def act_reg(bi):
            if not act_group:
                return bi
            if act_seq["prev"] is not None:
                add_dep_helper(bi.ins, act_seq["prev"], sync=False,
                               reason="act-table-group")
            act_seq["cur"].append(bi)
            return bi

        def act_break():
            if not act_group or not act_seq["cur"]:
                return
            j = nc.scalar.nop()
            for bi in act_seq["cur"]:
                add_dep_helper(j.ins, bi.ins, sync=False,
                               reason="act-table-junction")
            act_seq["prev"] = j.ins
            act_seq["cur"] = []

        def tap(name, ap):
            if name in taps and name not in tap_tensors:
                tdn = "tap_" + name
                td = nc.declare_dram_parameter(tdn, list(ap.shape),
                                               ap.dtype, isOutput=True)
                nc.sync.dma_start(out=td[:], in_=ap)
                tap_tensors[name] = (tdn, tuple(ap.shape))

        # ---------------- constants ----------------
        ident_bf = const.tile([P, P], BF16)
        make_identity(nc, ident_bf)
        ident_f = const.tile([P, P], F32)
        make_identity(nc, ident_f)
        ones_row_bf = const.tile([1, P], BF16)       # K=1 lhsT for row broadcast
        nc.vector.memset(ones_row_bf, 1.0)
        sel_bf = const.tile([P, 2], BF16)            # head-pair selector
        nc.vector.memset(sel_bf, 0.0)
        nc.vector.memset(sel_bf[0:64, 0:1], 1.0)
        nc.vector.memset(sel_bf[64:128, 1:2], 1.0)
        eps_t = const.tile([P, 1], F32)
        nc.vector.memset(eps_t, EPS)

        # ---------------- load helpers ----------------
        def load_fm_vec(pool, ap1d, n=D, tag=None):
            nkt = n // P
            t = pool.tile([P, nkt], F32, tag=tag)
            nc.sync.dma_start(out=t, in_=ap1d.rearrange("(k p) -> p k", p=P))
            return t

        def load_row(pool, ap1d, n, tag=None, bufs=None):
            kw = {} if bufs is None else {"bufs": bufs}
            t = pool.tile([1, n], F32, tag=tag, **kw)
            nc.sync.dma_start(out=t, in_=ap1d.rearrange("(a n) -> a n", a=1))
            return t

        def load_w_fm(pool, wap, kdim, ndim, gfm=None, tag="w", dtype=BF16,
                      scale=None, paired=False, bufs=None):
            """Load weight [kdim, ndim] -> list of FM tiles.

            paired=False: kdim//P tiles of [P, ndim].
            paired=True: kdim//(2P) tiles of [P, 2, ndim] (DoubleRow k-pairs).
            gfm: per-row (input-feature) scale column tile [P, kdim//P].
            scale: extra scalar premultiplier (fp8 range scaling).
            """
            nkt = kdim // P
            tiles = []
            kw = {} if bufs is None else {"bufs": bufs}
            for kt in range(nkt):
                if paired:
                    if kt % 2 == 0:
                        wt = pool.tile([P, 2, ndim], dtype,
                                       tag=f"{tag}{kt // 2}", **kw)
                        tiles.append(wt)
                    dst = tiles[-1][:, kt % 2, :]
                else:
                    wt = pool.tile([P, ndim], dtype, tag=f"{tag}{kt}", **kw)
                    tiles.append(wt)
                    dst = wt
                if gfm is None and scale is None and dtype == BF16:
                    # cast-on-DMA (software DGE): skip the stage+convert hop
                    nc.gpsimd.dma_start(out=dst,
                                        in_=wap[kt * P:(kt + 1) * P, :])
                    continue
                # cast-on-DMA to a bf16 stage, then scale/convert in SBUF.
                # One whole-row DMA per k-tile: parallel DMA fan-out, no
                # per-chunk stage ping-pong.
                stg = pool.tile([P, ndim], BF16, tag=f"wstg{ndim}",
                                name="wstg", bufs=2)
                nc.gpsimd.dma_start(out=stg, in_=wap[kt * P:(kt + 1) * P, :])
                for c0 in range(0, ndim, 1024):
                    cw = min(1024, ndim - c0)
                    eng = nc.vector if (kt + c0 // 1024) % 2 == 0 else nc.scalar
                    if gfm is not None and scale is not None:
                        gs = small.tile([P, 1], F32, tag="gsc")
                        nc.vector.tensor_scalar(out=gs, in0=gfm[:, kt:kt + 1],
                                                scalar1=scale, scalar2=None,
                                                op0=OP.mult)
                        nc.vector.tensor_scalar(out=dst[:, c0:c0 + cw],
                                                in0=stg[:, c0:c0 + cw],
                                                scalar1=gs,
                                                scalar2=None, op0=OP.mult)
                    elif gfm is not None:
                        r = (kt + c0 // 1024) % 3
                        if r == 0:
                            nc.vector.tensor_scalar(out=dst[:, c0:c0 + cw],
                                                    in0=stg[:, c0:c0 + cw],
                                                    scalar1=gfm[:, kt:kt + 1],
                                                    scalar2=None, op0=OP.mult)
                        elif r == 1:
                            nc.scalar.activation(out=dst[:, c0:c0 + cw],
                                                 in_=stg[:, c0:c0 + cw],
                                                 func=AF.Copy,
                                                 scale=gfm[:, kt:kt + 1])
                        else:
                            nc.gpsimd.tensor_scalar(out=dst[:, c0:c0 + cw],
                                                    in0=stg[:, c0:c0 + cw],
                                                    scalar1=gfm[:, kt:kt + 1],
                                                    scalar2=None, op0=OP.mult)
                    elif scale is not None:
                        if eng is nc.vector:
                            nc.vector.tensor_scalar(out=dst[:, c0:c0 + cw],
                                                    in0=stg[:, c0:c0 + cw],
                                                    scalar1=scale, scalar2=None,
                                                    op0=OP.mult)
                        else:
                            nc.scalar.activation(out=dst[:, c0:c0 + cw],
                                                 in_=stg[:, c0:c0 + cw],
                                                 func=AF.Copy,
                                                 scale=scale)
                    else:
                        if eng is nc.vector:
                            nc.vector.tensor_copy(out=dst[:, c0:c0 + cw],
                                                  in_=stg[:, c0:c0 + cw])
                        else:
                            nc.scalar.copy(out=dst[:, c0:c0 + cw],
                                           in_=stg[:, c0:c0 + cw])
            return tiles

        def fold_bias(pool, w_tiles, blnbf, bproj_row, ndim, tag, wscale=1.0,
                      paired=False, want_bf=True):
            """bias' = b_ln @ W' + b_proj (undoing wscale) -> [1,ndim] f32/bf16."""
            pst = ps["S"].tile([1, ndim], F32, tag="sm", name="foldps")
            if paired:
                flat = []
                for wt in w_tiles:
                    flat.append(wt[:, 0, :])
                    flat.append(wt[:, 1, :])
            else:
                flat = w_tiles
            nk = len(flat)
            for kt, wt in enumerate(flat):
                nc.tensor.matmul(out=pst, lhsT=blnbf[:, kt:kt + 1], rhs=wt,
                                 start=(kt == 0), stop=(kt == nk - 1))
            row = pool.tile([1, ndim], F32, tag="foldtmp", bufs=1)
            if wscale != 1.0:
                nc.vector.scalar_tensor_tensor(out=row, in0=pst,
                                               scalar=1.0 / wscale,
                                               in1=bproj_row, op0=OP.mult,
                                               op1=OP.add)
            else:
                nc.vector.tensor_tensor(out=row, in0=pst, in1=bproj_row,
                                        op=OP.add)
            rowbf = None
            if want_bf:
                rowbf = pool.tile([1, ndim], BF16, tag=tag + "b")
                nc.vector.tensor_copy(out=rowbf, in_=row)
            return row, rowbf

        def row_to_fm(pool, row, n, tag):
            col = pool.tile([P, n // P], F32, tag=tag)
            for kt in range(n // P):
                pt = ps["S"].tile([P, 1], F32, tag="sm", name="r2fps")
                nc.tensor.transpose(out=pt, in_=row[:, kt * P:(kt + 1) * P],
                                    identity=ident_f[0:1, 0:1])
                nc.scalar.copy(out=col[:, kt:kt + 1], in_=pt)
            return col

        # ---------------- LN stats ----------------
        RSQRT_MAGIC = 0x5F3759DF
        magic_t = const.tile([P, 1], dt.uint32)
        nc.vector.memset(magic_t, RSQRT_MAGIC)

        def ln_stats(src_tile, nt, tag, engine="dve", bufs=None):
            if bufs is None:
                _sv = small.tile
            else:
                def _sv(shape, dtype, tag=None):
                    return small.tile(shape, dtype, tag=tag, bufs=bufs,
                                      name="ln" + (tag or "t"))
            """-> (stats, rstd [P,nt], neg_ms [P,nt] = -mu*rstd).

            engine="act": rstd = exp(-0.5*ln(var+eps)) on ACT (needs nle table).
            engine="dve": Newton rsqrt entirely on DVE (table-free)."""
            stats = _sv([P, nt, 2], F32, tag=tag + "st")
            for tt in range(nt):
                bstat = _sv([P, 6], F32, tag=tag + "bn")
                nc.vector.bn_stats(out=bstat, in_=src_tile[:, tt, :])
                nc.vector.bn_aggr(out=stats[:, tt, :], in_=bstat)
            rstd = _sv([P, nt], F32, tag=tag + "rs")
            if engine == "act":
                lnv = _sv([P, nt], F32, tag=tag + "lv")
                nc.scalar.activation(out=lnv, in_=stats[:, :, 1],
                                     func=AF.Ln, bias=eps_t)
                nc.scalar.activation(out=rstd, in_=lnv, func=AF.Exp, scale=-0.5)
            else:
                ve = _sv([P, nt], F32, tag=tag + "ve")
                nc.vector.tensor_scalar(out=ve, in0=stats[:, :, 1],
                                        scalar1=EPS, scalar2=None, op0=OP.add)
                sh = _sv([P, nt], dt.uint32, tag=tag + "sh")
                nc.vector.tensor_scalar(out=sh, in0=ve[:].bitcast(dt.uint32),
                                        scalar1=1, scalar2=None,
                                        op0=OP.logical_shift_right)
                mg = bass.AP(tensor=magic_t.tensor, offset=magic_t[:].offset,
                             ap=[magic_t[:].ap[0], [0, nt]])
                nc.vector.tensor_tensor(out=rstd[:].bitcast(dt.uint32),
                                        in0=mg, in1=sh, op=OP.subtract)
                for _ in range(2):
                    t1 = _sv([P, nt], F32, tag=tag + "t1")
                    nc.vector.tensor_tensor(out=t1, in0=rstd, in1=rstd,
                                            op=OP.mult)
                    nc.vector.tensor_tensor(out=t1, in0=t1, in1=ve, op=OP.mult)
                    nc.vector.tensor_scalar(out=t1, in0=t1, scalar1=-0.5,
                                            scalar2=1.5, op0=OP.mult, op1=OP.add)
                    nc.vector.tensor_tensor(out=rstd, in0=rstd, in1=t1,
                                            op=OP.mult)
            neg_ms = _sv([P, nt], F32, tag=tag + "nm")
            nc.vector.tensor_tensor(out=neg_ms, in0=stats[:, :, 0], in1=rstd,
                                    op=OP.mult)
            nc.vector.tensor_scalar(out=neg_ms, in0=neg_ms, scalar1=-1.0,
                                    scalar2=None, op0=OP.mult)
            return stats, rstd, neg_ms

        def make_z(src_tile, nt, st, dtype=BF16, tag="z", pool=None):
            """normalize (or plain convert) src [P,nt,512] -> z [P,nt,512] dtype.
            st = (stats, rstd, neg_ms) from ln_stats, or None for plain copy.
            Splits tt's between DVE and ACT."""
            pool = pool or zpool
            z = pool.tile([P, nt, D], dtype, tag=tag + ("8" if dtype == FP8 else ""))
            for tt in range(nt):
                if st is None:
                    if tt % 2 == 0:
                        nc.vector.tensor_copy(out=z[:, tt, :],
                                              in_=src_tile[:, tt, :])
                    else:
                        nc.scalar.copy(out=z[:, tt, :], in_=src_tile[:, tt, :])
                else:
                    stats, rstd, neg_ms = st
                    if tt % 2 == 0:
                        nc.vector.tensor_scalar(out=z[:, tt, :],
                                                in0=src_tile[:, tt, :],
                                                scalar1=stats[:, tt, 0:1],
                                                scalar2=rstd[:, tt:tt + 1],
                                                op0=OP.subtract, op1=OP.mult)
                    else:
                        nc.scalar.activation(out=z[:, tt, :],
                                             in_=src_tile[:, tt, :],
                                             func=AF.Identity,
                                             bias=neg_ms[:, tt:tt + 1],
                                             scale=rstd[:, tt:tt + 1])
            return z

        identities = {BF16: ident_bf, F32: ident_f}
        if fp8_ffn or fp8_qkv or fp8_out:
            ident_f8 = const.tile([P, P], FP8)
            make_identity(nc, ident_f8)
            identities[FP8] = ident_f8

        def transpose_fm(z, nt, pool, tag, dtype=BF16):
            """z [P,nt,D] -> fm [P, FT, nt*P] via PE transposes.

            fp8 out: the PE transpose itself runs in bf16 (hw fp8-transpose
            needs a packed output layout); the PSUM->SBUF copy converts."""
            tdt = BF16 if dtype == FP8 else dtype
            ident = identities[tdt]
            fm = pool.tile([P, FT, nt * P], dtype,
                           tag=tag + ("8" if dtype == FP8 else ""))
            ngrp = (nt + 3) // 4
            for ft in range(FT):
                for g in range(ngrp):
                    n_in = min(4, nt - g * 4)
                    pt = ps["B"].tile([P, 512], tdt, tag="tr", name="trps")
                    for i in range(n_in):
                        tt = g * 4 + i
                        nc.tensor.transpose(out=pt[:, i * P:(i + 1) * P],
                                            in_=z[:, tt, ft * P:(ft + 1) * P],
                                            identity=ident)
                    if ft % 2 == 0:
                        nc.vector.tensor_copy(
                            out=fm[:, ft, g * 512:g * 512 + n_in * P],
                            in_=pt[:, 0:n_in * P])
                    else:
                        nc.scalar.copy(
                            out=fm[:, ft, g * 512:g * 512 + n_in * P],
                            in_=pt[:, 0:n_in * P])
            return fm

        # ---------------- stylization vectors (silu table) ----------------
        AC = {}
        with tc.tile_pool(name="embp", bufs=1) as embp, \
             tc.tile_pool(name="psSe", bufs=2, space="PSUM") as _psSe:
            ps["S"] = _psSe
            e_sb = embp.tile([BL, TE], F32)
            nc.sync.dma_start(out=e_sb, in_=emb_d[:])
            semb = embp.tile([P, FT, BL], BF16)     # silu(emb)^T fm
            for kt in range(FT):
                pt = ps["S"].tile([P, BL], F32, tag="sm", name="embtr")
                nc.tensor.transpose(out=pt, in_=e_sb[:, kt * P:(kt + 1) * P],
                                    identity=ident_f[0:BL, 0:BL])
                act_reg(nc.scalar.activation(out=semb[:, kt, :], in_=pt,
                                             func=AF.Silu))
            for blk in ("sa", "ca", "ffn"):
                ew = load_w_fm(embp, W[blk + "_emb_w"][:], TE, 2 * D, tag="ew")
                eb_row = load_row(embp, W[blk + "_emb_b"][:], 2 * D, tag="ebr")
                eb_bf = embp.tile([1, 2 * D], BF16, tag="ebb")
                nc.vector.tensor_copy(out=eb_bf, in_=eb_row)
                pnb = load_fm_vec(embp, W[blk + "_pnorm_b"][:], D, tag="pnb")
                e_full = embp.tile([BL, 2 * D], F32, tag="ef")
                for half in range(2):
                    pse = ps["S"].tile([BL, 512], F32, tag="sm", name="embmm")
                    nc.tensor.matmul(out=pse, lhsT=ones_row_bf[:, 0:BL],
                                     rhs=eb_bf[:, half * 512:(half + 1) * 512],
                                     start=True, stop=False)
                    for kt in range(FT):
                        nc.tensor.matmul(out=pse, lhsT=semb[:, kt, :],
                                         rhs=ew[kt][:, half * 512:(half + 1) * 512],
                                         start=False, stop=(kt == FT - 1))
                    nc.scalar.copy(out=e_full[:, half * 512:(half + 1) * 512],
                                   in_=pse)
                A = vecp.tile([P, FT, BL], F32, tag=blk + "A")
                Cs = vecp.tile([P, FT, BL], F32, tag=blk + "C")
                sh = embp.tile([P, FT, BL], F32, tag="sh")
                for kt in range(FT):
                    pt = ps["S"].tile([P, BL], F32, tag="sm", name="embtr")
                    nc.tensor.transpose(out=pt, in_=e_full[:, kt * P:(kt + 1) * P],
                                        identity=ident_f[0:BL, 0:BL])
                    nc.vector.tensor_scalar(out=A[:, kt, :], in0=pt, scalar1=1.0,
                                            scalar2=None, op0=OP.add)
                    pt2 = ps["S"].tile([P, BL], F32, tag="sm", name="embtr2")
                    nc.tensor.transpose(out=pt2,
                                        in_=e_full[:, D + kt * P:D + (kt + 1) * P],
                                        identity=ident_f[0:BL, 0:BL])
                    nc.scalar.copy(out=sh[:, kt, :], in_=pt2)
                pnb_b = bass.AP(tensor=pnb.tensor, offset=pnb[:].offset,
                                ap=[pnb[:].ap[0], pnb[:].ap[1], [0, BL]])
                nc.vector.tensor_tensor(out=Cs, in0=A, in1=pnb_b, op=OP.mult)
                nc.vector.tensor_tensor(out=Cs, in0=Cs, in1=sh, op=OP.add)
                AC[blk] = (A, Cs)
        act_break()

        # ---------------- load x (TM), bf16 residual stream ----------------
        x_sb = []
        for b in range(BL):
            xt = xpool.tile([P, NT, D], BF16, tag=f"x{b}")
            for tt in range(NT):
                nc.gpsimd.dma_start(out=xt[:, tt, :],
                                    in_=x_d[b, tt * P:(tt + 1) * P, :])
            x_sb.append(xt)

        # ================= shared attention (SA/CA) =================
        def attention(b, xhat, wq, wk, wv, qb_fm, vb_bf, kv_fm, nkv,
                      m_sb, maskb, vscale, fp8, dbg=None):
            dr = DR if fp8 else None
            escale = RWS if fp8 else 1.0
            # ---- Q projection (FM out) + exp ----
            expq = qpool.tile([P, FT, T], BF16, tag="expq")
            for mt in range(FT):
                psq0 = ps["A"].tile([P, 512], F32, tag="mm", name="qmm0")
                psq1 = ps["A"].tile([P, 512], F32, tag="mm", name="qmm1")
                if fp8:
                    for p_ in range(2):
                        lhs = wq[p_][:, :, mt * P:(mt + 1) * P]
                        nc.tensor.matmul(out=psq0, lhsT=lhs,
                                         rhs=xhat[:, 2 * p_:2 * p_ + 2, 0:512],
                                         start=(p_ == 0), stop=(p_ == 1),
                                         perf_mode=dr)
                        nc.tensor.matmul(out=psq1, lhsT=lhs,
                                         rhs=xhat[:, 2 * p_:2 * p_ + 2, 512:1024],
                                         start=(p_ == 0), stop=(p_ == 1),
                                         perf_mode=dr)
                else:
                    for kt in range(FT):
                        lhs = wq[kt][:, mt * P:(mt + 1) * P]
                        nc.tensor.matmul(out=psq0, lhsT=lhs,
                                         rhs=xhat[:, kt, 0:512],
                                         start=(kt == 0), stop=(kt == FT - 1))
                        nc.tensor.matmul(out=psq1, lhsT=lhs,
                                         rhs=xhat[:, kt, 512:1024],
                                         start=(kt == 0), stop=(kt == FT - 1))
                for th, psq in ((0, psq0), (1, psq1)):
                    act_reg(nc.scalar.activation(
                        out=expq[:, mt, th * 512:(th + 1) * 512],
                        in_=psq, func=AF.Exp, scale=escale,
                        bias=qb_fm[:, mt:mt + 1]))
            # ---- K/V streamed into einsum1; s_k via mask column ----
            e1a = ps["B"].tile([P, 512], F32, tag="e1", name="e1a")
            e1b = ps["B"].tile([P, 512], F32, tag="e1", name="e1b")
            e1t = (e1a, e1a, e1b, e1b)
            for tt in range(nkv):
                kps = ps["A"].tile([P, 512], F32, tag="mm", name="kps")
                vps = ps["A"].tile([P, 512], F32, tag="mm", name="vps")
                nc.tensor.matmul(out=vps, lhsT=ones_row_bf, rhs=vb_bf,
                                 start=True, stop=False)
                if fp8:
                    for p_ in range(2):
                        lhs = kv_fm[:, 2 * p_:2 * p_ + 2, tt * P:(tt + 1) * P]
                        nc.tensor.matmul(out=kps, lhsT=lhs, rhs=wk[p_],
                                         start=(p_ == 0), stop=(p_ == 1),
                                         perf_mode=dr)
                        nc.tensor.matmul(out=vps, lhsT=lhs, rhs=wv[p_],
                                         start=False, stop=(p_ == 1),
                                         perf_mode=dr)
                else:
                    nkt = len(wk)
                    for kt in range(nkt):
                        lhs = kv_fm[:, kt, tt * P:(tt + 1) * P]
                        nc.tensor.matmul(out=kps, lhsT=lhs, rhs=wk[kt],
                                         start=(kt == 0), stop=(kt == nkt - 1))
                        nc.tensor.matmul(out=vps, lhsT=lhs, rhs=wv[kt],
                                         start=False, stop=(kt == nkt - 1))
                expk = kvp.tile([P, 512], BF16, tag="expk")
                v_ext = kvp.tile([P, FT, 132], BF16, tag="vext")
                if maskb is not None:
                    # mask folds entirely into expk's bias (exp(-80) == 0 to
                    # fp precision): v and the s_k ones-column stay unmasked
                    act_reg(nc.scalar.activation(out=expk, in_=kps, func=AF.Exp,
                                                 scale=escale,
                                                 bias=maskb[:, tt:tt + 1]))
                    if fp8:
                        nc.scalar.activation(out=v_ext[:, :, 0:128], in_=vps,
                                             func=AF.Copy, scale=escale)
                    else:
                        nc.scalar.copy(out=v_ext[:, :, 0:128], in_=vps)
                    nc.vector.memset(v_ext[:, :, 128:129], 1.0)
                else:
                    if fp8:
                        act_reg(nc.scalar.activation(out=expk, in_=kps,
                                                     func=AF.Exp, scale=escale))
                        nc.scalar.activation(out=v_ext[:, :, 0:128], in_=vps,
                                             func=AF.Copy, scale=escale)
                    else:
                        act_reg(nc.scalar.activation(out=expk, in_=kps,
                                                     func=AF.Exp))
                        nc.scalar.copy(out=v_ext[:, :, 0:128], in_=vps)
                    nc.vector.memset(v_ext[:, :, 128:129], 1.0)
                for ft in range(FT):
                    # one accumulation group per BANK: start=True clears the
                    # whole bank's has_written bits, so only the first matmul
                    # into each bank may set it (the ft-odd range then
                    # overwrites-where-clear at tt==0 and accumulates after).
                    off = (ft % 2) * 256
                    nc.tensor.matmul(out=e1t[ft][:, off:off + 129],
                                     lhsT=expk[:, ft * P:(ft + 1) * P],
                                     rhs=v_ext[:, ft, 0:129],
                                     start=(tt == 0 and ft % 2 == 0),
                                     stop=(tt == nkv - 1 and ft % 2 == 1))
            # ---- rsk + att_bd (block-diag per head pair) ----
            rsk = small.tile([P, FT], F32, tag="rsk")
            for ft in range(FT):
                off = (ft % 2) * 256
                nc.vector.reciprocal(out=rsk[:, ft:ft + 1],
                                     in_=e1t[ft][:, off + 128:off + 129])
            att_bd = abp.tile([P, FT, P], BF16, tag="attbd")
            nc.vector.memset(att_bd, 0.0)
            for ft in range(FT):
                off = (ft % 2) * 256
                for r in range(2):
                    s = slice(64 * r, 64 * r + 64)
                    nc.vector.tensor_scalar(
                        out=att_bd[s, ft, s],
                        in0=e1t[ft][s, off + 64 * r:off + 64 * r + 64],
                        scalar1=rsk[s, ft:ft + 1], scalar2=None, op0=OP.mult)
            if dbg is not None:
                tap(dbg + "_attbd", att_bd[:])
                tap(dbg + "_expq", expq[:])
            # ---- einsum2 + s_q + normalize -> y TM ----
            y = ypool.tile([P, NT, D], BF16, tag="y")
            sqps = ps["S"].tile([P, NT, H], F32, tag="sm", name="sqps")
            for tt in range(NT):
                yps = ps["A"].tile([P, 512], F32, tag="mm", name="ymm")
                for ft in range(FT):
                    lhs = expq[:, ft, tt * P:(tt + 1) * P]
                    nc.tensor.matmul(out=yps[:, ft * P:(ft + 1) * P], lhsT=lhs,
                                     rhs=att_bd[:, ft, :], start=True, stop=True)
                    nc.tensor.matmul(out=sqps[:, tt, 2 * ft:2 * ft + 2], lhsT=lhs,
                                     rhs=sel_bf, start=True, stop=True)
                rsq = small.tile([P, H], F32, tag="rsq")
                nc.vector.reciprocal(out=rsq, in_=sqps[:, tt, :])
                nc.vector.tensor_tensor(
                    out=y[:, tt, :].rearrange("p (g d) -> p g d", g=H),
                    in0=yps[:].rearrange("p (g d) -> p g d", g=H),
                    in1=bass.AP(tensor=rsq.tensor, offset=rsq[:].offset,
                                ap=[rsq[:].ap[0], rsq[:].ap[1], [0, DH]]),
                    op=OP.mult)
            return y

        # ---- stylize split: pre (nle table) / post (silu table) ----
        def stylize_pre(y, engine=None):
            if engine is None:
                engine = "dve" if pre_dve else "act"
            st = ln_stats(y, NT, tag="pn", engine=engine)
            zy = make_z(y, NT, st)
            return transpose_fm(zy, NT, zfmp, tag="zfm")

        def stylize_post(blk, b, zfm, wo, ob_bf, fp8, final=False):
            A, Cs = AC[blk]
            sdt = FP8 if fp8 else BF16
            for th in range(2):
                sfm = sfmp.tile([P, FT, 512], sdt, tag="sfm", bufs=2)
                for ft in range(FT):
                    act_reg(nc.scalar.activation(
                        out=sfm[:, ft, :],
                        in_=zfm[:, ft, th * 512:(th + 1) * 512],
                        func=AF.Silu, scale=A[:, ft, b:b + 1],
                        bias=Cs[:, ft, b:b + 1]))
                for ti in range(4):
                    tt = th * 4 + ti
                    ops = ps["A"].tile([P, 512], F32, tag="mm", name="omm")
                    nc.tensor.matmul(out=ops, lhsT=ones_row_bf, rhs=ob_bf,
                                     start=True, stop=False)
                    if final:
                        # last residual: evict in f32 straight to output DMA
                        dst = xop.tile([P, 512], F32, tag="xo", name="xo")
                    else:
                        dst = x_sb[b][:, tt, :]
                    if fp8:
                        for p_ in range(2):
                            nc.tensor.matmul(
                                out=ops,
                                lhsT=sfm[:, 2 * p_:2 * p_ + 2,
                                         ti * P:(ti + 1) * P],
                                rhs=wo[p_], start=False, stop=(p_ == 1),
                                perf_mode=DR)
                        nc.vector.scalar_tensor_tensor(
                            out=dst, in0=ops, scalar=RWS,
                            in1=x_sb[b][:, tt, :], op0=OP.mult, op1=OP.add)
                    else:
                        for ft in range(FT):
                            nc.tensor.matmul(out=ops,
                                             lhsT=sfm[:, ft, ti * P:(ti + 1) * P],
                                             rhs=wo[ft], start=False,
                                             stop=(ft == FT - 1))
                        nc.vector.tensor_tensor(out=dst, in0=ops,
                                                in1=x_sb[b][:, tt, :],
                                                op=OP.add)
                    if final:
                        nc.sync.dma_start(out=out_d[b, tt * P:(tt + 1) * P, :],
                                          in_=dst)

        # ================= SA phase =================
        wdt = FP8 if fp8_qkv else BF16
        wodt = FP8 if fp8_out else BF16
        wsc = WS if fp8_qkv else None
        wosc = WS if fp8_out else None
        with tc.tile_pool(name="psA_sa", bufs=3, space="PSUM") as _pa, \
             tc.tile_pool(name="psB_sa", bufs=2, space="PSUM") as _pb, \
             tc.tile_pool(name="psS_sa", bufs=1, space="PSUM") as _psx, \
             tc.tile_pool(name="wmain", bufs=1) as wmain, \
             tc.tile_pool(name="wrow", bufs=1) as wrow, \
             tc.tile_pool(name="cap", bufs=1) as cap:
            ps["A"], ps["B"], ps["S"] = _pa, _pb, _psx

            def prep_attn(pre):
                """Load+fold one attention block's weights. pre in {sa, ca}."""
                g_fm = load_fm_vec(wrow, W[pre + "_norm_g"][:], D, tag="g")
                bln = load_fm_vec(wrow, W[pre + "_norm_b"][:], D, tag="bln")
                if pre == "ca":
                    tg_fm = load_fm_vec(wrow, W["ca_tnorm_g"][:], L, tag="tg")
                    tbln = load_fm_vec(wrow, W["ca_tnorm_b"][:], L, tag="tbln")
                else:
                    tg_fm, tbln = g_fm, bln
                kvdim = L if pre == "ca" else D
                wq = load_w_fm(wmain, W[pre + "_q_w"][:], D, D, gfm=g_fm,
                               tag="wq", dtype=wdt, scale=wsc, paired=fp8_qkv)
                wk = load_w_fm(wmain, W[pre + "_k_w"][:], kvdim, D, gfm=tg_fm,
                               tag="wk", dtype=wdt, scale=wsc, paired=fp8_qkv)
                wv = load_w_fm(wmain, W[pre + "_v_w"][:], kvdim, D, gfm=tg_fm,
                               tag="wv", dtype=wdt, scale=wsc, paired=fp8_qkv)
                wo = load_w_fm(wmain, W[pre + "_out_w"][:], D, D, tag="wo",
                               dtype=wodt, scale=wosc, paired=fp8_out, bufs=1)
                bln_c = wrow.tile([P, FT], wdt, tag="blnb")
                tbln_c = wrow.tile([P, FT], wdt, tag="tblnb")
                bsc = WS if fp8_qkv else 1.0
                nc.vector.tensor_scalar(out=bln_c, in0=bln, scalar1=bsc,
                                        scalar2=None, op0=OP.mult)
                nc.vector.tensor_scalar(out=tbln_c, in0=tbln, scalar1=bsc,
                                        scalar2=None, op0=OP.mult)
                wsq = (WS * WS) if fp8_qkv else 1.0
                qb_row = load_row(wrow, W[pre + "_q_b"][:], D, tag="rowtmp",
                                  bufs=2)
                vb_row = load_row(wrow, W[pre + "_v_b"][:], D, tag="rowtmp",
                                  bufs=2)
                qb_row2, _ = fold_bias(wrow, wq, bln_c, qb_row, D, tag="qbf",
                                       wscale=wsq, paired=fp8_qkv,
                                       want_bf=False)
                qb_fm = row_to_fm(wrow, qb_row2, D, tag="qbfm")
                ob_row = load_row(wrow, W[pre + "_out_b"][:], D, tag="rowtmp",
                                  bufs=2)
                vb_row2, vb_bf = fold_bias(wrow, wv, tbln_c, vb_row, D,
                                           tag="vbf", wscale=wsq,
                                           paired=fp8_qkv,
                                           want_bf=not fp8_qkv)
                if fp8_qkv:
                    vb_sc = wrow.tile([1, D], BF16, tag="vbfb")
                    nc.vector.tensor_scalar(out=vb_sc, in0=vb_row2, scalar1=WS,
                                            scalar2=None, op0=OP.mult)
                    vb_bf = vb_sc
                ob_bf = wrow.tile([1, D], BF16, tag="obbf")
                if fp8_out:
                    nc.vector.tensor_scalar(out=ob_bf, in0=ob_row, scalar1=WS,
                                            scalar2=None, op0=OP.mult)
                else:
                    nc.vector.tensor_copy(out=ob_bf, in_=ob_row)
                return wq, wk, wv, wo, qb_fm, vb_bf, ob_bf

            # SA x stats: x_sb fresh from DMA, compute during weight prep
            sts = [ln_stats(x_sb[b], NT, tag=f"sxln{b}", bufs=1)
                   for b in range(BL)]
            for pre in ("sa", "ca"):
                wq, wk, wv, wo, qb_fm, vb_bf, ob_bf = prep_attn(pre)
                zfms = []
                ys = {}

                def stage_a(b):
                    if pre == "sa":
                        m_sb = small.tile([P, NT], F32, tag="msb",
                                          name="m_sb")
                        for tt in range(NT):
                            nc.sync.dma_start(out=m_sb[:, tt:tt + 1],
                                              in_=mask_d[b, tt * P:(tt + 1) * P, :])
                        maskb = small.tile([P, NT], F32, tag="mbias",
                                           name="maskb")
                        nc.vector.tensor_scalar(out=maskb, in0=m_sb,
                                                scalar1=-1.0, scalar2=-MASK_NEG,
                                                op0=OP.add, op1=OP.mult)
                        if fp8_qkv:
                            vscale = small.tile([P, NT], F32, tag="msc",
                                                name="vscale")
                            nc.vector.tensor_scalar(out=vscale, in0=m_sb,
                                                    scalar1=RWS, scalar2=None,
                                                    op0=OP.mult)
                        else:
                            vscale = m_sb
                        kv_src, nkv = None, NT
                    else:
                        m_sb = maskb = vscale = None
                        xf_sb = cap.tile([P, NTC, L], F32, tag="xfsb", bufs=1,
                                         name="xf_sb")
                        for tt in range(NTC):
                            nc.sync.dma_start(out=xf_sb[:, tt, :],
                                              in_=xf_d[b, tt * P:(tt + 1) * P, :])
                        tst = ln_stats(xf_sb, NTC, tag="tln")
                        ztn = make_z(xf_sb, NTC, tst, tag="zt", pool=cap)
                        kv_src = transpose_fm(ztn, NTC, cap, tag="tnfm",
                                              dtype=wdt)
                        nkv = NTC
                    zx = make_z(x_sb[b], NT, sts[b])
                    xhat = transpose_fm(zx, NT, fmp, tag="fm", dtype=wdt)
                    if kv_src is None:
                        kv_src = xhat
                    return attention(b, xhat, wq, wk, wv, qb_fm, vb_bf,
                                     kv_src, nkv, m_sb, maskb, vscale,
                                     fp8_qkv, dbg=(pre if b == 0 else None))

                # software-pipelined: stylize_pre(b-1)'s serial DVE stats are
                # emitted after batch b's PE-heavy attention work
                for b in range(BL):
                    ys[b] = stage_a(b)
                    if b == 0:
                        tap(pre + "_y", ys[b][:])
                    if b >= 1:
                        zfms.append(stylize_pre(ys.pop(b - 1)))
                zfms.append(stylize_pre(ys.pop(BL - 1)))
                act_break()   # exps before silus
                nxt = []
                for b in range(BL):
                    stylize_post(pre, b, zfms[b], wo, ob_bf, fp8_out)
                    if pre == "sa":
                        # CA x stats right behind each residual update
                        nxt.append(ln_stats(x_sb[b], NT, tag=f"cxln{b}",
                                            bufs=1))
                    if b == 0:
                        tap("x_after_" + pre, x_sb[b][:])
                act_break()   # silus before next phase's exps
                sts = nxt

        # ================= FFN phase =================
        fdt = FP8 if fp8_ffn else BF16
        fsc = WS if fp8_ffn else None
        with tc.tile_pool(name="ffn_w", bufs=1) as wp, \
             tc.tile_pool(name="gelu_p", bufs=2) as gp, \
             tc.tile_pool(name="psA_f", bufs=2, space="PSUM") as _pa, \
             tc.tile_pool(name="psB_f", bufs=2, space="PSUM") as _pb, \
             tc.tile_pool(name="psyf", bufs=1, space="PSUM") as psyf:
            ps["A"], ps["B"], ps["S"] = _pa, _pb, _pb
            w1 = load_w_fm(wp, W["ffn_w1"][:], D, FF, tag="w1", dtype=fdt,
                           scale=fsc, paired=fp8_ffn)
            w2 = load_w_fm(wp, W["ffn_w2"][:], FF, D, tag="w2", dtype=fdt,
                           scale=fsc, paired=fp8_ffn)
            b1_fm = load_fm_vec(wp, W["ffn_b1"][:], FF, tag="b1")
            b2_row = load_row(wp, W["ffn_b2"][:], D, tag="rowtmp", bufs=1)
            b2_bf = wp.tile([1, D], BF16, tag="b2b")
            if fp8_ffn:
                nc.vector.tensor_scalar(out=b2_bf, in0=b2_row, scalar1=WS,
                                        scalar2=None, op0=OP.mult)
            else:
                nc.vector.tensor_copy(out=b2_bf, in_=b2_row)
            wo = load_w_fm(wp, W["ffn_out_w"][:], D, D, tag="wo", dtype=wodt,
                           scale=wosc, paired=fp8_out)
            ob_row = load_row(wp, W["ffn_out_b"][:], D, tag="rowtmp", bufs=1)
            ob_bf = wp.tile([1, D], BF16, tag="obbf")
            if fp8_out:
                nc.vector.tensor_scalar(out=ob_bf, in0=ob_row, scalar1=WS,
                                        scalar2=None, op0=OP.mult)
            else:
                nc.vector.tensor_copy(out=ob_bf, in_=ob_row)

            zfms = []
            ys = {}
            for b in range(BL):
                zx = make_z(x_sb[b], NT, None, tag="z")
                x_fm = transpose_fm(zx, NT, fmp, tag="fm", dtype=fdt)
                y = ypool.tile([P, NT, D], BF16, tag="y")
                gsc = RWS if fp8_ffn else None
                for th in range(2):
                    yps = [psyf.tile([P, 512], F32, tag=f"yf{i}",
                                     name=f"yf{i}") for i in range(4)]
                    for i in range(4):
                        nc.tensor.matmul(out=yps[i], lhsT=ones_row_bf,
                                         rhs=b2_bf, start=True, stop=False)
                    nmt = FF // P
                    for mt in range(nmt):
                        gps = ps["A"].tile([P, 512], F32, tag="mm", name="gmm")
                        if fp8_ffn:
                            for p_ in range(2):
                                nc.tensor.matmul(
                                    out=gps,
                                    lhsT=w1[p_][:, :, mt * P:(mt + 1) * P],
                                    rhs=x_fm[:, 2 * p_:2 * p_ + 2,
                                             th * 512:(th + 1) * 512],
                                    start=(p_ == 0), stop=(p_ == 1),
                                    perf_mode=DR)
                        else:
                            for kt in range(FT):
                                nc.tensor.matmul(
                                    out=gps, lhsT=w1[kt][:, mt * P:(mt + 1) * P],
                                    rhs=x_fm[:, kt, th * 512:(th + 1) * 512],
                                    start=(kt == 0), stop=(kt == FT - 1))
                        if fp8_ffn:
                            if mt % 2 == 0:
                                gpair = gp.tile([P, 2, 512], FP8, tag="g")
                            act_reg(nc.scalar.activation(
                                out=gpair[:, mt % 2, :], in_=gps, func=AF.Gelu,
                                scale=RWS, bias=b1_fm[:, mt:mt + 1]))
                            if mt % 2 == 1:
                                for i in range(4):
                                    nc.tensor.matmul(
                                        out=yps[i],
                                        lhsT=gpair[:, :, i * P:(i + 1) * P],
                                        rhs=w2[mt // 2], start=False,
                                        stop=(mt == nmt - 1), perf_mode=DR)
                        else:
                            gsb = gp.tile([P, 512], BF16, tag="g")
                            act_reg(nc.scalar.activation(
                                out=gsb, in_=gps, func=AF.Gelu,
                                bias=b1_fm[:, mt:mt + 1]))
                            for i in range(4):
                                nc.tensor.matmul(
                                    out=yps[i], lhsT=gsb[:, i * P:(i + 1) * P],
                                    rhs=w2[mt], start=False,
                                    stop=(mt == nmt - 1))
                    for i in range(4):
                        tt = th * 4 + i
                        if fp8_ffn:
                            if i % 2 == 0:
                                nc.vector.tensor_scalar(out=y[:, tt, :],
                                                        in0=yps[i], scalar1=RWS,
                                                        scalar2=None,
                                                        op0=OP.mult)
                            else:
                                nc.scalar.activation(out=y[:, tt, :],
                                                     in_=yps[i], func=AF.Copy,
                                                     scale=RWS)
                        elif i % 2 == 0:
                            nc.vector.tensor_copy(out=y[:, tt, :], in_=yps[i])
                        else:
                            nc.scalar.copy(out=y[:, tt, :], in_=yps[i])
                if b == 0:
                    tap("ffn_y", y[:])
                ys[b] = y
                if b >= 1:
                    zfms.append(stylize_pre(ys.pop(b - 1), engine="dve"))
            zfms.append(stylize_pre(ys.pop(BL - 1), engine="dve"))
            act_break()   # gelus before final silus
            for b in range(BL):
                stylize_post("ffn", b, zfms[b], wo, ob_bf, fp8_out, final=True)

    nc.compile()
    return nc, tap_tensors
# ======================= runner =======================


def make_runner(nc, n_cores=8):
    from concourse.bass2jax import (_bass_exec_p, install_neuronx_cc_hook,
                                    partition_id_tensor)
    from jax.sharding import Mesh, PartitionSpec
    from jax.experimental.shard_map import shard_map
    install_neuronx_cc_hook()
    partition_name = nc.partition_id_tensor.name if nc.partition_id_tensor else None
    in_names, out_names, out_avals, zero_outs = [], [], [], []
    for alloc in nc.m.functions[0].allocations:
        if not isinstance(alloc, mybir.MemoryLocationSet):
            continue
        name = alloc.memorylocations[0].name
        if alloc.kind == "ExternalInput":
            if name != partition_name:
                in_names.append(name)
        elif alloc.kind == "ExternalOutput":
            out_names.append(name)
            shape = tuple(alloc.tensor_shape)
            dtype = mybir.dt.np(alloc.dtype)
            out_avals.append(jax.core.ShapedArray(shape, dtype))
            zero_outs.append(np.zeros(shape, dtype))
    n_params = len(in_names)
    in_names_full = list(in_names) + out_names + ([partition_name] if partition_name else [])

    def _body(*args):
        operands = list(args)
        if partition_name is not None:
            operands.append(partition_id_tensor())
        return tuple(_bass_exec_p.bind(
            *operands, out_avals=tuple(out_avals), in_names=tuple(in_names_full),
            out_names=tuple(out_names), lowering_input_output_aliases=(),
            sim_require_finite=False, sim_require_nnan=False, nc=nc))

    devices = jax.devices()[:n_cores]
    mesh = Mesh(np.asarray(devices), ("core",))
    in_specs = (PartitionSpec("core"),) * (n_params + len(out_names))
    out_specs = (PartitionSpec("core"),) * len(out_names)
    sharded = jax.jit(shard_map(_body, mesh=mesh, in_specs=in_specs,
                                out_specs=out_specs, check_rep=False),
                      keep_unused=True)

    class Runner:
        def __init__(self):
            self.sharded = sharded
            self.in_names = in_names
            self.out_names = out_names
            self.zero_outs = zero_outs
            self.n_cores = n_cores

        def upload(self, in_maps):
            '''Pre-place inputs on device; returns device arg list.'''
            from jax.sharding import NamedSharding, PartitionSpec
            concat_in = [np.concatenate([np.asarray(in_maps[c][n])
                                         for c in range(self.n_cores)], axis=0)
                         for n in self.in_names]
            concat_zeros = [np.zeros((self.n_cores * z.shape[0], *z.shape[1:]),
                                     z.dtype) for z in self.zero_outs]
            sh = NamedSharding(mesh, PartitionSpec("core"))
            args = [jax.device_put(a, sh) for a in concat_in + concat_zeros]
            jax.block_until_ready(args)
            return args

        def run_dev(self, args):
            outs = sharded(*args)
            jax.block_until_ready(outs)
            return outs

        def __call__(self, in_maps):
            args = self.upload(in_maps)
            outs = self.run_dev(args)
            return [{name: np.asarray(outs[i]).reshape(self.n_cores,
                                                       *self.zero_outs[i].shape)[c]
                     for i, name in enumerate(self.out_names)}
                    for c in range(self.n_cores)]
    return Runner()


# ======================= public entry point =======================
_CACHE = {}
N_CORES = 8
B_FULL = 32
NB = B_FULL // N_CORES


import os
_FP8_FFN = os.environ.get("KERNEL_FP8_FFN", "1") == "1"
_FP8_QKV = os.environ.get("KERNEL_FP8_QKV", "0") == "1"
_FP8_OUT = os.environ.get("KERNEL_FP8_OUT", "0") == "1"


def _get_runner():
    if "runner" not in _CACHE:
        nc, _ = build(n_batch=NB, taps=(), fp8_ffn=_FP8_FFN,
                      fp8_qkv=_FP8_QKV, fp8_out=_FP8_OUT)
        _CACHE["runner"] = make_runner(nc, n_cores=N_CORES)
    return _CACHE["runner"]


def make_in_maps(inputs):
    """Pack the 41 weight tensors into one flat array + shard batch inputs."""
    inp = {k: np.ascontiguousarray(np.asarray(v, dtype=np.float32))
           for k, v in inputs.items()}
    wflat = np.ascontiguousarray(
        np.concatenate([inp[n].ravel() for n in PARAM_SHAPES])[None, :],
        dtype=np.float32)
    in_maps = []
    for c in range(N_CORES):
        sl = slice(c * NB, (c + 1) * NB)
        m = {"inflat": np.concatenate(
            [inp["x"][sl].ravel(), inp["xf"][sl].ravel(),
             inp["emb"][sl].ravel(), inp["src_mask"][sl].ravel()])[None, :],
             "wflat": wflat}
        in_maps.append(m)
    return in_maps


def kernel(**inputs) -> np.ndarray:
    runner = _get_runner()
    res = runner(make_in_maps(inputs))
    out = np.concatenate([res[c]["out"] for c in range(N_CORES)], axis=0)
    return out.astype(np.float32)

